# revision 21
# baseline (speedup 1.0000x reference)
"""Trainium2 Bass kernel for nn_ConditionalJiTBlock (DiT-style block with
AdaLN modulation, self-attention, cross-attention and SwiGLU FFN).

Sharding: 8 NeuronCores = 4 batch elements x 2 token-halves. Each core
computes its 512 query tokens end-to-end. v2 additions over the baseline:

- All projection/FFN GEMMs run in fp8(e4m3) with perf_mode=DoubleRow
  (2 MACs/cell/cycle): weights are host-prescaled by 64 (pow2) and stored
  pair-interleaved [K/256*128, 2*F]; activations are written on-chip as
  fp8 "pair tiles" [128, 2*T] (feature tiles 2j/2j+1 side by side), so
  every contraction is 4 DR matmuls of K=256 instead of 8 bf16 matmuls.
  The 1/64 de-scale folds into the PSUM-consuming op (ACT scale / DVE
  scalar / affine_then_add scale).
- The AdaLN mods GEMV is sharded 8 ways: every core computes all 4 batch
  elements' mods over 1/8 of the 9216 columns, then an 8-way AllGather
  (147KB) broadcasts them; a per-core one-hot (bsel) selects the core's
  batch row with 4 vector ops. Kills the 18.9MB ada load + 54us of PE.
- Cross-attention K/V are computed for the LOCAL 512 source tokens only
  and pair-exchanged (AllGather over core pairs, 2.1MB) during the
  self-attention phase: both cores then read back both halves into the
  full K/V tiles (identical layout on both cores, so no per-core
  branching is needed).
- Softmax exp is split across engines: even key-tiles use ScalarE Exp,
  odd key-tiles use a DVE Schraudolph approximation (single tensor_scalar
  writing int16 bf16-bits: bits = rint(s*ATT_SCALE*log2e*128 + 16248)),
  halving the ACT-bound stretches of attention.
- Attention scores (K=64 contraction) are emitted half-pair-interleaved
  so the two 64-row-group matmuls run concurrently in the PE array.

Layout: as the baseline - activations feature-major (features on
partitions, tokens free), per-token scalars broadcast via small selector
matmuls, per-feature scalars as per-partition operands. Residual stream
f32; scores/PV bf16; projections fp8.
"""

import numpy as np
import ml_dtypes

BF16 = ml_dtypes.bfloat16
F8 = ml_dtypes.float8_e4m3

B, N, M, D, H, HD = 4, 1024, 1024, 1024, 16, 64
MH = 2730
MHP = 2816          # MH padded to 22*128
EPS = 1e-6
NCORES = 8
T = 512             # local query tokens per core
DT = D // 128       # 8
KP = DT // 2        # 4 contraction k-pairs for D
FHT = MHP // 128    # 22
FHP = FHT // 2      # 11
NMOD = 9
ASH = NMOD * D // NCORES  # 1152 ada columns per core
ATT_SCALE = HD ** -0.5
WS = 64.0           # fp8 weight pre-scale (pow2)
IWS = 1.0 / WS
LOG2E = 1.4426950408889634
SCHR_A = ATT_SCALE * LOG2E * 128.0
SCHR_B = 16248.0
DVE_KTS = (1, 3, 5, 7)  # key-tiles whose exp runs on DVE (Schraudolph)


# ==========================================================================
# device graph
# ==========================================================================

def build_graph(sim_compat=False):
    import concourse.bacc as bacc
    import concourse.mybir as mybir
    import concourse.tile as tile

    F32 = mybir.dt.float32
    BT = mybir.dt.bfloat16

    nc = bacc.Bacc("TRN2", target_bir_lowering=False, debug=False,
                   num_devices=NCORES)

    def din(name, shape, dtype):
        return nc.dram_tensor(name, shape, dtype, kind="ExternalInput").ap()

    F8D = mybir.dt.float8e4
    p = {}
    # activations
    p["xt"] = din("xt", [D, N], BT)          # x[b].T, local tokens first
    p["xres"] = din("xres", [D, T], F32)     # f32 residual columns (local)
    p["srcp"] = din("srcp", [4 * 128, 2 * T], F8D)  # local src tokens, paired
    p["cmat"] = din("cmat", [D, B], F32)     # c for all batch elements
    p["bsel"] = din("bsel", [128, B], F32)   # one-hot row of this core's b
    p["adash"] = din("adash", [D, ASH], BT)   # ada columns of this core
    # fp8 pair-interleaved weights [K/256*128, 2*F], pre-scaled by WS
    p["wqkv"] = din("wqkv", [512, 2 * 3 * D], F8D)
    p["wo"] = din("wo", [512, 2 * D], F8D)
    p["wcq"] = din("wcq", [512, 2 * D], F8D)
    p["wckv"] = din("wckv", [512, 2 * 2 * D], F8D)
    p["wco"] = din("wco", [512, 2 * D], F8D)
    p["w1"] = din("w1", [512, 2 * MHP], F8D)
    p["w2"] = din("w2", [512, 2 * MHP], F8D)
    p["w3"] = din("w3", [FHP * 128, 2 * D], F8D)
    # feature-major f32 vectors [128, k]  (column j = feature tile j)
    p["adab"] = din("adab", [128, NMOD * DT], F32)
    p["n1w"] = din("n1w", [128, DT], F32)
    p["ncw"] = din("ncw", [128, DT], F32)
    p["n2w"] = din("n2w", [128, DT], F32)
    p["qkvb"] = din("qkvb", [128, 3 * DT], F32)
    p["obf"] = din("obf", [128, DT], F32)    # sa_o_b + v_bias @ Wo (host fold)
    p["cqb"] = din("cqb", [128, DT], F32)
    p["ckb"] = din("ckb", [128, DT], F32)    # cross-k bias
    p["cobf"] = din("cobf", [128, DT], F32)  # ca_o_b + cross-v bias @ Wco
    p["b1f"] = din("b1f", [128, FHT], F32)
    p["b2f"] = din("b2f", [128, FHT], F32)
    p["b3f"] = din("b3f", [128, DT], F32)
    # constant selector matrices, bf16
    p["ones128"] = din("ones128", [128, 128], BT)
    p["bd16"] = din("bd16", [128, 128], BT)
    p["qsel"] = din("qsel", [16, D], BT)
    p["ksel"] = din("ksel", [16, D], BT)
    p["cqsel"] = din("cqsel", [16, D], BT)
    p["cksel"] = din("cksel", [16, D], BT)
    p["rsel2"] = din("rsel2", [2, 128], BT)

    p["out"] = nc.dram_tensor("out", [D, T], F32, kind="ExternalOutput").ap()

    with tile.TileContext(nc) as tc:
        _emit(nc, tc, p, mybir)
    nc.compile()
    return nc


def _emit(nc, tc, p, mybir):
    ALU = mybir.AluOpType
    ACTF = mybir.ActivationFunctionType
    F32 = mybir.dt.float32
    BT = mybir.dt.bfloat16
    F8D = mybir.dt.float8e4
    I16 = mybir.dt.int16
    DR = mybir.MatmulPerfMode.DoubleRow

    pg = tc.alloc_tile_pool(name="pg", bufs=1)
    ps = tc.alloc_tile_pool(name="ps", bufs=8, space="PSUM")
    dram = tc.alloc_tile_pool(name="dram", bufs=1, space="DRAM")

    # shared-tag allocators
    def bigw(name):   # wide bf16 tiles (xt / k / v)
        return pg.tile([128, 1040], BT, tag="bigw", name=name, bufs=26)

    def xf(name):     # f32 [128, T] residual-stream tiles
        return pg.tile([128, T], F32, tag="xf", name=name, bufs=9)

    def qt(name):     # bf16 [128, T] q tiles
        return pg.tile([128, T], BT, tag="qt", name=name, bufs=11)

    def xp(name):     # fp8 pair tiles [128, 2048] (xn1 over N)
        return pg.tile([128, 2 * N], F8D, tag="xp", name=name, bufs=4)

    def sp(name):     # fp8 pair tiles [128, 1024] (T-sized pairs, h, o)
        return pg.tile([128, 2 * T], F8D, tag="sp", name=name, bufs=15)

    def wg8(name):    # fp8 DR weight group tiles [128, 2048]
        return pg.tile([128, 2048], F8D, tag="wg8", name=name, bufs=8)

    def w38(name):    # fp8 DR w3 tiles [128, 1024]
        return pg.tile([128, 1024], F8D, tag="w38", name=name, bufs=11)

    def ptile(name):  # exp(p) tiles
        return pg.tile([128, T], BT, tag="pt", name=name, bufs=13)

    def sqt(name, wid=512):    # square scratch bf16
        return pg.tile([128, wid], BT, tag="sq", name=name, bufs=3)

    def xnb(name):    # bf16 normed-x scratch [128, 1024]
        return pg.tile([128, N], BT, tag="xnb", name=name, bufs=2)

    def scratch4k(name, rows=128, wid=1024):  # f32 scratch (rr/ssq/den)
        return pg.tile([rows, wid], F32, tag="s4k", name=name, bufs=1)

    def scrbf(name, rows=16, wid=1024):
        return pg.tile([rows, wid], BT, tag="sbf", name=name, bufs=1)

    def psum(name):
        return ps.tile([128, 512], F32, tag="ps_n", name=name)

    # ---------------- constants ----------------
    cst = {}
    c_eps = pg.tile([128, 1], F32, tag="c_eps", name="c_eps")
    nc.any.memset(c_eps[:], EPS)
    for nm, k in (("ones128", 128), ("bd16", 128)):
        t = pg.tile([128, k], BT, tag=nm, name=f"c_{nm}")
        nc.sync.dma_start(t[:], p[nm][:])
        cst[nm] = t
    for nm in ("qsel", "ksel", "cqsel", "cksel"):
        t = pg.tile([16, D], BT, tag=nm, name=f"c_{nm}")
        nc.sync.dma_start(t[:], p[nm][:])
        cst[nm] = t
    t = pg.tile([2, 128], BT, tag="rsel2", name="c_rsel2")
    nc.sync.dma_start(t[:], p["rsel2"][:])
    cst["rsel2"] = t
    for nm, k in (("adab", NMOD * DT), ("n1w", DT), ("ncw", DT), ("n2w", DT),
                  ("qkvb", 3 * DT), ("obf", DT), ("cqb", DT), ("ckb", DT),
                  ("cobf", DT), ("b1f", FHT), ("b2f", FHT), ("b3f", DT),
                  ("bsel", B)):
        t = pg.tile([128, k], F32, tag=nm, name=f"c_{nm}")
        nc.sync.dma_start(t[:], p[nm][:])
        cst[nm] = t

    # =====================================================================
    # Stage 0a: sharded ada GEMV + 8-way AllGather of mods.
    # Every core computes mods[all 4 b, its 1152 columns].
    # =====================================================================
    cv = pg.tile([128, DT * B], F32, tag="cv", name="cv")
    nc.scalar.dma_start(cv[:].rearrange("p (k b) -> p k b", k=DT),
                        p["cmat"][:].rearrange("(k p) b -> p k b", p=128))
    scs = pg.tile([128, DT * B], BT, tag="sc", name="scs")
    nc.scalar.activation(scs[:], cv[:], ACTF.Sigmoid)
    nc.vector.tensor_tensor(scs[:], scs[:], cv[:], ALU.mult)

    adat = []
    for k in range(DT):
        t = pg.tile([128, ASH], BT, tag="adat", name=f"adat{k}", bufs=DT)
        nc.scalar.dma_start(t[:], p["adash"][k * 128:(k + 1) * 128, :])
        adat.append(t)
    strip = pg.tile([4, ASH], F32, tag="strip", name="strip")
    for ch in range(3):  # 3 chunks of 384 columns
        pm = psum(f"pm{ch}")
        for k in range(DT):
            nc.tensor.matmul(pm[0:4, 0:384], scs[:, k * B:(k + 1) * B],
                             adat[k][:, ch * 384:(ch + 1) * 384],
                             start=(k == 0), stop=(k == DT - 1))
        nc.vector.tensor_copy(strip[:, ch * 384:(ch + 1) * 384],
                                pm[0:4, 0:384])

    agin = dram.tile([B * ASH], F32, tag="agin", name="agin")
    ago = dram.tile([NCORES * B * ASH], F32, tag="ago", name="ago",
                    addr_space="Shared")
    nc.scalar.dma_start(agin[:].rearrange("(g j) -> g j", g=4), strip[:])
    nc.gpsimd.collective_compute(
        "AllGather", ALU.bypass, replica_groups=[list(range(NCORES))],
        ins=[agin[:]], outs=[ago[:]])

    mods_all = pg.tile([128, B * NMOD * DT], F32, tag="mall", name="mods_all")
    ago4 = ago[:].rearrange("(c g q p) -> c g p q", c=NCORES, g=B, p=128)
    mall4 = mods_all[:].rearrange("p (g c q) -> p g c q", g=B, c=NCORES)
    for cc in range(NCORES):
        for g in range(B):
            nc.scalar.dma_start(mall4[:, g, cc, :], ago4[cc, g])
    ma3 = mods_all[:].rearrange("p (g j) -> p g j", g=B)

    mods = pg.tile([128, NMOD * DT], F32, tag="mods", name="mods")
    nc.vector.tensor_scalar(mods[:], ma3[:, 0], cst["bsel"][:, 0:1], None,
                            ALU.mult)
    for g in range(1, B):
        nc.vector.scalar_tensor_tensor(mods[:], ma3[:, g],
                                       cst["bsel"][:, g:g + 1], mods[:],
                                       ALU.mult, ALU.add)
    nc.vector.tensor_tensor(mods[:], mods[:], cst["adab"][:], ALU.add)

    def msl(i):  # mods columns of modulation param i
        return mods[:, i * DT:(i + 1) * DT]

    seff = {}
    for nm, i_scale, w in (("sa", 1, "n1w"), ("ca", 4, "ncw"), ("ff", 7, "n2w")):
        s1 = pg.tile([128, DT], F32, tag=f"seff_{nm}", name=f"seff_{nm}")
        nc.vector.tensor_scalar(s1[:], msl(i_scale), 1.0, None, ALU.add)
        nc.vector.tensor_tensor(s1[:], s1[:], cst[w][:], ALU.mult)
        seff[nm] = s1
    gb = {}
    g64 = {}
    for nm, i_gate, bias in (("sa", 2, "obf"), ("ca", 5, "cobf"), ("ff", 8, "b3f")):
        t = pg.tile([128, DT], F32, tag=f"gb_{nm}", name=f"gb_{nm}")
        nc.vector.tensor_tensor(t[:], msl(i_gate), cst[bias][:], ALU.mult)
        gb[nm] = t
        t2 = pg.tile([128, DT], F32, tag=f"g64_{nm}", name=f"g64_{nm}")
        nc.vector.tensor_scalar(t2[:], msl(i_gate), IWS, None, ALU.mult)
        g64[nm] = t2
    sh_col = {"sa": 0, "ca": 3, "ff": 6}

    # =====================================================================
    # helpers
    # =====================================================================
    def load_wp(w_ap, cols0, cols, tagname, alloc=wg8):
        """Load DR weight tiles: per k-pair a [128, 2*cols] tile."""
        nkp = w_ap.shape[0] // 128
        tiles = []
        for kp in range(nkp):
            t = alloc(f"{tagname}_{kp}")
            nc.sync.dma_start(
                t[:, 0:2 * cols].rearrange("p (ko m) -> p ko m", ko=2),
                w_ap[kp * 128:(kp + 1) * 128, :]
                .rearrange("p (ko m) -> p ko m", ko=2)[:, :, cols0:cols0 + cols])
            tiles.append(t)
        return tiles

    def norm_mod(xtiles, Ttok, seff_t, sh_slice, name, sq_engine):
        """RMS + AdaLN modulate of feature-major tiles -> fp8 pair tiles."""
        NCH = Ttok // 512
        pss = [psum(f"ssn_{name}{c}") for c in range(NCH)]
        for k in range(DT):
            for c in range(NCH):
                sq = sqt(f"sq_{name}{k}_{c}")
                nc.scalar.activation(sq[:], xtiles[k][:, c * 512:(c + 1) * 512],
                                     ACTF.Square)
                nc.tensor.matmul(pss[c][:], cst["ones128"][:], sq[:],
                                 start=(k == 0), stop=(k == DT - 1))
        rr = scratch4k(f"rr_{name}")
        for c in range(NCH):
            nc.scalar.activation(rr[:, c * 512:(c + 1) * 512], pss[c][:],
                                 ACTF.Sqrt, bias=c_eps[:], scale=1.0 / D)
        nc.vector.reciprocal_approx_fast(rr[:, 0:Ttok], rr[:, 0:Ttok])
        alloc = xp if Ttok == N else sp
        xn = [alloc(f"xn_{name}{j}") for j in range(KP)]
        for k in range(DT):
            t1 = xnb(f"xnb_{name}{k}")
            nc.vector.tensor_tensor(t1[:, 0:Ttok], xtiles[k][:, 0:Ttok],
                                    rr[:, 0:Ttok], ALU.mult)
            half = xn[k // 2][:, (k % 2) * Ttok:(k % 2 + 1) * Ttok]
            nc.vector.tensor_scalar(half, t1[:, 0:Ttok],
                                    seff_t[:, k:k + 1], sh_slice[:, k:k + 1],
                                    ALU.mult, ALU.add)
        return xn

    def qk_norm(qtiles, Ttok, selname, name, sq_eng="gpsimd"):
        """Per-head RMS norm in place; head-norm weight folded into sel."""
        NCH = Ttok // 512
        ssq = scratch4k(f"ssq_{name}", rows=16)
        eng = nc.gpsimd if sq_eng == "gpsimd" else nc.vector
        for c in range(NCH):
            pq = psum(f"psq_{name}{c}")
            for t in range(DT):
                sq = sqt(f"qs_{name}{t}_{c}")
                eng.tensor_tensor(sq[:], qtiles[t][:, c * 512:(c + 1) * 512],
                                  qtiles[t][:, c * 512:(c + 1) * 512],
                                  ALU.mult)
                nc.tensor.matmul(pq[0:16, :],
                                 cst["bd16"][:, t * 16:(t + 1) * 16], sq[:],
                                 start=(t == 0), stop=(t == DT - 1))
            nc.scalar.activation(ssq[:, c * 512:(c + 1) * 512], pq[0:16, :],
                                 ACTF.Sqrt, bias=c_eps[0:16, :], scale=1.0 / HD)
        nc.vector.reciprocal_approx_fast(ssq[:, 0:Ttok], ssq[:, 0:Ttok])
        rqb = scrbf(f"rqb_{name}")
        nc.scalar.activation(rqb[:, 0:Ttok], ssq[:, 0:Ttok], ACTF.Copy)
        for t in range(DT):
            for c in range(NCH):
                pb = psum(f"qb_{name}{t}_{c}")
                nc.tensor.matmul(pb[:], cst[selname][:, t * 128:(t + 1) * 128],
                                 rqb[:, c * 512:(c + 1) * 512],
                                 start=True, stop=True)
                nc.vector.tensor_tensor(qtiles[t][:, c * 512:(c + 1) * 512],
                                        qtiles[t][:, c * 512:(c + 1) * 512],
                                        pb[:], ALU.mult)

    def attention(q_sb, k_sb, v_sb, Tk, name, o_pair):
        """softmax(q k^T / 8) v. Scores are emitted half-pair interleaved
        (concurrent 64-row-group matmuls); exp alternates ACT/DVE per
        (kt, half) so both engines run every step. PV trails two kt steps
        so its operands are always ready and the PE streams back-to-back.
        Per-pair denominator handling (recip + K=2 broadcast matmul) and
        the 1/den scaling writes the fp8 o_pair halves straight from PSUM."""
        KTk = Tk // 128

        def do_pair(t):
            po = [psum(f"po_{name}{2 * t}"), psum(f"po_{name}{2 * t + 1}")]
            pipe = []

            def pv(kt):
                for half in range(2):
                    h16 = 2 * t + half
                    nc.tensor.matmul(po[half][0:65, :],
                                     v_sb[kt][:, h16 * 65:(h16 + 1) * 65],
                                     pipe[kt][half][:],
                                     start=(kt == 0), stop=(kt == KTk - 1),
                                     skip_group_check=True)

            for kt in range(KTk):
                cur = []
                for half in range(2):
                    lo = 64 * half
                    h16 = 2 * t + half
                    s_ps = psum(f"s_{name}{h16}_{kt}")
                    nc.tensor.matmul(
                        s_ps[:], k_sb[t][lo:lo + 64, kt * 128:(kt + 1) * 128],
                        q_sb[t][lo:lo + 64, 0:T], start=True, stop=True)
                    pt = ptile(f"pt_{name}{h16}_{kt}")
                    if (kt + half) % 2 == 1:
                        nc.vector.tensor_scalar(pt[:].bitcast(I16), s_ps[:],
                                                SCHR_A, SCHR_B,
                                                ALU.mult, ALU.add)
                    else:
                        nc.scalar.activation(pt[:], s_ps[:], ACTF.Exp,
                                             scale=ATT_SCALE)
                    cur.append(pt)
                pipe.append(cur)
                if kt >= 2:
                    pv(kt - 2)
            pv(KTk - 2)
            pv(KTk - 1)
            # denominator: 1/row64, broadcast to the 2x64 partition halves.
            # Engine ops need 32-aligned partition bases, so the two strips
            # go through [1,T] tiles + DMA into the base-0 [2,T] tile.
            drow = pg.tile([2, T], F32, tag="drow", name=f"dr_{name}{t}", bufs=2)
            rdb2 = pg.tile([2, T], BT, tag="rdb2", name=f"rb_{name}{t}", bufs=2)
            for half in range(2):
                ds = pg.tile([1, T], F32, tag="dstr", name=f"ds_{name}{t}_{half}",
                             bufs=3)
                nc.vector.tensor_copy(ds[:], po[half][64:65, :])
                nc.sync.dma_start(drow[half:half + 1, :], ds[:])
            nc.vector.reciprocal_approx_fast(drow[:], drow[:])
            nc.scalar.activation(rdb2[:], drow[:], ACTF.Copy)
            osb = qt(f"o_{name}{t}")
            for half in range(2):
                nc.scalar.activation(osb[64 * half:64 * half + 64, :],
                                     po[half][0:64, :], ACTF.Copy)
            pb = psum(f"ob_{name}{t}")
            nc.tensor.matmul(pb[:], cst["rsel2"][:], rdb2[:],
                             start=True, stop=True)
            nc.vector.tensor_tensor(
                o_pair[t // 2][:, (t % 2) * T:(t % 2 + 1) * T],
                osb[:], pb[:], ALU.mult)

        for t in range(DT):
            do_pair(t)
        return o_pair

    def proj_dr(wap, wcols0, xnp, Tt, name, n_f=DT, nkp=KP,
                consume=None):
        """Feature-major DR projection: out f-tiles via 4 K=256 matmuls.
        `consume(f, c, pp)` turns each PSUM chunk into SBUF."""
        NCH = Tt // 512
        for f0 in range(0, n_f, 8):
            nf = min(8, n_f - f0)
            wt = load_wp(wap, wcols0 + f0 * 128, nf * 128, f"{name}_w{f0}")
            for f in range(nf):
                pps = [psum(f"p_{name}{f0 + f}_{c}") for c in range(NCH)]
                for kp in range(nkp):
                    for c in range(NCH):
                        nc.tensor.matmul(
                            pps[c][:],
                            wt[kp][:, 0:2 * nf * 128].rearrange(
                                "p (ko m) -> p ko m", ko=2)[:, :, f * 128:(f + 1) * 128],
                            xnp[kp][:, 0:2 * Tt].rearrange(
                                "p (ko n) -> p ko n", ko=2)[:, :, c * 512:(c + 1) * 512],
                            start=(kp == 0), stop=(kp == nkp - 1),
                            perf_mode=DR, skip_group_check=True)
                for c in range(NCH):
                    consume(f0 + f, c, pps[c])

    def proj_tok_dr(wap, wcols0, xnp, Tt, name, outs):
        """Token-major V projection (DR): stationary = xn pair slices."""
        ntt = Tt // 128
        wt = load_wp(wap, wcols0, D, f"{name}_w")
        for tt in range(ntt):
            pps = [psum(f"pv_{name}{tt}_{c}") for c in range(2)]
            for kp in range(KP):
                for c in range(2):
                    nc.tensor.matmul(
                        pps[c][:],
                        xnp[kp][:, 0:2 * Tt].rearrange(
                            "p (ko n) -> p ko n", ko=2)[:, :, tt * 128:(tt + 1) * 128],
                        wt[kp][:, 0:2 * D].rearrange(
                            "p (ko m) -> p ko m", ko=2)[:, :, c * 512:(c + 1) * 512],
                        start=(kp == 0), stop=(kp == KP - 1),
                        perf_mode=DR, skip_group_check=True)
            for c in range(2):
                dst = outs[tt][:, c * 8 * 65:(c * 8 + 8) * 65].rearrange(
                    "p (g e) -> p g e", g=8)[:, :, 0:64]
                nc.scalar.activation(dst, pps[c][:].rearrange("p (g e) -> p g e", g=8),
                                     ACTF.Copy, scale=IWS)

    # =====================================================================
    # Stage 0b: local cross-attention K/V from the local 512 source
    # tokens, then pair AllGather; both halves are read back into the
    # full-width tiles (same layout on both cores of a pair).
    # =====================================================================
    srcp = []
    for kp in range(KP):
        t = pg.tile([128, 2 * T], F8D, tag="srcp", name=f"srcp{kp}", bufs=KP)
        nc.sync.dma_start(t[:], p["srcp"][kp * 128:(kp + 1) * 128, :])
        srcp.append(t)

    kcaL = [qt(f"kcaL{f}") for f in range(DT)]

    def ckv_consume(f, c, pp):
        nc.scalar.activation(kcaL[f][:], pp[:], ACTF.Identity,
                             bias=cst["ckb"][:, f:f + 1], scale=IWS)

    proj_dr(p["wckv"], 0, srcp, T, "kca", consume=ckv_consume)
    vcaL = []
    for tt in range(T // 128):
        o = bigw(f"vcaL{tt}")
        nc.any.memset(o[:], 1.0)
        vcaL.append(o)
    proj_tok_dr(p["wckv"], D, srcp, T, "vca", vcaL)
    qk_norm(kcaL, T, "cksel", "kca", sq_eng="vector")

    KBYTES = 128 * 512
    VBYTES = 128 * 1040
    kvin = dram.tile([DT * KBYTES + 4 * VBYTES], BT, tag="kvin", name="kvin")
    kvout = dram.tile([2 * (DT * KBYTES + 4 * VBYTES)], BT, tag="kvout",
                      name="kvout")
    for f in range(DT):
        nc.scalar.dma_start(
            kvin[f * KBYTES:(f + 1) * KBYTES].rearrange("(p n) -> p n", p=128),
            kcaL[f][:, 0:512])
    for tt in range(4):
        nc.scalar.dma_start(
            kvin[DT * KBYTES + tt * VBYTES:DT * KBYTES + (tt + 1) * VBYTES]
            .rearrange("(p n) -> p n", p=128), vcaL[tt][:])
    nc.gpsimd.collective_compute(
        "AllGather", ALU.bypass,
        replica_groups=[[2 * i, 2 * i + 1] for i in range(B)],
        ins=[kvin[:]], outs=[kvout[:]])

    # =====================================================================
    # Stage 1: self-attention sublayer
    # =====================================================================
    xt_sb = []
    for k in range(DT):
        t = bigw(f"xt{k}")
        nc.sync.dma_start(t[:, 0:N], p["xt"][k * 128:(k + 1) * 128, :])
        xt_sb.append(t)
    xres_sb = []
    for k in range(DT):
        t = xf(f"xres{k}")
        nc.sync.dma_start(t[:], p["xres"][k * 128:(k + 1) * 128, :])
        xres_sb.append(t)

    xn1 = norm_mod(xt_sb, N, seff["sa"], msl(sh_col["sa"]), "n1", "scalar")
    q_sa = [qt(f"qsa{f}") for f in range(DT)]

    def q_consume(f, c, pp):
        nc.scalar.activation(q_sa[f][:], pp[:], ACTF.Identity,
                             bias=cst["qkvb"][:, f:f + 1], scale=IWS)

    k_sa = [bigw(f"ksa{f}") for f in range(DT)]

    def k_consume(f, c, pp):
        nc.scalar.activation(k_sa[f][:, c * 512:(c + 1) * 512], pp[:],
                             ACTF.Identity,
                             bias=cst["qkvb"][:, DT + f:DT + f + 1], scale=IWS)

    proj_dr(p["wqkv"], 0, xn1, T, "qsa", consume=q_consume)
    proj_dr(p["wqkv"], D, xn1, N, "ksa", consume=k_consume)
    v_sa = []
    for tt in range(N // 128):
        o = bigw(f"vsa{tt}")
        nc.any.memset(o[:], 1.0)
        v_sa.append(o)
    proj_tok_dr(p["wqkv"], 2 * D, xn1, N, "vsa", v_sa)
    qk_norm(q_sa, T, "qsel", "qsa", sq_eng="vector")
    qk_norm(k_sa, N, "ksel", "ksa", sq_eng="gpsimd")

    o1p = [sp(f"o1p{j}") for j in range(KP)]
    attention(q_sa, k_sa, v_sa, N, "a1", o1p)

    x1 = xres_sb

    def o1_consume(f, c, pp):
        nc.vector.affine_then_add(x1[f][:], pp[:], x1[f][:],
                                  g64["sa"][:, f:f + 1], gb["sa"][:, f:f + 1])

    proj_dr(p["wo"], 0, o1p, T, "o1", consume=o1_consume)

    kca = [bigw(f"kca{f}") for f in range(DT)]
    vca = [bigw(f"vca{tt}") for tt in range(8)]
    HALF_OFF = DT * KBYTES + 4 * VBYTES
    for h in range(2):
        for f in range(DT):
            o = h * HALF_OFF + f * KBYTES
            nc.scalar.dma_start(
                kca[f][:, h * 512:(h + 1) * 512],
                kvout[o:o + KBYTES].rearrange("(p n) -> p n", p=128))
        for tt in range(4):
            o = h * HALF_OFF + DT * KBYTES + tt * VBYTES
            nc.scalar.dma_start(
                vca[h * 4 + tt][:, 0:1040],
                kvout[o:o + VBYTES].rearrange("(p n) -> p n", p=128))

    # =====================================================================
    # Stage 2: cross-attention sublayer
    # =====================================================================
    xnc = norm_mod(x1, T, seff["ca"], msl(sh_col["ca"]), "nc", "scalar")
    q_ca = [qt(f"qca{f}") for f in range(DT)]

    def qca_consume(f, c, pp):
        nc.scalar.activation(q_ca[f][:], pp[:], ACTF.Identity,
                             bias=cst["cqb"][:, f:f + 1], scale=IWS)

    proj_dr(p["wcq"], 0, xnc, T, "qca", consume=qca_consume)
    qk_norm(q_ca, T, "cqsel", "qca", sq_eng="gpsimd")
    o2p = [sp(f"o2p{j}") for j in range(KP)]
    attention(q_ca, kca, vca, M, "a2", o2p)

    x2 = x1

    def o2_consume(f, c, pp):
        nc.vector.affine_then_add(x2[f][:], pp[:], x2[f][:],
                                  g64["ca"][:, f:f + 1], gb["ca"][:, f:f + 1])

    proj_dr(p["wco"], 0, o2p, T, "o2", consume=o2_consume)

    # =====================================================================
    # Stage 3: SwiGLU FFN sublayer
    # =====================================================================
    xn2 = norm_mod(x2, T, seff["ff"], msl(sh_col["ff"]), "n2", "scalar")
    h_pair = [sp(f"hp{j}") for j in range(FHP)]
    for f0 in range(0, FHT, 8):
        nf = min(8, FHT - f0)
        w1t = load_wp(p["w1"], f0 * 128, nf * 128, f"w1_{f0}")
        w2t = load_wp(p["w2"], f0 * 128, nf * 128, f"w2_{f0}")
        for f in range(nf):
            fi = f0 + f
            pp1 = psum(f"ph1_{fi}")
            for kp in range(KP):
                nc.tensor.matmul(
                    pp1[:],
                    w1t[kp][:, 0:2 * nf * 128].rearrange(
                        "p (ko m) -> p ko m", ko=2)[:, :, f * 128:(f + 1) * 128],
                    xn2[kp][:].rearrange("p (ko n) -> p ko n", ko=2),
                    start=(kp == 0), stop=(kp == KP - 1), perf_mode=DR)
            h1 = sqt(f"h1_{fi}")
            nc.scalar.activation(h1[:], pp1[:], ACTF.Silu,
                                 bias=cst["b1f"][:, fi:fi + 1], scale=IWS)
            pp2 = psum(f"ph2_{fi}")
            for kp in range(KP):
                nc.tensor.matmul(
                    pp2[:],
                    w2t[kp][:, 0:2 * nf * 128].rearrange(
                        "p (ko m) -> p ko m", ko=2)[:, :, f * 128:(f + 1) * 128],
                    xn2[kp][:].rearrange("p (ko n) -> p ko n", ko=2),
                    start=(kp == 0), stop=(kp == KP - 1), perf_mode=DR)
            h2 = ptile(f"h2_{fi}")
            nc.vector.tensor_scalar(h2[:], pp2[:], IWS,
                                    cst["b2f"][:, fi:fi + 1],
                                    ALU.mult, ALU.add)
            nc.vector.tensor_tensor(
                h_pair[fi // 2][:, (fi % 2) * T:(fi % 2 + 1) * T],
                h1[:], h2[:], ALU.mult)

    # out = h @ w3: 2 groups of 4 feature tiles, 4 live psums each
    for fg in range(0, DT, 4):
        psf = [psum(f"pf{fg + f}") for f in range(4)]
        for kp in range(FHP):
            w3t = w38(f"w3_{fg}_{kp}")
            nc.sync.dma_start(
                w3t[:].rearrange("p (ko m) -> p ko m", ko=2),
                p["w3"][kp * 128:(kp + 1) * 128, :]
                .rearrange("p (ko m) -> p ko m", ko=2)[:, :, fg * 128:(fg + 4) * 128])
            for f in range(4):
                nc.tensor.matmul(
                    psf[f][:],
                    w3t[:].rearrange("p (ko m) -> p ko m", ko=2)[:, :, f * 128:(f + 1) * 128],
                    h_pair[kp][:].rearrange("p (ko n) -> p ko n", ko=2),
                    start=(kp == 0), stop=(kp == FHP - 1), perf_mode=DR)
        for f in range(4):
            xo = x2[fg + f]
            nc.vector.affine_then_add(
                xo[:], psf[f][:], xo[:],
                g64["ff"][:, fg + f:fg + f + 1],
                gb["ff"][:, fg + f:fg + f + 1])
            nc.sync.dma_start(p["out"][(fg + f) * 128:(fg + f + 1) * 128, :], xo[:])

    pg.release()
    ps.release()
    dram.release()


# ==========================================================================
# host side
# ==========================================================================

def _fm(vec):
    """[128*k] f32 vector -> feature-major [128, k] (col j = feature tile j)."""
    v = np.asarray(vec, np.float32)
    return np.ascontiguousarray(v.reshape(-1, 128).T)


def _pair8(W, scale=WS):
    """[K, F] f32 -> DR pair-interleaved fp8 [K/256*128, 2*F], x scale."""
    W = np.asarray(W, np.float32) * scale
    W = np.clip(W, -240.0, 240.0)
    K, F = W.shape
    assert K % 256 == 0
    Wp = W.reshape(K // 256, 2, 128, F).transpose(0, 2, 1, 3).reshape(
        K // 256 * 128, 2 * F)
    return np.ascontiguousarray(Wp).astype(F8)


def _bd16():
    bd = np.zeros((128, 128), np.float32)
    for t in range(8):
        for p_ in range(128):
            bd[p_, t * 16 + 2 * t + p_ // 64] = 1.0
    return bd.astype(BF16)


def _rsel2():
    r = np.zeros((2, 128), np.float32)
    r[0, 0:64] = 1.0
    r[1, 64:128] = 1.0
    return r.astype(BF16)


def _sel(weights64):
    """[16, 1024] selector: sel[i, t*128+p] = w[p%64] * (i == 2t + p//64)."""
    w = np.ones(64, np.float32) if weights64 is None else \
        np.asarray(weights64, np.float32)
    s = np.zeros((16, D), np.float32)
    for col in range(D):
        i = 2 * (col // 128) + (col % 128) // 64
        s[i, col] = w[col % 64]
    return s.astype(BF16)


def make_in_maps(inputs):
    f32 = lambda a: np.ascontiguousarray(np.asarray(a, np.float32))
    bf = lambda a: np.ascontiguousarray(np.asarray(a, np.float32)).astype(BF16)

    x = f32(inputs["x"]); src = f32(inputs["source_tokens"]); c = f32(inputs["c"])
    qkv_b = f32(inputs["sa_qkv_b"])
    o_w = f32(inputs["sa_o_w"]); o_b = f32(inputs["sa_o_b"])
    ckv_b = f32(inputs["ca_kv_b"])
    co_w = f32(inputs["ca_o_w"]); co_b = f32(inputs["ca_o_b"])
    w1 = f32(inputs["mlp_w1"]); b1 = f32(inputs["mlp_b1"])
    w2 = f32(inputs["mlp_w2"]); b2 = f32(inputs["mlp_b2"])
    w3 = f32(inputs["mlp_w3"]); b3 = f32(inputs["mlp_b3"])

    # pad SwiGLU hidden to 2816; zero pads keep silu(0)*0 == 0 exact
    w1p = np.zeros((D, MHP), np.float32); w1p[:, :MH] = w1
    w2p = np.zeros((D, MHP), np.float32); w2p[:, :MH] = w2
    w3p = np.zeros((MHP, D), np.float32); w3p[:MH, :] = w3
    b1p = np.zeros(MHP, np.float32); b1p[:MH] = b1
    b2p = np.zeros(MHP, np.float32); b2p[:MH] = b2

    # fold the V biases through the linear attention + output projection
    obf = qkv_b[2 * D:3 * D] @ o_w + o_b
    cobf = ckv_b[D:2 * D] @ co_w + co_b

    ada_w = f32(inputs["ada_w"])
    shared = dict(
        wqkv=_pair8(inputs["sa_qkv_w"]), wo=_pair8(o_w),
        wcq=_pair8(inputs["ca_q_w"]), wckv=_pair8(inputs["ca_kv_w"]),
        wco=_pair8(co_w),
        w1=_pair8(w1p), w2=_pair8(w2p), w3=_pair8(w3p),
        adab=_fm(f32(inputs["ada_b"])), n1w=_fm(f32(inputs["n1_w"])),
        ncw=_fm(f32(inputs["nc_w"])), n2w=_fm(f32(inputs["n2_w"])),
        qkvb=_fm(qkv_b), obf=_fm(obf), cqb=_fm(f32(inputs["ca_q_b"])),
        ckb=_fm(ckv_b[0:D]), cobf=_fm(cobf),
        b1f=_fm(b1p), b2f=_fm(b2p), b3f=_fm(b3),
        ones128=np.ones((128, 128), BF16),
        bd16=_bd16(),
        qsel=_sel(inputs["sa_qn_w"]), ksel=_sel(inputs["sa_kn_w"]),
        cqsel=_sel(inputs["ca_qn_w"]), cksel=_sel(inputs["ca_kn_w"]),
        rsel2=_rsel2(),
        cmat=np.ascontiguousarray(c.T),
    )

    in_maps = []
    for cidx in range(NCORES):
        b, half = divmod(cidx, 2)
        xT = x[b].T  # [D, N]
        if half:
            xTp = np.concatenate([xT[:, T:], xT[:, :T]], axis=1)
        else:
            xTp = xT
        m = dict(shared)
        m["xt"] = np.ascontiguousarray(xTp).astype(BF16)
        m["xres"] = np.ascontiguousarray(xTp[:, :T])
        # local source tokens, fp8 pair-interleaved [512, 1024]
        sl = src[b].T[:, half * T:(half + 1) * T]  # [D, T]
        sl8 = np.clip(sl, -240, 240).reshape(4, 2, 128, T).transpose(
            0, 2, 1, 3).reshape(512, 2 * T)
        m["srcp"] = np.ascontiguousarray(sl8).astype(F8)
        m["adash"] = np.ascontiguousarray(
            ada_w[:, cidx * ASH:(cidx + 1) * ASH]).astype(BF16)
        bs = np.zeros((128, B), np.float32)
        bs[:, b] = 1.0
        m["bsel"] = bs
        in_maps.append(m)
    return in_maps


def assemble(results):
    out = np.empty((B, N, D), np.float32)
    for cidx in range(NCORES):
        b, half = divmod(cidx, 2)
        out[b, half * T:(half + 1) * T, :] = results[cidx]["out"].T
    return out


_NC_CACHE = []


def kernel(**inputs):
    from concourse.bass_utils import run_bass_kernel_spmd
    if not _NC_CACHE:
        _NC_CACHE.append(build_graph())
    nc = _NC_CACHE[0]
    in_maps = make_in_maps(inputs)
    res = run_bass_kernel_spmd(nc, in_maps, core_ids=list(range(NCORES)))
    return assemble(res.results)


if __name__ == "__main__":
    nc = build_graph()
    print("graph built OK; instructions:",
          sum(len(bb.instructions) for bb in nc.main_func.blocks))


# revision 22
# speedup vs baseline: 1.0245x; 1.0245x over previous
"""Trainium2 Bass kernel for nn_ConditionalJiTBlock (DiT-style block with
AdaLN modulation, self-attention, cross-attention and SwiGLU FFN).

Sharding: 8 NeuronCores = 4 batch elements x 2 token-halves. Each core
computes its 512 query tokens end-to-end. v2 additions over the baseline:

- All projection/FFN GEMMs run in fp8(e4m3) with perf_mode=DoubleRow
  (2 MACs/cell/cycle): weights are host-prescaled by 64 (pow2) and stored
  pair-interleaved [K/256*128, 2*F]; activations are written on-chip as
  fp8 "pair tiles" [128, 2*T] (feature tiles 2j/2j+1 side by side), so
  every contraction is 4 DR matmuls of K=256 instead of 8 bf16 matmuls.
  The 1/64 de-scale folds into the PSUM-consuming op (ACT scale / DVE
  scalar / affine_then_add scale).
- The AdaLN mods GEMV is sharded 8 ways: every core computes all 4 batch
  elements' mods over 1/8 of the 9216 columns, then an 8-way AllGather
  (147KB) broadcasts them; a per-core one-hot (bsel) selects the core's
  batch row with 4 vector ops. Kills the 18.9MB ada load + 54us of PE.
- Cross-attention K/V are computed for the LOCAL 512 source tokens only
  and pair-exchanged (AllGather over core pairs, 2.1MB) during the
  self-attention phase: both cores then read back both halves into the
  full K/V tiles (identical layout on both cores, so no per-core
  branching is needed).
- Softmax exp is split across engines: even key-tiles use ScalarE Exp,
  odd key-tiles use a DVE Schraudolph approximation (single tensor_scalar
  writing int16 bf16-bits: bits = rint(s*ATT_SCALE*log2e*128 + 16248)),
  halving the ACT-bound stretches of attention.
- Attention scores (K=64 contraction) are emitted half-pair-interleaved
  so the two 64-row-group matmuls run concurrently in the PE array.

Layout: as the baseline - activations feature-major (features on
partitions, tokens free), per-token scalars broadcast via small selector
matmuls, per-feature scalars as per-partition operands. Residual stream
f32; scores/PV bf16; projections fp8.
"""

import numpy as np
import ml_dtypes

BF16 = ml_dtypes.bfloat16
F8 = ml_dtypes.float8_e4m3

B, N, M, D, H, HD = 4, 1024, 1024, 1024, 16, 64
MH = 2730
MHP = 2816          # MH padded to 22*128
EPS = 1e-6
NCORES = 8
T = 512             # local query tokens per core
DT = D // 128       # 8
KP = DT // 2        # 4 contraction k-pairs for D
FHT = MHP // 128    # 22
FHP = FHT // 2      # 11
NMOD = 9
ASH = NMOD * D // NCORES  # 1152 ada columns per core
ATT_SCALE = HD ** -0.5
WS = 64.0           # fp8 weight pre-scale (pow2)
IWS = 1.0 / WS
LOG2E = 1.4426950408889634
SCHR_A = ATT_SCALE * LOG2E * 128.0
SCHR_B = 16248.0
DVE_KTS = (1, 3, 5, 7)  # key-tiles whose exp runs on DVE (Schraudolph)


# ==========================================================================
# device graph
# ==========================================================================

def build_graph(sim_compat=False):
    import concourse.bacc as bacc
    import concourse.mybir as mybir
    import concourse.tile as tile

    F32 = mybir.dt.float32
    BT = mybir.dt.bfloat16

    nc = bacc.Bacc("TRN2", target_bir_lowering=False, debug=False,
                   num_devices=NCORES)

    def din(name, shape, dtype):
        return nc.dram_tensor(name, shape, dtype, kind="ExternalInput").ap()

    F8D = mybir.dt.float8e4
    p = {}
    # activations
    p["xt"] = din("xt", [D, N], BT)          # x[b].T, local tokens first
    p["xres"] = din("xres", [D, T], F32)     # f32 residual columns (local)
    p["srcp"] = din("srcp", [4 * 128, 2 * T], F8D)  # local src tokens, paired
    p["cmat"] = din("cmat", [D, B], F32)     # c for all batch elements
    p["bsel"] = din("bsel", [128, B], F32)   # one-hot row of this core's b
    p["adash"] = din("adash", [D, ASH], BT)   # ada columns of this core
    # fp8 pair-interleaved weights [K/256*128, 2*F], pre-scaled by WS
    p["wqkv"] = din("wqkv", [512, 2 * 3 * D], F8D)
    p["wo"] = din("wo", [512, 2 * D], F8D)
    p["wcq"] = din("wcq", [512, 2 * D], F8D)
    p["wckv"] = din("wckv", [512, 2 * 2 * D], F8D)
    p["wco"] = din("wco", [512, 2 * D], F8D)
    p["w1"] = din("w1", [512, 2 * MHP], F8D)
    p["w2"] = din("w2", [512, 2 * MHP], F8D)
    p["w3"] = din("w3", [FHP * 128, 2 * D], F8D)
    # feature-major f32 vectors [128, k]  (column j = feature tile j)
    p["adab"] = din("adab", [128, NMOD * DT], F32)
    p["n1w"] = din("n1w", [128, DT], F32)
    p["ncw"] = din("ncw", [128, DT], F32)
    p["n2w"] = din("n2w", [128, DT], F32)
    p["qkvb"] = din("qkvb", [128, 3 * DT], F32)
    p["obf"] = din("obf", [128, DT], F32)    # sa_o_b + v_bias @ Wo (host fold)
    p["cqb"] = din("cqb", [128, DT], F32)
    p["ckb"] = din("ckb", [128, DT], F32)    # cross-k bias
    p["cobf"] = din("cobf", [128, DT], F32)  # ca_o_b + cross-v bias @ Wco
    p["b1f"] = din("b1f", [128, FHT], F32)
    p["b2f"] = din("b2f", [128, FHT], F32)
    p["b3f"] = din("b3f", [128, DT], F32)
    # constant selector matrices, bf16
    p["ones128"] = din("ones128", [128, 128], BT)
    p["bd16"] = din("bd16", [128, 128], BT)
    p["qsel"] = din("qsel", [16, D], BT)
    p["ksel"] = din("ksel", [16, D], BT)
    p["cqsel"] = din("cqsel", [16, D], BT)
    p["cksel"] = din("cksel", [16, D], BT)
    p["rsel2"] = din("rsel2", [2, 128], BT)

    p["out"] = nc.dram_tensor("out", [D, T], F32, kind="ExternalOutput").ap()

    with tile.TileContext(nc) as tc:
        _emit(nc, tc, p, mybir)
    nc.compile()
    return nc


def _emit(nc, tc, p, mybir):
    ALU = mybir.AluOpType
    ACTF = mybir.ActivationFunctionType
    F32 = mybir.dt.float32
    BT = mybir.dt.bfloat16
    F8D = mybir.dt.float8e4
    I16 = mybir.dt.int16
    DR = mybir.MatmulPerfMode.DoubleRow

    pg = tc.alloc_tile_pool(name="pg", bufs=1)
    ps = tc.alloc_tile_pool(name="ps", bufs=8, space="PSUM")
    dram = tc.alloc_tile_pool(name="dram", bufs=1, space="DRAM")

    # shared-tag allocators
    def bigw(name):   # wide bf16 tiles (xt / k / v)
        return pg.tile([128, 1040], BT, tag="bigw", name=name, bufs=26)

    def xf(name):     # f32 [128, T] residual-stream tiles
        return pg.tile([128, T], F32, tag="xf", name=name, bufs=9)

    def qt(name):     # bf16 [128, T] q tiles
        return pg.tile([128, T], BT, tag="qt", name=name, bufs=11)

    def xp(name):     # fp8 pair tiles [128, 2048] (xn1 over N)
        return pg.tile([128, 2 * N], F8D, tag="xp", name=name, bufs=4)

    def sp(name):     # fp8 pair tiles [128, 1024] (T-sized pairs, h, o)
        return pg.tile([128, 2 * T], F8D, tag="sp", name=name, bufs=15)

    def wg8(name):    # fp8 DR weight group tiles [128, 2048]
        return pg.tile([128, 2048], F8D, tag="wg8", name=name, bufs=8)

    def w38(name):    # fp8 DR w3 tiles [128, 1024]
        return pg.tile([128, 1024], F8D, tag="w38", name=name, bufs=11)

    def ptile(name):  # exp(p) tiles
        return pg.tile([128, T], BT, tag="pt", name=name, bufs=13)

    def sqt(name, wid=512):    # square scratch bf16
        return pg.tile([128, wid], BT, tag="sq", name=name, bufs=3)

    def xnb(name):    # bf16 normed-x scratch [128, 1024]
        return pg.tile([128, N], BT, tag="xnb", name=name, bufs=2)

    def scratch4k(name, rows=128, wid=1024):  # f32 scratch (rr/ssq/den)
        return pg.tile([rows, wid], F32, tag="s4k", name=name, bufs=1)

    def scrbf(name, rows=16, wid=1024):
        return pg.tile([rows, wid], BT, tag="sbf", name=name, bufs=1)

    def psum(name):
        return ps.tile([128, 512], F32, tag="ps_n", name=name)

    # ---------------- constants ----------------
    cst = {}
    c_eps = pg.tile([128, 1], F32, tag="c_eps", name="c_eps")
    nc.any.memset(c_eps[:], EPS)
    for nm, k in (("ones128", 128), ("bd16", 128)):
        t = pg.tile([128, k], BT, tag=nm, name=f"c_{nm}")
        nc.sync.dma_start(t[:], p[nm][:])
        cst[nm] = t
    for nm in ("qsel", "ksel", "cqsel", "cksel"):
        t = pg.tile([16, D], BT, tag=nm, name=f"c_{nm}")
        nc.sync.dma_start(t[:], p[nm][:])
        cst[nm] = t
    t = pg.tile([2, 128], BT, tag="rsel2", name="c_rsel2")
    nc.sync.dma_start(t[:], p["rsel2"][:])
    cst["rsel2"] = t
    for nm, k in (("adab", NMOD * DT), ("n1w", DT), ("ncw", DT), ("n2w", DT),
                  ("qkvb", 3 * DT), ("obf", DT), ("cqb", DT), ("ckb", DT),
                  ("cobf", DT), ("b1f", FHT), ("b2f", FHT), ("b3f", DT),
                  ("bsel", B)):
        t = pg.tile([128, k], F32, tag=nm, name=f"c_{nm}")
        nc.sync.dma_start(t[:], p[nm][:])
        cst[nm] = t

    # =====================================================================
    # Stage 0a: sharded ada GEMV + 8-way AllGather of mods.
    # Every core computes mods[all 4 b, its 1152 columns].
    # =====================================================================
    cv = pg.tile([128, DT * B], F32, tag="cv", name="cv")
    nc.sync.dma_start(cv[:].rearrange("p (k b) -> p k b", k=DT),
                      p["cmat"][:].rearrange("(k p) b -> p k b", p=128))
    scs = pg.tile([128, DT * B], BT, tag="sc", name="scs")
    nc.scalar.activation(scs[:], cv[:], ACTF.Sigmoid)
    nc.vector.tensor_tensor(scs[:], scs[:], cv[:], ALU.mult)

    adat = []
    for k in range(DT):
        t = pg.tile([128, ASH], BT, tag="adat", name=f"adat{k}", bufs=DT)
        nc.sync.dma_start(t[:], p["adash"][k * 128:(k + 1) * 128, :])
        adat.append(t)
    strip = pg.tile([4, ASH], F32, tag="strip", name="strip")
    for ch in range(3):  # 3 chunks of 384 columns
        pm = psum(f"pm{ch}")
        for k in range(DT):
            nc.tensor.matmul(pm[0:4, 0:384], scs[:, k * B:(k + 1) * B],
                             adat[k][:, ch * 384:(ch + 1) * 384],
                             start=(k == 0), stop=(k == DT - 1))
        nc.vector.tensor_copy(strip[:, ch * 384:(ch + 1) * 384],
                                pm[0:4, 0:384])

    agin = dram.tile([B * ASH], F32, tag="agin", name="agin")
    ago = dram.tile([NCORES * B * ASH], F32, tag="ago", name="ago",
                    addr_space="Shared")
    nc.gpsimd.dma_start(agin[:].rearrange("(g j) -> g j", g=4), strip[:])
    nc.gpsimd.collective_compute(
        "AllGather", ALU.bypass, replica_groups=[list(range(NCORES))],
        ins=[agin[:]], outs=[ago[:]])

    mods_all = pg.tile([128, B * NMOD * DT], F32, tag="mall", name="mods_all")
    ago4 = ago[:].rearrange("(c g q p) -> c g p q", c=NCORES, g=B, p=128)
    mall4 = mods_all[:].rearrange("p (g c q) -> p g c q", g=B, c=NCORES)
    for cc in range(NCORES):
        for g in range(B):
            nc.scalar.dma_start(mall4[:, g, cc, :], ago4[cc, g])
    ma3 = mods_all[:].rearrange("p (g j) -> p g j", g=B)

    mods = pg.tile([128, NMOD * DT], F32, tag="mods", name="mods")
    nc.vector.tensor_scalar(mods[:], ma3[:, 0], cst["bsel"][:, 0:1], None,
                            ALU.mult)
    for g in range(1, B):
        nc.vector.scalar_tensor_tensor(mods[:], ma3[:, g],
                                       cst["bsel"][:, g:g + 1], mods[:],
                                       ALU.mult, ALU.add)
    nc.vector.tensor_tensor(mods[:], mods[:], cst["adab"][:], ALU.add)

    def msl(i):  # mods columns of modulation param i
        return mods[:, i * DT:(i + 1) * DT]

    seff = {}
    for nm, i_scale, w in (("sa", 1, "n1w"), ("ca", 4, "ncw"), ("ff", 7, "n2w")):
        s1 = pg.tile([128, DT], F32, tag=f"seff_{nm}", name=f"seff_{nm}")
        nc.vector.tensor_scalar(s1[:], msl(i_scale), 1.0, None, ALU.add)
        nc.vector.tensor_tensor(s1[:], s1[:], cst[w][:], ALU.mult)
        seff[nm] = s1
    gb = {}
    g64 = {}
    for nm, i_gate, bias in (("sa", 2, "obf"), ("ca", 5, "cobf"), ("ff", 8, "b3f")):
        t = pg.tile([128, DT], F32, tag=f"gb_{nm}", name=f"gb_{nm}")
        nc.vector.tensor_tensor(t[:], msl(i_gate), cst[bias][:], ALU.mult)
        gb[nm] = t
        t2 = pg.tile([128, DT], F32, tag=f"g64_{nm}", name=f"g64_{nm}")
        nc.vector.tensor_scalar(t2[:], msl(i_gate), IWS, None, ALU.mult)
        g64[nm] = t2
    sh_col = {"sa": 0, "ca": 3, "ff": 6}

    # =====================================================================
    # helpers
    # =====================================================================
    def load_wp(w_ap, cols0, cols, tagname, alloc=wg8):
        """Load DR weight tiles: per k-pair a [128, 2*cols] tile."""
        nkp = w_ap.shape[0] // 128
        tiles = []
        for kp in range(nkp):
            t = alloc(f"{tagname}_{kp}")
            nc.sync.dma_start(
                t[:, 0:2 * cols].rearrange("p (ko m) -> p ko m", ko=2),
                w_ap[kp * 128:(kp + 1) * 128, :]
                .rearrange("p (ko m) -> p ko m", ko=2)[:, :, cols0:cols0 + cols])
            tiles.append(t)
        return tiles

    def norm_mod(xtiles, Ttok, seff_t, sh_slice, name, sq_engine):
        """RMS + AdaLN modulate of feature-major tiles -> fp8 pair tiles."""
        NCH = Ttok // 512
        pss = [psum(f"ssn_{name}{c}") for c in range(NCH)]
        for k in range(DT):
            for c in range(NCH):
                sq = sqt(f"sq_{name}{k}_{c}")
                nc.scalar.activation(sq[:], xtiles[k][:, c * 512:(c + 1) * 512],
                                     ACTF.Square)
                nc.tensor.matmul(pss[c][:], cst["ones128"][:], sq[:],
                                 start=(k == 0), stop=(k == DT - 1))
        rr = scratch4k(f"rr_{name}")
        for c in range(NCH):
            nc.scalar.activation(rr[:, c * 512:(c + 1) * 512], pss[c][:],
                                 ACTF.Sqrt, bias=c_eps[:], scale=1.0 / D)
        nc.vector.reciprocal_approx_fast(rr[:, 0:Ttok], rr[:, 0:Ttok])
        alloc = xp if Ttok == N else sp
        xn = [alloc(f"xn_{name}{j}") for j in range(KP)]
        for k in range(DT):
            t1 = xnb(f"xnb_{name}{k}")
            nc.vector.tensor_tensor(t1[:, 0:Ttok], xtiles[k][:, 0:Ttok],
                                    rr[:, 0:Ttok], ALU.mult)
            half = xn[k // 2][:, (k % 2) * Ttok:(k % 2 + 1) * Ttok]
            nc.vector.tensor_scalar(half, t1[:, 0:Ttok],
                                    seff_t[:, k:k + 1], sh_slice[:, k:k + 1],
                                    ALU.mult, ALU.add)
        return xn

    def qk_norm(qtiles, Ttok, selname, name, sq_eng="gpsimd"):
        """Per-head RMS norm in place; head-norm weight folded into sel."""
        NCH = Ttok // 512
        ssq = scratch4k(f"ssq_{name}", rows=16)
        eng = nc.gpsimd if sq_eng == "gpsimd" else nc.vector
        for c in range(NCH):
            pq = psum(f"psq_{name}{c}")
            for t in range(DT):
                sq = sqt(f"qs_{name}{t}_{c}")
                eng.tensor_tensor(sq[:], qtiles[t][:, c * 512:(c + 1) * 512],
                                  qtiles[t][:, c * 512:(c + 1) * 512],
                                  ALU.mult)
                nc.tensor.matmul(pq[0:16, :],
                                 cst["bd16"][:, t * 16:(t + 1) * 16], sq[:],
                                 start=(t == 0), stop=(t == DT - 1))
            nc.scalar.activation(ssq[:, c * 512:(c + 1) * 512], pq[0:16, :],
                                 ACTF.Sqrt, bias=c_eps[0:16, :], scale=1.0 / HD)
        nc.vector.reciprocal_approx_fast(ssq[:, 0:Ttok], ssq[:, 0:Ttok])
        rqb = scrbf(f"rqb_{name}")
        nc.scalar.activation(rqb[:, 0:Ttok], ssq[:, 0:Ttok], ACTF.Copy)
        for t in range(DT):
            for c in range(NCH):
                pb = psum(f"qb_{name}{t}_{c}")
                nc.tensor.matmul(pb[:], cst[selname][:, t * 128:(t + 1) * 128],
                                 rqb[:, c * 512:(c + 1) * 512],
                                 start=True, stop=True)
                nc.vector.tensor_tensor(qtiles[t][:, c * 512:(c + 1) * 512],
                                        qtiles[t][:, c * 512:(c + 1) * 512],
                                        pb[:], ALU.mult)

    def attention(q_sb, k_sb, v_sb, Tk, name, o_pair):
        """softmax(q k^T / 8) v. Scores are emitted half-pair interleaved
        (concurrent 64-row-group matmuls); exp alternates ACT/DVE per
        (kt, half) so both engines run every step. PV trails two kt steps
        so its operands are always ready and the PE streams back-to-back.
        Per-pair denominator handling (recip + K=2 broadcast matmul) and
        the 1/den scaling writes the fp8 o_pair halves straight from PSUM."""
        KTk = Tk // 128

        deferred = [None]

        def do_pair(t):
            po = [psum(f"po_{name}{2 * t}"), psum(f"po_{name}{2 * t + 1}")]
            pipe = []

            def pv(kt):
                for half in range(2):
                    h16 = 2 * t + half
                    nc.tensor.matmul(po[half][0:65, :],
                                     v_sb[kt][:, h16 * 65:(h16 + 1) * 65],
                                     pipe[kt][half][:],
                                     start=(kt == 0), stop=(kt == KTk - 1),
                                     skip_group_check=True)

            for kt in range(KTk):
                cur = []
                for half in range(2):
                    lo = 64 * half
                    h16 = 2 * t + half
                    s_ps = psum(f"s_{name}{h16}_{kt}")
                    nc.tensor.matmul(
                        s_ps[:], k_sb[t][lo:lo + 64, kt * 128:(kt + 1) * 128],
                        q_sb[t][lo:lo + 64, 0:T], start=True, stop=True)
                    pt = ptile(f"pt_{name}{h16}_{kt}")
                    if (kt + half) % 2 == 1:
                        nc.vector.tensor_scalar(pt[:].bitcast(I16), s_ps[:],
                                                SCHR_A, SCHR_B,
                                                ALU.mult, ALU.add)
                    else:
                        nc.scalar.activation(pt[:], s_ps[:], ACTF.Exp,
                                             scale=ATT_SCALE)
                    cur.append(pt)
                pipe.append(cur)
                if kt >= 2:
                    pv(kt - 2)
                if kt == 3 and deferred[0] is not None:
                    deferred[0]()
                    deferred[0] = None
            pv(KTk - 2)
            pv(KTk - 1)
            # denominator: 1/row64 -> [2,T] base-0 tile (via Act-queue DMA;
            # engine ops need 32-aligned partition bases). The broadcast +
            # o_pair write is deferred into the next pair's matmul stream so
            # the chain latency never stalls the PE.
            drow = pg.tile([2, T], F32, tag="drow", name=f"dr_{name}{t}", bufs=2)
            rdb2 = pg.tile([2, T], BT, tag="rdb2", name=f"rb_{name}{t}", bufs=2)
            osb = qt(f"o_{name}{t}")
            for half in range(2):
                ds = pg.tile([1, T], F32, tag="dstr", name=f"ds_{name}{t}_{half}",
                             bufs=3)
                nc.vector.tensor_copy(ds[:], po[half][64:65, :])
                nc.scalar.dma_start(drow[half:half + 1, :], ds[:])
                nc.scalar.activation(osb[64 * half:64 * half + 64, :],
                                     po[half][0:64, :], ACTF.Copy)

            def tail(t=t, drow=drow, rdb2=rdb2, osb=osb):
                nc.vector.reciprocal_approx_fast(drow[:], drow[:])
                nc.scalar.activation(rdb2[:], drow[:], ACTF.Copy)
                pb = psum(f"ob_{name}{t}")
                nc.tensor.matmul(pb[:], cst["rsel2"][:], rdb2[:],
                                 start=True, stop=True)
                nc.vector.tensor_tensor(
                    o_pair[t // 2][:, (t % 2) * T:(t % 2 + 1) * T],
                    osb[:], pb[:], ALU.mult)

            deferred[0] = tail

        for t in range(DT):
            do_pair(t)
        deferred[0]()
        return o_pair

    def proj_dr(wap, wcols0, xnp, Tt, name, n_f=DT, nkp=KP,
                consume=None):
        """Feature-major DR projection: out f-tiles via 4 K=256 matmuls.
        `consume(f, c, pp)` turns each PSUM chunk into SBUF."""
        NCH = Tt // 512
        for f0 in range(0, n_f, 8):
            nf = min(8, n_f - f0)
            wt = load_wp(wap, wcols0 + f0 * 128, nf * 128, f"{name}_w{f0}")
            for f in range(nf):
                pps = [psum(f"p_{name}{f0 + f}_{c}") for c in range(NCH)]
                for kp in range(nkp):
                    for c in range(NCH):
                        nc.tensor.matmul(
                            pps[c][:],
                            wt[kp][:, 0:2 * nf * 128].rearrange(
                                "p (ko m) -> p ko m", ko=2)[:, :, f * 128:(f + 1) * 128],
                            xnp[kp][:, 0:2 * Tt].rearrange(
                                "p (ko n) -> p ko n", ko=2)[:, :, c * 512:(c + 1) * 512],
                            start=(kp == 0), stop=(kp == nkp - 1),
                            perf_mode=DR, skip_group_check=True)
                for c in range(NCH):
                    consume(f0 + f, c, pps[c])

    def proj_tok_dr(wap, wcols0, xnp, Tt, name, outs):
        """Token-major V projection (DR): stationary = xn pair slices."""
        ntt = Tt // 128
        wt = load_wp(wap, wcols0, D, f"{name}_w")
        for tt in range(ntt):
            pps = [psum(f"pv_{name}{tt}_{c}") for c in range(2)]
            for kp in range(KP):
                for c in range(2):
                    nc.tensor.matmul(
                        pps[c][:],
                        xnp[kp][:, 0:2 * Tt].rearrange(
                            "p (ko n) -> p ko n", ko=2)[:, :, tt * 128:(tt + 1) * 128],
                        wt[kp][:, 0:2 * D].rearrange(
                            "p (ko m) -> p ko m", ko=2)[:, :, c * 512:(c + 1) * 512],
                        start=(kp == 0), stop=(kp == KP - 1),
                        perf_mode=DR, skip_group_check=True)
            for c in range(2):
                dst = outs[tt][:, c * 8 * 65:(c * 8 + 8) * 65].rearrange(
                    "p (g e) -> p g e", g=8)[:, :, 0:64]
                nc.scalar.activation(dst, pps[c][:].rearrange("p (g e) -> p g e", g=8),
                                     ACTF.Copy, scale=IWS)

    # =====================================================================
    # Stage 0b: local cross-attention K/V from the local 512 source
    # tokens, then pair AllGather; both halves are read back into the
    # full-width tiles (same layout on both cores of a pair).
    # =====================================================================
    srcp = []
    for kp in range(KP):
        t = pg.tile([128, 2 * T], F8D, tag="srcp", name=f"srcp{kp}", bufs=KP)
        nc.sync.dma_start(t[:], p["srcp"][kp * 128:(kp + 1) * 128, :])
        srcp.append(t)

    kcaL = [qt(f"kcaL{f}") for f in range(DT)]

    def ckv_consume(f, c, pp):
        nc.scalar.activation(kcaL[f][:], pp[:], ACTF.Identity,
                             bias=cst["ckb"][:, f:f + 1], scale=IWS)

    proj_dr(p["wckv"], 0, srcp, T, "kca", consume=ckv_consume)
    vcaL = []
    for tt in range(T // 128):
        o = bigw(f"vcaL{tt}")
        nc.any.memset(o[:], 1.0)
        vcaL.append(o)
    proj_tok_dr(p["wckv"], D, srcp, T, "vca", vcaL)
    qk_norm(kcaL, T, "cksel", "kca", sq_eng="vector")

    KBYTES = 128 * 512
    VBYTES = 128 * 1040
    kvin = dram.tile([DT * KBYTES + 4 * VBYTES], BT, tag="kvin", name="kvin")
    kvout = dram.tile([2 * (DT * KBYTES + 4 * VBYTES)], BT, tag="kvout",
                      name="kvout")
    for f in range(DT):
        nc.gpsimd.dma_start(
            kvin[f * KBYTES:(f + 1) * KBYTES].rearrange("(p n) -> p n", p=128),
            kcaL[f][:, 0:512])
    for tt in range(4):
        nc.gpsimd.dma_start(
            kvin[DT * KBYTES + tt * VBYTES:DT * KBYTES + (tt + 1) * VBYTES]
            .rearrange("(p n) -> p n", p=128), vcaL[tt][:])
    nc.gpsimd.collective_compute(
        "AllGather", ALU.bypass,
        replica_groups=[[2 * i, 2 * i + 1] for i in range(B)],
        ins=[kvin[:]], outs=[kvout[:]])

    # =====================================================================
    # Stage 1: self-attention sublayer
    # =====================================================================
    xt_sb = []
    for k in range(DT):
        t = bigw(f"xt{k}")
        nc.sync.dma_start(t[:, 0:N], p["xt"][k * 128:(k + 1) * 128, :])
        xt_sb.append(t)
    xres_sb = []
    for k in range(DT):
        t = xf(f"xres{k}")
        nc.sync.dma_start(t[:], p["xres"][k * 128:(k + 1) * 128, :])
        xres_sb.append(t)

    xn1 = norm_mod(xt_sb, N, seff["sa"], msl(sh_col["sa"]), "n1", "scalar")
    q_sa = [qt(f"qsa{f}") for f in range(DT)]

    def q_consume(f, c, pp):
        nc.scalar.activation(q_sa[f][:], pp[:], ACTF.Identity,
                             bias=cst["qkvb"][:, f:f + 1], scale=IWS)

    k_sa = [bigw(f"ksa{f}") for f in range(DT)]

    def k_consume(f, c, pp):
        nc.scalar.activation(k_sa[f][:, c * 512:(c + 1) * 512], pp[:],
                             ACTF.Identity,
                             bias=cst["qkvb"][:, DT + f:DT + f + 1], scale=IWS)

    proj_dr(p["wqkv"], 0, xn1, T, "qsa", consume=q_consume)
    proj_dr(p["wqkv"], D, xn1, N, "ksa", consume=k_consume)
    v_sa = []
    for tt in range(N // 128):
        o = bigw(f"vsa{tt}")
        nc.any.memset(o[:], 1.0)
        v_sa.append(o)
    proj_tok_dr(p["wqkv"], 2 * D, xn1, N, "vsa", v_sa)
    qk_norm(q_sa, T, "qsel", "qsa", sq_eng="vector")
    qk_norm(k_sa, N, "ksel", "ksa", sq_eng="gpsimd")

    o1p = [sp(f"o1p{j}") for j in range(KP)]
    attention(q_sa, k_sa, v_sa, N, "a1", o1p)

    x1 = xres_sb

    def o1_consume(f, c, pp):
        nc.vector.affine_then_add(x1[f][:], pp[:], x1[f][:],
                                  g64["sa"][:, f:f + 1], gb["sa"][:, f:f + 1])

    proj_dr(p["wo"], 0, o1p, T, "o1", consume=o1_consume)

    kca = [bigw(f"kca{f}") for f in range(DT)]
    vca = [bigw(f"vca{tt}") for tt in range(8)]
    HALF_OFF = DT * KBYTES + 4 * VBYTES
    for h in range(2):
        for f in range(DT):
            o = h * HALF_OFF + f * KBYTES
            nc.scalar.dma_start(
                kca[f][:, h * 512:(h + 1) * 512],
                kvout[o:o + KBYTES].rearrange("(p n) -> p n", p=128))
        for tt in range(4):
            o = h * HALF_OFF + DT * KBYTES + tt * VBYTES
            nc.scalar.dma_start(
                vca[h * 4 + tt][:, 0:1040],
                kvout[o:o + VBYTES].rearrange("(p n) -> p n", p=128))

    # =====================================================================
    # Stage 2: cross-attention sublayer
    # =====================================================================
    xnc = norm_mod(x1, T, seff["ca"], msl(sh_col["ca"]), "nc", "scalar")
    q_ca = [qt(f"qca{f}") for f in range(DT)]

    def qca_consume(f, c, pp):
        nc.scalar.activation(q_ca[f][:], pp[:], ACTF.Identity,
                             bias=cst["cqb"][:, f:f + 1], scale=IWS)

    proj_dr(p["wcq"], 0, xnc, T, "qca", consume=qca_consume)
    qk_norm(q_ca, T, "cqsel", "qca", sq_eng="gpsimd")
    o2p = [sp(f"o2p{j}") for j in range(KP)]
    attention(q_ca, kca, vca, M, "a2", o2p)

    x2 = x1

    def o2_consume(f, c, pp):
        nc.vector.affine_then_add(x2[f][:], pp[:], x2[f][:],
                                  g64["ca"][:, f:f + 1], gb["ca"][:, f:f + 1])

    proj_dr(p["wco"], 0, o2p, T, "o2", consume=o2_consume)

    # =====================================================================
    # Stage 3: SwiGLU FFN sublayer
    # =====================================================================
    xn2 = norm_mod(x2, T, seff["ff"], msl(sh_col["ff"]), "n2", "scalar")
    h_pair = [sp(f"hp{j}") for j in range(FHP)]
    for f0 in range(0, FHT, 8):
        nf = min(8, FHT - f0)
        w1t = load_wp(p["w1"], f0 * 128, nf * 128, f"w1_{f0}")
        w2t = load_wp(p["w2"], f0 * 128, nf * 128, f"w2_{f0}")
        for f in range(nf):
            fi = f0 + f
            pp1 = psum(f"ph1_{fi}")
            for kp in range(KP):
                nc.tensor.matmul(
                    pp1[:],
                    w1t[kp][:, 0:2 * nf * 128].rearrange(
                        "p (ko m) -> p ko m", ko=2)[:, :, f * 128:(f + 1) * 128],
                    xn2[kp][:].rearrange("p (ko n) -> p ko n", ko=2),
                    start=(kp == 0), stop=(kp == KP - 1), perf_mode=DR)
            h1 = sqt(f"h1_{fi}")
            nc.scalar.activation(h1[:], pp1[:], ACTF.Silu,
                                 bias=cst["b1f"][:, fi:fi + 1], scale=IWS)
            pp2 = psum(f"ph2_{fi}")
            for kp in range(KP):
                nc.tensor.matmul(
                    pp2[:],
                    w2t[kp][:, 0:2 * nf * 128].rearrange(
                        "p (ko m) -> p ko m", ko=2)[:, :, f * 128:(f + 1) * 128],
                    xn2[kp][:].rearrange("p (ko n) -> p ko n", ko=2),
                    start=(kp == 0), stop=(kp == KP - 1), perf_mode=DR)
            h2 = ptile(f"h2_{fi}")
            nc.vector.tensor_scalar(h2[:], pp2[:], IWS,
                                    cst["b2f"][:, fi:fi + 1],
                                    ALU.mult, ALU.add)
            nc.vector.tensor_tensor(
                h_pair[fi // 2][:, (fi % 2) * T:(fi % 2 + 1) * T],
                h1[:], h2[:], ALU.mult)

    # out = h @ w3: 2 groups of 4 feature tiles, 4 live psums each
    for fg in range(0, DT, 4):
        psf = [psum(f"pf{fg + f}") for f in range(4)]
        for kp in range(FHP):
            w3t = w38(f"w3_{fg}_{kp}")
            nc.sync.dma_start(
                w3t[:].rearrange("p (ko m) -> p ko m", ko=2),
                p["w3"][kp * 128:(kp + 1) * 128, :]
                .rearrange("p (ko m) -> p ko m", ko=2)[:, :, fg * 128:(fg + 4) * 128])
            for f in range(4):
                nc.tensor.matmul(
                    psf[f][:],
                    w3t[:].rearrange("p (ko m) -> p ko m", ko=2)[:, :, f * 128:(f + 1) * 128],
                    h_pair[kp][:].rearrange("p (ko n) -> p ko n", ko=2),
                    start=(kp == 0), stop=(kp == FHP - 1), perf_mode=DR)
        for f in range(4):
            xo = x2[fg + f]
            nc.vector.affine_then_add(
                xo[:], psf[f][:], xo[:],
                g64["ff"][:, fg + f:fg + f + 1],
                gb["ff"][:, fg + f:fg + f + 1])
            nc.sync.dma_start(p["out"][(fg + f) * 128:(fg + f + 1) * 128, :], xo[:])

    pg.release()
    ps.release()
    dram.release()


# ==========================================================================
# host side
# ==========================================================================

def _fm(vec):
    """[128*k] f32 vector -> feature-major [128, k] (col j = feature tile j)."""
    v = np.asarray(vec, np.float32)
    return np.ascontiguousarray(v.reshape(-1, 128).T)


def _pair8(W, scale=WS):
    """[K, F] f32 -> DR pair-interleaved fp8 [K/256*128, 2*F], x scale."""
    W = np.asarray(W, np.float32) * scale
    W = np.clip(W, -240.0, 240.0)
    K, F = W.shape
    assert K % 256 == 0
    Wp = W.reshape(K // 256, 2, 128, F).transpose(0, 2, 1, 3).reshape(
        K // 256 * 128, 2 * F)
    return np.ascontiguousarray(Wp).astype(F8)


def _bd16():
    bd = np.zeros((128, 128), np.float32)
    for t in range(8):
        for p_ in range(128):
            bd[p_, t * 16 + 2 * t + p_ // 64] = 1.0
    return bd.astype(BF16)


def _rsel2():
    r = np.zeros((2, 128), np.float32)
    r[0, 0:64] = 1.0
    r[1, 64:128] = 1.0
    return r.astype(BF16)


def _sel(weights64):
    """[16, 1024] selector: sel[i, t*128+p] = w[p%64] * (i == 2t + p//64)."""
    w = np.ones(64, np.float32) if weights64 is None else \
        np.asarray(weights64, np.float32)
    s = np.zeros((16, D), np.float32)
    for col in range(D):
        i = 2 * (col // 128) + (col % 128) // 64
        s[i, col] = w[col % 64]
    return s.astype(BF16)


def make_in_maps(inputs):
    f32 = lambda a: np.ascontiguousarray(np.asarray(a, np.float32))
    bf = lambda a: np.ascontiguousarray(np.asarray(a, np.float32)).astype(BF16)

    x = f32(inputs["x"]); src = f32(inputs["source_tokens"]); c = f32(inputs["c"])
    qkv_b = f32(inputs["sa_qkv_b"])
    o_w = f32(inputs["sa_o_w"]); o_b = f32(inputs["sa_o_b"])
    ckv_b = f32(inputs["ca_kv_b"])
    co_w = f32(inputs["ca_o_w"]); co_b = f32(inputs["ca_o_b"])
    w1 = f32(inputs["mlp_w1"]); b1 = f32(inputs["mlp_b1"])
    w2 = f32(inputs["mlp_w2"]); b2 = f32(inputs["mlp_b2"])
    w3 = f32(inputs["mlp_w3"]); b3 = f32(inputs["mlp_b3"])

    # pad SwiGLU hidden to 2816; zero pads keep silu(0)*0 == 0 exact
    w1p = np.zeros((D, MHP), np.float32); w1p[:, :MH] = w1
    w2p = np.zeros((D, MHP), np.float32); w2p[:, :MH] = w2
    w3p = np.zeros((MHP, D), np.float32); w3p[:MH, :] = w3
    b1p = np.zeros(MHP, np.float32); b1p[:MH] = b1
    b2p = np.zeros(MHP, np.float32); b2p[:MH] = b2

    # fold the V biases through the linear attention + output projection
    obf = qkv_b[2 * D:3 * D] @ o_w + o_b
    cobf = ckv_b[D:2 * D] @ co_w + co_b

    ada_w = f32(inputs["ada_w"])
    shared = dict(
        wqkv=_pair8(inputs["sa_qkv_w"]), wo=_pair8(o_w),
        wcq=_pair8(inputs["ca_q_w"]), wckv=_pair8(inputs["ca_kv_w"]),
        wco=_pair8(co_w),
        w1=_pair8(w1p), w2=_pair8(w2p), w3=_pair8(w3p),
        adab=_fm(f32(inputs["ada_b"])), n1w=_fm(f32(inputs["n1_w"])),
        ncw=_fm(f32(inputs["nc_w"])), n2w=_fm(f32(inputs["n2_w"])),
        qkvb=_fm(qkv_b), obf=_fm(obf), cqb=_fm(f32(inputs["ca_q_b"])),
        ckb=_fm(ckv_b[0:D]), cobf=_fm(cobf),
        b1f=_fm(b1p), b2f=_fm(b2p), b3f=_fm(b3),
        ones128=np.ones((128, 128), BF16),
        bd16=_bd16(),
        qsel=_sel(inputs["sa_qn_w"]), ksel=_sel(inputs["sa_kn_w"]),
        cqsel=_sel(inputs["ca_qn_w"]), cksel=_sel(inputs["ca_kn_w"]),
        rsel2=_rsel2(),
        cmat=np.ascontiguousarray(c.T),
    )

    in_maps = []
    for cidx in range(NCORES):
        b, half = divmod(cidx, 2)
        xT = x[b].T  # [D, N]
        if half:
            xTp = np.concatenate([xT[:, T:], xT[:, :T]], axis=1)
        else:
            xTp = xT
        m = dict(shared)
        m["xt"] = np.ascontiguousarray(xTp).astype(BF16)
        m["xres"] = np.ascontiguousarray(xTp[:, :T])
        # local source tokens, fp8 pair-interleaved [512, 1024]
        sl = src[b].T[:, half * T:(half + 1) * T]  # [D, T]
        sl8 = np.clip(sl, -240, 240).reshape(4, 2, 128, T).transpose(
            0, 2, 1, 3).reshape(512, 2 * T)
        m["srcp"] = np.ascontiguousarray(sl8).astype(F8)
        m["adash"] = np.ascontiguousarray(
            ada_w[:, cidx * ASH:(cidx + 1) * ASH]).astype(BF16)
        bs = np.zeros((128, B), np.float32)
        bs[:, b] = 1.0
        m["bsel"] = bs
        in_maps.append(m)
    return in_maps


def assemble(results):
    out = np.empty((B, N, D), np.float32)
    for cidx in range(NCORES):
        b, half = divmod(cidx, 2)
        out[b, half * T:(half + 1) * T, :] = results[cidx]["out"].T
    return out


_NC_CACHE = []


def kernel(**inputs):
    from concourse.bass_utils import run_bass_kernel_spmd
    if not _NC_CACHE:
        _NC_CACHE.append(build_graph())
    nc = _NC_CACHE[0]
    in_maps = make_in_maps(inputs)
    res = run_bass_kernel_spmd(nc, in_maps, core_ids=list(range(NCORES)))
    return assemble(res.results)


if __name__ == "__main__":
    nc = build_graph()
    print("graph built OK; instructions:",
          sum(len(bb.instructions) for bb in nc.main_func.blocks))


# revision 31
# speedup vs baseline: 1.0270x; 1.0024x over previous
"""Trainium2 Bass kernel for nn_ConditionalJiTBlock (DiT-style block with
AdaLN modulation, self-attention, cross-attention and SwiGLU FFN).

Sharding: 8 NeuronCores = 4 batch elements x 2 token-halves. Each core
computes its 512 query tokens end-to-end. v2 additions over the baseline:

- All projection/FFN GEMMs run in fp8(e4m3) with perf_mode=DoubleRow
  (2 MACs/cell/cycle): weights are host-prescaled by 64 (pow2) and stored
  pair-interleaved [K/256*128, 2*F]; activations are written on-chip as
  fp8 "pair tiles" [128, 2*T] (feature tiles 2j/2j+1 side by side), so
  every contraction is 4 DR matmuls of K=256 instead of 8 bf16 matmuls.
  The 1/64 de-scale folds into the PSUM-consuming op (ACT scale / DVE
  scalar / affine_then_add scale).
- The AdaLN mods GEMV is sharded 8 ways: every core computes all 4 batch
  elements' mods over 1/8 of the 9216 columns, then an 8-way AllGather
  (147KB) broadcasts them; a per-core one-hot (bsel) selects the core's
  batch row with 4 vector ops. Kills the 18.9MB ada load + 54us of PE.
- Cross-attention K/V are computed for the LOCAL 512 source tokens only
  and pair-exchanged (AllGather over core pairs, 2.1MB) during the
  self-attention phase: both cores then read back both halves into the
  full K/V tiles (identical layout on both cores, so no per-core
  branching is needed).
- Softmax exp is split across engines: even key-tiles use ScalarE Exp,
  odd key-tiles use a DVE Schraudolph approximation (single tensor_scalar
  writing int16 bf16-bits: bits = rint(s*ATT_SCALE*log2e*128 + 16248)),
  halving the ACT-bound stretches of attention.
- Attention scores (K=64 contraction) are emitted half-pair-interleaved
  so the two 64-row-group matmuls run concurrently in the PE array.

Layout: as the baseline - activations feature-major (features on
partitions, tokens free), per-token scalars broadcast via small selector
matmuls, per-feature scalars as per-partition operands. Residual stream
f32; scores/PV bf16; projections fp8.
"""

import numpy as np
import ml_dtypes

BF16 = ml_dtypes.bfloat16
F8 = ml_dtypes.float8_e4m3

B, N, M, D, H, HD = 4, 1024, 1024, 1024, 16, 64
MH = 2730
MHP = 2816          # MH padded to 22*128
EPS = 1e-6
NCORES = 8
T = 512             # local query tokens per core
DT = D // 128       # 8
KP = DT // 2        # 4 contraction k-pairs for D
FHT = MHP // 128    # 22
FHP = FHT // 2      # 11
NMOD = 9
ASH = NMOD * D // NCORES  # 1152 ada columns per core
ATT_SCALE = HD ** -0.5
WS = 64.0           # fp8 weight pre-scale (pow2)
IWS = 1.0 / WS
LOG2E = 1.4426950408889634
SCHR_A = ATT_SCALE * LOG2E * 128.0
SCHR_B = 16248.0
DVE_KTS = (1, 3, 5, 7)  # key-tiles whose exp runs on DVE (Schraudolph)


# ==========================================================================
# device graph
# ==========================================================================

def build_graph(sim_compat=False):
    import concourse.bacc as bacc
    import concourse.mybir as mybir
    import concourse.tile as tile

    F32 = mybir.dt.float32
    BT = mybir.dt.bfloat16

    nc = bacc.Bacc("TRN2", target_bir_lowering=False, debug=False,
                   num_devices=NCORES)

    def din(name, shape, dtype):
        return nc.dram_tensor(name, shape, dtype, kind="ExternalInput").ap()

    F8D = mybir.dt.float8e4
    p = {}
    # activations
    p["xt"] = din("xt", [D, N], BT)          # x[b].T, local tokens first
    p["xres"] = din("xres", [D, T], F32)     # f32 residual columns (local)
    p["srcp"] = din("srcp", [4 * 128, 2 * T], F8D)  # local src tokens, paired
    p["cmat"] = din("cmat", [D, B], F32)     # c for all batch elements
    p["bsel"] = din("bsel", [128, B], F32)   # one-hot row of this core's b
    p["adash"] = din("adash", [D, ASH], BT)   # ada columns of this core
    # fp8 pair-interleaved weights [K/256*128, 2*F], pre-scaled by WS
    p["wqkv"] = din("wqkv", [512, 2 * 3 * D], F8D)
    p["wo"] = din("wo", [512, 2 * D], F8D)
    p["wcq"] = din("wcq", [512, 2 * D], F8D)
    p["wckv"] = din("wckv", [512, 2 * 2 * D], F8D)
    p["wco"] = din("wco", [512, 2 * D], F8D)
    p["w1"] = din("w1", [512, 2 * MHP], F8D)
    p["w2"] = din("w2", [512, 2 * MHP], F8D)
    p["w3"] = din("w3", [FHP * 128, 2 * D], F8D)
    # feature-major f32 vectors [128, k]  (column j = feature tile j)
    p["adab"] = din("adab", [128, NMOD * DT], F32)
    p["n1w"] = din("n1w", [128, DT], F32)
    p["ncw"] = din("ncw", [128, DT], F32)
    p["n2w"] = din("n2w", [128, DT], F32)
    p["qkvb"] = din("qkvb", [128, 3 * DT], F32)
    p["obf"] = din("obf", [128, DT], F32)    # sa_o_b + v_bias @ Wo (host fold)
    p["cqb"] = din("cqb", [128, DT], F32)
    p["ckb"] = din("ckb", [128, DT], F32)    # cross-k bias
    p["cobf"] = din("cobf", [128, DT], F32)  # ca_o_b + cross-v bias @ Wco
    p["b1f"] = din("b1f", [128, FHT], F32)
    p["b2f"] = din("b2f", [128, FHT], F32)
    p["b3f"] = din("b3f", [128, DT], F32)
    # constant selector matrices, bf16
    p["ones128"] = din("ones128", [128, 128], BT)
    p["bd16"] = din("bd16", [128, 128], BT)
    p["qsel"] = din("qsel", [16, D], BT)
    p["ksel"] = din("ksel", [16, D], BT)
    p["cqsel"] = din("cqsel", [16, D], BT)
    p["cksel"] = din("cksel", [16, D], BT)
    p["rsel2"] = din("rsel2", [2, 128], BT)

    p["out"] = nc.dram_tensor("out", [D, T], F32, kind="ExternalOutput").ap()

    with tile.TileContext(nc) as tc:
        _emit(nc, tc, p, mybir)
    nc.compile()
    return nc


def _emit(nc, tc, p, mybir):
    ALU = mybir.AluOpType
    ACTF = mybir.ActivationFunctionType
    F32 = mybir.dt.float32
    BT = mybir.dt.bfloat16
    F8D = mybir.dt.float8e4
    I16 = mybir.dt.int16
    DR = mybir.MatmulPerfMode.DoubleRow

    pg = tc.alloc_tile_pool(name="pg", bufs=1)
    ps = tc.alloc_tile_pool(name="ps", bufs=4, space="PSUM")
    dram = tc.alloc_tile_pool(name="dram", bufs=1, space="DRAM")

    # shared-tag allocators
    def bigw(name):   # wide bf16 tiles (xt / k / v)
        return pg.tile([128, 1040], BT, tag="bigw", name=name, bufs=26)

    def xf(name):     # f32 [128, T] residual-stream tiles
        return pg.tile([128, T], F32, tag="xf", name=name, bufs=9)

    def qt(name):     # bf16 [128, T] q tiles
        return pg.tile([128, T], BT, tag="qt", name=name, bufs=10)

    def xp(name):     # fp8 pair tiles [128, 2048] (xn1 over N)
        return pg.tile([128, 2 * N], F8D, tag="xp", name=name, bufs=4)

    def sp(name):     # fp8 pair tiles [128, 1024] (T-sized pairs, h, o)
        return pg.tile([128, 2 * T], F8D, tag="sp", name=name, bufs=15)

    def wg8(name):    # fp8 DR weight group tiles [128, 2048]
        return pg.tile([128, 2048], F8D, tag="wg8", name=name, bufs=8)

    def w38(name):    # fp8 DR w3 tiles [128, 1024]
        return pg.tile([128, 1024], F8D, tag="w38", name=name, bufs=11)

    def ptile(name):  # exp(p) double tiles (two key-tiles side by side)
        return pg.tile([128, 2 * T], BT, tag="pt", name=name, bufs=6)

    def sqt(name, wid=512):    # square scratch bf16
        return pg.tile([128, wid], BT, tag="sq", name=name, bufs=4)

    def xnb(name):    # bf16 normed-x scratch [128, 1024]
        return pg.tile([128, N], BT, tag="xnb", name=name, bufs=2)

    def scratch4k(name, rows=128, wid=1024):  # f32 scratch (rr/ssq/den)
        return pg.tile([rows, wid], F32, tag="s4k", name=name, bufs=1)

    def scrbf(name, rows=16, wid=1024):
        return pg.tile([rows, wid], BT, tag="sbf", name=name, bufs=1)

    def psum(name):
        return ps.tile([128, 512], F32, tag="ps_n", name=name, bufs=4)

    # ---------------- PE warmup + early ada loads ----------------
    warm = pg.tile([128, 2], BT, tag="warm", name="warm")
    nc.vector.memset(warm[:], 1.0)
    wps = ps.tile([128, 512], F32, tag="ps_n", name="warm_ps")
    for i in range(80):
        nc.tensor.matmul(wps[0:1, 0:1], warm[:, 0:1], warm[:, 1:2],
                         start=True, stop=True, skip_group_check=True)
    cv = pg.tile([128, DT * B], F32, tag="cv", name="cv")
    nc.sync.dma_start(cv[:].rearrange("p (k b) -> p k b", k=DT),
                      p["cmat"][:].rearrange("(k p) b -> p k b", p=128))
    adat = []
    for k in range(DT):
        t = pg.tile([128, ASH], BT, tag="adat", name=f"adat{k}", bufs=DT)
        nc.sync.dma_start(t[:], p["adash"][k * 128:(k + 1) * 128, :])
        adat.append(t)

    # ---------------- constants ----------------
    cst = {}
    c_eps = pg.tile([128, 1], F32, tag="c_eps", name="c_eps")
    nc.vector.memset(c_eps[:], EPS)
    for nm, k in (("ones128", 128), ("bd16", 128)):
        t = pg.tile([128, k], BT, tag=nm, name=f"c_{nm}")
        nc.sync.dma_start(t[:], p[nm][:])
        cst[nm] = t
    for nm in ("qsel", "ksel", "cqsel", "cksel"):
        t = pg.tile([16, D], BT, tag=nm, name=f"c_{nm}")
        nc.sync.dma_start(t[:], p[nm][:])
        cst[nm] = t
    t = pg.tile([2, 128], BT, tag="rsel2", name="c_rsel2")
    nc.sync.dma_start(t[:], p["rsel2"][:])
    cst["rsel2"] = t
    for nm, k in (("adab", NMOD * DT), ("n1w", DT), ("ncw", DT), ("n2w", DT),
                  ("qkvb", 3 * DT), ("obf", DT), ("cqb", DT), ("ckb", DT),
                  ("cobf", DT), ("b1f", FHT), ("b2f", FHT), ("b3f", DT),
                  ("bsel", B)):
        t = pg.tile([128, k], F32, tag=nm, name=f"c_{nm}")
        nc.sync.dma_start(t[:], p[nm][:])
        cst[nm] = t

    # =====================================================================
    # Stage 0a: sharded ada GEMV + 8-way AllGather of mods.
    # Every core computes mods[all 4 b, its 1152 columns].
    # =====================================================================
    scs = pg.tile([128, DT * B], BT, tag="sc", name="scs")
    nc.scalar.activation(scs[:], cv[:], ACTF.Sigmoid)
    nc.vector.tensor_tensor(scs[:], scs[:], cv[:], ALU.mult)

    strip = pg.tile([4, ASH], F32, tag="strip", name="strip")
    for ch in range(3):  # 3 chunks of 384 columns
        pm = psum(f"pm{ch}")
        for k in range(DT):
            nc.tensor.matmul(pm[0:4, 0:384], scs[:, k * B:(k + 1) * B],
                             adat[k][:, ch * 384:(ch + 1) * 384],
                             start=(k == 0), stop=(k == DT - 1))
        nc.vector.tensor_copy(strip[:, ch * 384:(ch + 1) * 384],
                                pm[0:4, 0:384])

    agin = dram.tile([B * ASH], F32, tag="agin", name="agin")
    ago = dram.tile([NCORES * B * ASH], F32, tag="ago", name="ago",
                    addr_space="Shared")
    nc.gpsimd.dma_start(agin[:].rearrange("(g j) -> g j", g=4), strip[:])
    nc.gpsimd.collective_compute(
        "AllGather", ALU.bypass, replica_groups=[list(range(NCORES))],
        ins=[agin[:]], outs=[ago[:]])

    sh_col = {"sa": 0, "ca": 3, "ff": 6}

    # =====================================================================
    # helpers
    # =====================================================================
    def load_wp(w_ap, cols0, cols, tagname, alloc=wg8):
        """Load DR weight tiles: per k-pair a [128, 2*cols] tile."""
        nkp = w_ap.shape[0] // 128
        tiles = []
        for kp in range(nkp):
            t = alloc(f"{tagname}_{kp}")
            nc.sync.dma_start(
                t[:, 0:2 * cols].rearrange("p (ko m) -> p ko m", ko=2),
                w_ap[kp * 128:(kp + 1) * 128, :]
                .rearrange("p (ko m) -> p ko m", ko=2)[:, :, cols0:cols0 + cols])
            tiles.append(t)
        return tiles

    def norm_mod(xtiles, Ttok, seff_t, sh_slice, name, sq_engine):
        """RMS + AdaLN modulate of feature-major tiles -> fp8 pair tiles."""
        NCH = Ttok // 512
        pss = [psum(f"ssn_{name}{c}") for c in range(NCH)]
        for k in range(DT):
            for c in range(NCH):
                sq = sqt(f"sq_{name}{k}_{c}")
                nc.scalar.activation(sq[:], xtiles[k][:, c * 512:(c + 1) * 512],
                                     ACTF.Square)
                nc.tensor.matmul(pss[c][:], cst["ones128"][:], sq[:],
                                 start=(k == 0), stop=(k == DT - 1))
        rr = scratch4k(f"rr_{name}")
        for c in range(NCH):
            nc.scalar.activation(rr[:, c * 512:(c + 1) * 512], pss[c][:],
                                 ACTF.Sqrt, bias=c_eps[:], scale=1.0 / D)
        nc.vector.reciprocal_approx_fast(rr[:, 0:Ttok], rr[:, 0:Ttok])
        alloc = xp if Ttok == N else sp
        xn = [alloc(f"xn_{name}{j}") for j in range(KP)]
        for k in range(DT):
            t1 = xnb(f"xnb_{name}{k}")
            nc.vector.tensor_tensor(t1[:, 0:Ttok], xtiles[k][:, 0:Ttok],
                                    rr[:, 0:Ttok], ALU.mult)
            half = xn[k // 2][:, (k % 2) * Ttok:(k % 2 + 1) * Ttok]
            nc.vector.tensor_scalar(half, t1[:, 0:Ttok],
                                    seff_t[:, k:k + 1], sh_slice[:, k:k + 1],
                                    ALU.mult, ALU.add)
        return xn

    def qk_norm(qtiles, Ttok, selname, name, sq_eng="gpsimd"):
        """Per-head RMS norm in place; head-norm weight folded into sel."""
        NCH = Ttok // 512
        ssq = scratch4k(f"ssq_{name}", rows=16)
        eng = nc.gpsimd if sq_eng == "gpsimd" else nc.vector
        for c in range(NCH):
            pq = psum(f"psq_{name}{c}")
            for t in range(DT):
                sq = sqt(f"qs_{name}{t}_{c}")
                eng.tensor_tensor(sq[:], qtiles[t][:, c * 512:(c + 1) * 512],
                                  qtiles[t][:, c * 512:(c + 1) * 512],
                                  ALU.mult)
                nc.tensor.matmul(pq[0:16, :],
                                 cst["bd16"][:, t * 16:(t + 1) * 16], sq[:],
                                 start=(t == 0), stop=(t == DT - 1))
            nc.scalar.activation(ssq[:, c * 512:(c + 1) * 512], pq[0:16, :],
                                 ACTF.Sqrt, bias=c_eps[0:16, :], scale=1.0 / HD)
        nc.vector.reciprocal_approx_fast(ssq[:, 0:Ttok], ssq[:, 0:Ttok])
        rqb = scrbf(f"rqb_{name}")
        nc.scalar.activation(rqb[:, 0:Ttok], ssq[:, 0:Ttok], ACTF.Copy)
        for t in range(DT):
            for c in range(NCH):
                pb = psum(f"qb_{name}{t}_{c}")
                nc.tensor.matmul(pb[:], cst[selname][:, t * 128:(t + 1) * 128],
                                 rqb[:, c * 512:(c + 1) * 512],
                                 start=True, stop=True)
                nc.vector.tensor_tensor(qtiles[t][:, c * 512:(c + 1) * 512],
                                        qtiles[t][:, c * 512:(c + 1) * 512],
                                        pb[:], ALU.mult)

    def attention(q_sb, k_sb, v_sb, Tk, name, o_pair):
        """softmax(q k^T / 8) v. Scores are emitted half-pair interleaved
        (concurrent 64-row-group matmuls); exp alternates ACT/DVE per
        (kt, half) so both engines run every step. PV trails two kt steps
        so its operands are always ready and the PE streams back-to-back.
        Per-pair denominator handling (recip + K=2 broadcast matmul) and
        the 1/den scaling writes the fp8 o_pair halves straight from PSUM."""
        KTk = Tk // 128
        KT2 = KTk // 2

        deferred = [None]

        def do_pair(t):
            po = [psum(f"po_{name}{2 * t}"), psum(f"po_{name}{2 * t + 1}")]
            pipe = []

            def pv(kt2):
                for ktsub in range(2):
                    kt = 2 * kt2 + ktsub
                    for half in range(2):
                        h16 = 2 * t + half
                        ptd = pipe[kt2][half]
                        nc.tensor.matmul(
                            po[half][0:65, :],
                            v_sb[kt][:, h16 * 65:(h16 + 1) * 65],
                            ptd[:, ktsub * 512:(ktsub + 1) * 512],
                            start=(kt == 0), stop=(kt == KTk - 1),
                            skip_group_check=True)

            for kt2 in range(KT2):
                sds = [ps.tile([128, 1024], F32, tag="ps_d",
                               name=f"sd_{name}{2 * t + h}_{kt2}", bufs=2)
                       for h in range(2)]
                for ktsub in range(2):
                    kt = 2 * kt2 + ktsub
                    for half in range(2):
                        lo = 64 * half
                        nc.tensor.matmul(
                            sds[half][:, ktsub * 512:(ktsub + 1) * 512],
                            k_sb[t][lo:lo + 64, kt * 128:(kt + 1) * 128],
                            q_sb[t][lo:lo + 64, 0:T], start=True, stop=True,
                            skip_group_check=True)
                cur = []
                for half in range(2):
                    h16 = 2 * t + half
                    ptd = ptile(f"pt_{name}{h16}_{kt2}")
                    if (kt2 + half) % 2 == 1:
                        nc.vector.tensor_scalar(ptd[:].bitcast(I16),
                                                sds[half][:],
                                                SCHR_A, SCHR_B,
                                                ALU.mult, ALU.add)
                    else:
                        nc.scalar.activation(ptd[:], sds[half][:], ACTF.Exp,
                                             scale=ATT_SCALE)
                    cur.append(ptd)
                pipe.append(cur)
                if kt2 >= 1:
                    pv(kt2 - 1)
                if kt2 == 2 and deferred[0] is not None:
                    deferred[0]()
                    deferred[0] = None
            pv(KT2 - 1)
            # denominator: 1/row64 -> [2,T] base-0 tile (via Act-queue DMA;
            # engine ops need 32-aligned partition bases). The broadcast +
            # o_pair write is deferred into the next pair's matmul stream so
            # the chain latency never stalls the PE.
            drow = pg.tile([2, T], F32, tag="drow", name=f"dr_{name}{t}", bufs=2)
            rdb2 = pg.tile([2, T], BT, tag="rdb2", name=f"rb_{name}{t}", bufs=2)
            osb = qt(f"o_{name}{t}")
            for half in range(2):
                ds = pg.tile([1, T], F32, tag="dstr", name=f"ds_{name}{t}_{half}",
                             bufs=3)
                nc.vector.tensor_copy(ds[:], po[half][64:65, :])
                nc.scalar.dma_start(drow[half:half + 1, :], ds[:])
                nc.scalar.activation(osb[64 * half:64 * half + 64, :],
                                     po[half][0:64, :], ACTF.Copy)

            def tail(t=t, drow=drow, rdb2=rdb2, osb=osb):
                nc.vector.reciprocal_approx_fast(drow[:], drow[:])
                nc.scalar.activation(rdb2[:], drow[:], ACTF.Copy)
                pb = psum(f"ob_{name}{t}")
                nc.tensor.matmul(pb[:], cst["rsel2"][:], rdb2[:],
                                 start=True, stop=True)
                nc.vector.tensor_tensor(
                    o_pair[t // 2][:, (t % 2) * T:(t % 2 + 1) * T],
                    osb[:], pb[:], ALU.mult)

            deferred[0] = tail

        for t in range(DT):
            do_pair(t)
        deferred[0]()
        return o_pair

    def proj_dr(wap, wcols0, xnp, Tt, name, n_f=DT, nkp=KP,
                consume=None):
        """Feature-major DR projection: out f-tiles via 4 K=256 matmuls.
        `consume(f, c, pp)` turns each PSUM chunk into SBUF."""
        NCH = Tt // 512
        for f0 in range(0, n_f, 8):
            nf = min(8, n_f - f0)
            wt = load_wp(wap, wcols0 + f0 * 128, nf * 128, f"{name}_w{f0}")
            for f in range(nf):
                pps = [psum(f"p_{name}{f0 + f}_{c}") for c in range(NCH)]
                for kp in range(nkp):
                    for c in range(NCH):
                        nc.tensor.matmul(
                            pps[c][:],
                            wt[kp][:, 0:2 * nf * 128].rearrange(
                                "p (ko m) -> p ko m", ko=2)[:, :, f * 128:(f + 1) * 128],
                            xnp[kp][:, 0:2 * Tt].rearrange(
                                "p (ko n) -> p ko n", ko=2)[:, :, c * 512:(c + 1) * 512],
                            start=(kp == 0), stop=(kp == nkp - 1),
                            perf_mode=DR, skip_group_check=True)
                for c in range(NCH):
                    consume(f0 + f, c, pps[c])

    def proj_tok_dr(wap, wcols0, xnp, Tt, name, outs):
        """Token-major V projection (DR): stationary = xn pair slices."""
        ntt = Tt // 128
        wt = load_wp(wap, wcols0, D, f"{name}_w")
        for tt in range(ntt):
            pps = [psum(f"pv_{name}{tt}_{c}") for c in range(2)]
            for kp in range(KP):
                for c in range(2):
                    nc.tensor.matmul(
                        pps[c][:],
                        xnp[kp][:, 0:2 * Tt].rearrange(
                            "p (ko n) -> p ko n", ko=2)[:, :, tt * 128:(tt + 1) * 128],
                        wt[kp][:, 0:2 * D].rearrange(
                            "p (ko m) -> p ko m", ko=2)[:, :, c * 512:(c + 1) * 512],
                        start=(kp == 0), stop=(kp == KP - 1),
                        perf_mode=DR, skip_group_check=True)
            for c in range(2):
                dst = outs[tt][:, c * 8 * 65:(c * 8 + 8) * 65].rearrange(
                    "p (g e) -> p g e", g=8)[:, :, 0:64]
                nc.scalar.activation(dst, pps[c][:].rearrange("p (g e) -> p g e", g=8),
                                     ACTF.Copy, scale=IWS)

    # =====================================================================
    # Stage 0b: local cross-attention K/V from the local 512 source
    # tokens, then pair AllGather; both halves are read back into the
    # full-width tiles (same layout on both cores of a pair).
    # =====================================================================
    srcp = []
    for kp in range(KP):
        t = pg.tile([128, 2 * T], F8D, tag="srcp", name=f"srcp{kp}", bufs=KP)
        nc.sync.dma_start(t[:], p["srcp"][kp * 128:(kp + 1) * 128, :])
        srcp.append(t)

    kcaL = [qt(f"kcaL{f}") for f in range(DT)]

    def ckv_consume(f, c, pp):
        nc.scalar.activation(kcaL[f][:], pp[:], ACTF.Identity,
                             bias=cst["ckb"][:, f:f + 1], scale=IWS)

    proj_dr(p["wckv"], 0, srcp, T, "kca", consume=ckv_consume)
    vcaL = []
    for tt in range(T // 128):
        o = bigw(f"vcaL{tt}")
        nc.vector.memset(o[:], 1.0)
        vcaL.append(o)
    proj_tok_dr(p["wckv"], D, srcp, T, "vca", vcaL)
    qk_norm(kcaL, T, "cksel", "kca", sq_eng="vector")

    KBYTES = 128 * 512
    VBYTES = 128 * 1040
    kvin = dram.tile([DT * KBYTES + 4 * VBYTES], BT, tag="kvin", name="kvin")
    kvout = dram.tile([2 * (DT * KBYTES + 4 * VBYTES)], BT, tag="kvout",
                      name="kvout")
    for f in range(DT):
        nc.gpsimd.dma_start(
            kvin[f * KBYTES:(f + 1) * KBYTES].rearrange("(p n) -> p n", p=128),
            kcaL[f][:, 0:512])
    for tt in range(4):
        nc.gpsimd.dma_start(
            kvin[DT * KBYTES + tt * VBYTES:DT * KBYTES + (tt + 1) * VBYTES]
            .rearrange("(p n) -> p n", p=128), vcaL[tt][:])
    nc.gpsimd.collective_compute(
        "AllGather", ALU.bypass,
        replica_groups=[[2 * i, 2 * i + 1] for i in range(B)],
        ins=[kvin[:]], outs=[kvout[:]])

    mods_all = pg.tile([128, B * NMOD * DT], F32, tag="mall", name="mods_all")
    ago4 = ago[:].rearrange("(c g q p) -> c g p q", c=NCORES, g=B, p=128)
    mall4 = mods_all[:].rearrange("p (g c q) -> p g c q", g=B, c=NCORES)
    for cc in range(NCORES):
        for g in range(B):
            nc.scalar.dma_start(mall4[:, g, cc, :], ago4[cc, g])
    ma3 = mods_all[:].rearrange("p (g j) -> p g j", g=B)

    mods = pg.tile([128, NMOD * DT], F32, tag="mods", name="mods")
    nc.vector.tensor_scalar(mods[:], ma3[:, 0], cst["bsel"][:, 0:1], None,
                            ALU.mult)
    for g in range(1, B):
        nc.vector.scalar_tensor_tensor(mods[:], ma3[:, g],
                                       cst["bsel"][:, g:g + 1], mods[:],
                                       ALU.mult, ALU.add)
    nc.vector.tensor_tensor(mods[:], mods[:], cst["adab"][:], ALU.add)

    def msl(i):  # mods columns of modulation param i
        return mods[:, i * DT:(i + 1) * DT]

    seff = {}
    for nm, i_scale, w in (("sa", 1, "n1w"), ("ca", 4, "ncw"), ("ff", 7, "n2w")):
        s1 = pg.tile([128, DT], F32, tag=f"seff_{nm}", name=f"seff_{nm}")
        nc.vector.tensor_scalar(s1[:], msl(i_scale), 1.0, None, ALU.add)
        nc.vector.tensor_tensor(s1[:], s1[:], cst[w][:], ALU.mult)
        seff[nm] = s1
    gb = {}
    g64 = {}
    for nm, i_gate, bias in (("sa", 2, "obf"), ("ca", 5, "cobf"), ("ff", 8, "b3f")):
        t = pg.tile([128, DT], F32, tag=f"gb_{nm}", name=f"gb_{nm}")
        nc.vector.tensor_tensor(t[:], msl(i_gate), cst[bias][:], ALU.mult)
        gb[nm] = t
        t2 = pg.tile([128, DT], F32, tag=f"g64_{nm}", name=f"g64_{nm}")
        nc.vector.tensor_scalar(t2[:], msl(i_gate), IWS, None, ALU.mult)
        g64[nm] = t2
    # =====================================================================
    # Stage 1: self-attention sublayer
    # =====================================================================
    xt_sb = []
    for k in range(DT):
        t = bigw(f"xt{k}")
        nc.sync.dma_start(t[:, 0:N], p["xt"][k * 128:(k + 1) * 128, :])
        xt_sb.append(t)
    xres_sb = []
    for k in range(DT):
        t = xf(f"xres{k}")
        nc.sync.dma_start(t[:], p["xres"][k * 128:(k + 1) * 128, :])
        xres_sb.append(t)

    xn1 = norm_mod(xt_sb, N, seff["sa"], msl(sh_col["sa"]), "n1", "scalar")
    q_sa = [qt(f"qsa{f}") for f in range(DT)]

    def q_consume(f, c, pp):
        nc.scalar.activation(q_sa[f][:], pp[:], ACTF.Identity,
                             bias=cst["qkvb"][:, f:f + 1], scale=IWS)

    k_sa = [bigw(f"ksa{f}") for f in range(DT)]

    def k_consume(f, c, pp):
        nc.scalar.activation(k_sa[f][:, c * 512:(c + 1) * 512], pp[:],
                             ACTF.Identity,
                             bias=cst["qkvb"][:, DT + f:DT + f + 1], scale=IWS)

    proj_dr(p["wqkv"], 0, xn1, T, "qsa", consume=q_consume)
    qk_norm(q_sa, T, "qsel", "qsa", sq_eng="vector")
    proj_dr(p["wqkv"], D, xn1, N, "ksa", consume=k_consume)
    qk_norm(k_sa, N, "ksel", "ksa", sq_eng="gpsimd")
    v_sa = []
    for tt in range(N // 128):
        o = bigw(f"vsa{tt}")
        nc.vector.memset(o[:], 1.0)
        v_sa.append(o)
    proj_tok_dr(p["wqkv"], 2 * D, xn1, N, "vsa", v_sa)

    o1p = [sp(f"o1p{j}") for j in range(KP)]
    attention(q_sa, k_sa, v_sa, N, "a1", o1p)

    x1 = xres_sb

    def o1_consume(f, c, pp):
        nc.vector.affine_then_add(x1[f][:], pp[:], x1[f][:],
                                  g64["sa"][:, f:f + 1], gb["sa"][:, f:f + 1])

    proj_dr(p["wo"], 0, o1p, T, "o1", consume=o1_consume)

    kca = [bigw(f"kca{f}") for f in range(DT)]
    vca = [bigw(f"vca{tt}") for tt in range(8)]
    HALF_OFF = DT * KBYTES + 4 * VBYTES
    for h in range(2):
        for f in range(DT):
            o = h * HALF_OFF + f * KBYTES
            nc.gpsimd.dma_start(
                kca[f][:, h * 512:(h + 1) * 512],
                kvout[o:o + KBYTES].rearrange("(p n) -> p n", p=128))
        for tt in range(4):
            o = h * HALF_OFF + DT * KBYTES + tt * VBYTES
            nc.gpsimd.dma_start(
                vca[h * 4 + tt][:, 0:1040],
                kvout[o:o + VBYTES].rearrange("(p n) -> p n", p=128))

    # =====================================================================
    # Stage 2: cross-attention sublayer
    # =====================================================================
    xnc = norm_mod(x1, T, seff["ca"], msl(sh_col["ca"]), "nc", "scalar")
    q_ca = [qt(f"qca{f}") for f in range(DT)]

    def qca_consume(f, c, pp):
        nc.scalar.activation(q_ca[f][:], pp[:], ACTF.Identity,
                             bias=cst["cqb"][:, f:f + 1], scale=IWS)

    proj_dr(p["wcq"], 0, xnc, T, "qca", consume=qca_consume)
    qk_norm(q_ca, T, "cqsel", "qca", sq_eng="gpsimd")
    o2p = [sp(f"o2p{j}") for j in range(KP)]
    attention(q_ca, kca, vca, M, "a2", o2p)

    x2 = x1

    def o2_consume(f, c, pp):
        nc.vector.affine_then_add(x2[f][:], pp[:], x2[f][:],
                                  g64["ca"][:, f:f + 1], gb["ca"][:, f:f + 1])

    proj_dr(p["wco"], 0, o2p, T, "o2", consume=o2_consume)

    # =====================================================================
    # Stage 3: SwiGLU FFN sublayer
    # =====================================================================
    xn2 = norm_mod(x2, T, seff["ff"], msl(sh_col["ff"]), "n2", "scalar")
    h_pair = [sp(f"hp{j}") for j in range(FHP)]
    for f0 in range(0, FHT, 8):
        nf = min(8, FHT - f0)
        w1t = load_wp(p["w1"], f0 * 128, nf * 128, f"w1_{f0}")
        w2t = load_wp(p["w2"], f0 * 128, nf * 128, f"w2_{f0}")
        for f in range(nf):
            fi = f0 + f
            pp1 = psum(f"ph1_{fi}")
            for kp in range(KP):
                nc.tensor.matmul(
                    pp1[:],
                    w1t[kp][:, 0:2 * nf * 128].rearrange(
                        "p (ko m) -> p ko m", ko=2)[:, :, f * 128:(f + 1) * 128],
                    xn2[kp][:].rearrange("p (ko n) -> p ko n", ko=2),
                    start=(kp == 0), stop=(kp == KP - 1), perf_mode=DR)
            h1 = sqt(f"h1_{fi}")
            nc.scalar.activation(h1[:], pp1[:], ACTF.Silu,
                                 bias=cst["b1f"][:, fi:fi + 1], scale=IWS)
            pp2 = psum(f"ph2_{fi}")
            for kp in range(KP):
                nc.tensor.matmul(
                    pp2[:],
                    w2t[kp][:, 0:2 * nf * 128].rearrange(
                        "p (ko m) -> p ko m", ko=2)[:, :, f * 128:(f + 1) * 128],
                    xn2[kp][:].rearrange("p (ko n) -> p ko n", ko=2),
                    start=(kp == 0), stop=(kp == KP - 1), perf_mode=DR)
            h2 = sqt(f"h2_{fi}")
            nc.vector.tensor_scalar(h2[:], pp2[:], IWS,
                                    cst["b2f"][:, fi:fi + 1],
                                    ALU.mult, ALU.add)
            nc.vector.tensor_tensor(
                h_pair[fi // 2][:, (fi % 2) * T:(fi % 2 + 1) * T],
                h1[:], h2[:], ALU.mult)

    # out = h @ w3: 2 groups of 4 feature tiles, 4 live psums each
    for fg in range(0, DT, 4):
        psf = [psum(f"pf{fg + f}") for f in range(4)]
        for kp in range(FHP):
            w3t = w38(f"w3_{fg}_{kp}")
            nc.sync.dma_start(
                w3t[:].rearrange("p (ko m) -> p ko m", ko=2),
                p["w3"][kp * 128:(kp + 1) * 128, :]
                .rearrange("p (ko m) -> p ko m", ko=2)[:, :, fg * 128:(fg + 4) * 128])
            for f in range(4):
                nc.tensor.matmul(
                    psf[f][:],
                    w3t[:].rearrange("p (ko m) -> p ko m", ko=2)[:, :, f * 128:(f + 1) * 128],
                    h_pair[kp][:].rearrange("p (ko n) -> p ko n", ko=2),
                    start=(kp == 0), stop=(kp == FHP - 1), perf_mode=DR)
        for f in range(4):
            xo = x2[fg + f]
            nc.vector.affine_then_add(
                xo[:], psf[f][:], xo[:],
                g64["ff"][:, fg + f:fg + f + 1],
                gb["ff"][:, fg + f:fg + f + 1])
            nc.sync.dma_start(p["out"][(fg + f) * 128:(fg + f + 1) * 128, :], xo[:])

    pg.release()
    ps.release()
    dram.release()


# ==========================================================================
# host side
# ==========================================================================

def _fm(vec):
    """[128*k] f32 vector -> feature-major [128, k] (col j = feature tile j)."""
    v = np.asarray(vec, np.float32)
    return np.ascontiguousarray(v.reshape(-1, 128).T)


def _pair8(W, scale=WS):
    """[K, F] f32 -> DR pair-interleaved fp8 [K/256*128, 2*F], x scale."""
    W = np.asarray(W, np.float32) * scale
    W = np.clip(W, -240.0, 240.0)
    K, F = W.shape
    assert K % 256 == 0
    Wp = W.reshape(K // 256, 2, 128, F).transpose(0, 2, 1, 3).reshape(
        K // 256 * 128, 2 * F)
    return np.ascontiguousarray(Wp).astype(F8)


def _bd16():
    bd = np.zeros((128, 128), np.float32)
    for t in range(8):
        for p_ in range(128):
            bd[p_, t * 16 + 2 * t + p_ // 64] = 1.0
    return bd.astype(BF16)


def _rsel2():
    r = np.zeros((2, 128), np.float32)
    r[0, 0:64] = 1.0
    r[1, 64:128] = 1.0
    return r.astype(BF16)


def _sel(weights64):
    """[16, 1024] selector: sel[i, t*128+p] = w[p%64] * (i == 2t + p//64)."""
    w = np.ones(64, np.float32) if weights64 is None else \
        np.asarray(weights64, np.float32)
    s = np.zeros((16, D), np.float32)
    for col in range(D):
        i = 2 * (col // 128) + (col % 128) // 64
        s[i, col] = w[col % 64]
    return s.astype(BF16)


def make_in_maps(inputs):
    f32 = lambda a: np.ascontiguousarray(np.asarray(a, np.float32))
    bf = lambda a: np.ascontiguousarray(np.asarray(a, np.float32)).astype(BF16)

    x = f32(inputs["x"]); src = f32(inputs["source_tokens"]); c = f32(inputs["c"])
    qkv_b = f32(inputs["sa_qkv_b"])
    o_w = f32(inputs["sa_o_w"]); o_b = f32(inputs["sa_o_b"])
    ckv_b = f32(inputs["ca_kv_b"])
    co_w = f32(inputs["ca_o_w"]); co_b = f32(inputs["ca_o_b"])
    w1 = f32(inputs["mlp_w1"]); b1 = f32(inputs["mlp_b1"])
    w2 = f32(inputs["mlp_w2"]); b2 = f32(inputs["mlp_b2"])
    w3 = f32(inputs["mlp_w3"]); b3 = f32(inputs["mlp_b3"])

    # pad SwiGLU hidden to 2816; zero pads keep silu(0)*0 == 0 exact
    w1p = np.zeros((D, MHP), np.float32); w1p[:, :MH] = w1
    w2p = np.zeros((D, MHP), np.float32); w2p[:, :MH] = w2
    w3p = np.zeros((MHP, D), np.float32); w3p[:MH, :] = w3
    b1p = np.zeros(MHP, np.float32); b1p[:MH] = b1
    b2p = np.zeros(MHP, np.float32); b2p[:MH] = b2

    # fold the V biases through the linear attention + output projection
    obf = qkv_b[2 * D:3 * D] @ o_w + o_b
    cobf = ckv_b[D:2 * D] @ co_w + co_b

    ada_w = f32(inputs["ada_w"])
    shared = dict(
        wqkv=_pair8(inputs["sa_qkv_w"]), wo=_pair8(o_w),
        wcq=_pair8(inputs["ca_q_w"]), wckv=_pair8(inputs["ca_kv_w"]),
        wco=_pair8(co_w),
        w1=_pair8(w1p), w2=_pair8(w2p), w3=_pair8(w3p),
        adab=_fm(f32(inputs["ada_b"])), n1w=_fm(f32(inputs["n1_w"])),
        ncw=_fm(f32(inputs["nc_w"])), n2w=_fm(f32(inputs["n2_w"])),
        qkvb=_fm(qkv_b), obf=_fm(obf), cqb=_fm(f32(inputs["ca_q_b"])),
        ckb=_fm(ckv_b[0:D]), cobf=_fm(cobf),
        b1f=_fm(b1p), b2f=_fm(b2p), b3f=_fm(b3),
        ones128=np.ones((128, 128), BF16),
        bd16=_bd16(),
        qsel=_sel(inputs["sa_qn_w"]), ksel=_sel(inputs["sa_kn_w"]),
        cqsel=_sel(inputs["ca_qn_w"]), cksel=_sel(inputs["ca_kn_w"]),
        rsel2=_rsel2(),
        cmat=np.ascontiguousarray(c.T),
    )

    in_maps = []
    for cidx in range(NCORES):
        b, half = divmod(cidx, 2)
        xT = x[b].T  # [D, N]
        if half:
            xTp = np.concatenate([xT[:, T:], xT[:, :T]], axis=1)
        else:
            xTp = xT
        m = dict(shared)
        m["xt"] = np.ascontiguousarray(xTp).astype(BF16)
        m["xres"] = np.ascontiguousarray(xTp[:, :T])
        # local source tokens, fp8 pair-interleaved [512, 1024]
        sl = src[b].T[:, half * T:(half + 1) * T]  # [D, T]
        sl8 = np.clip(sl, -240, 240).reshape(4, 2, 128, T).transpose(
            0, 2, 1, 3).reshape(512, 2 * T)
        m["srcp"] = np.ascontiguousarray(sl8).astype(F8)
        m["adash"] = np.ascontiguousarray(
            ada_w[:, cidx * ASH:(cidx + 1) * ASH]).astype(BF16)
        bs = np.zeros((128, B), np.float32)
        bs[:, b] = 1.0
        m["bsel"] = bs
        in_maps.append(m)
    return in_maps


def assemble(results):
    out = np.empty((B, N, D), np.float32)
    for cidx in range(NCORES):
        b, half = divmod(cidx, 2)
        out[b, half * T:(half + 1) * T, :] = results[cidx]["out"].T
    return out


_NC_CACHE = []


def kernel(**inputs):
    from concourse.bass_utils import run_bass_kernel_spmd
    if not _NC_CACHE:
        _NC_CACHE.append(build_graph())
    nc = _NC_CACHE[0]
    in_maps = make_in_maps(inputs)
    res = run_bass_kernel_spmd(nc, in_maps, core_ids=list(range(NCORES)))
    return assemble(res.results)


if __name__ == "__main__":
    nc = build_graph()
    print("graph built OK; instructions:",
          sum(len(bb.instructions) for bb in nc.main_func.blocks))


# revision 33
# speedup vs baseline: 1.1788x; 1.1478x over previous
"""Trainium2 Bass kernel for nn_ConditionalJiTBlock (DiT-style block with
AdaLN modulation, self-attention, cross-attention and SwiGLU FFN).

Sharding: 8 NeuronCores = 4 batch elements x 2 token-halves. Each core
computes its 512 query tokens end-to-end. v2 additions over the baseline:

- All projection/FFN GEMMs run in fp8(e4m3) with perf_mode=DoubleRow
  (2 MACs/cell/cycle): weights are host-prescaled by 64 (pow2) and stored
  pair-interleaved [K/256*128, 2*F]; activations are written on-chip as
  fp8 "pair tiles" [128, 2*T] (feature tiles 2j/2j+1 side by side), so
  every contraction is 4 DR matmuls of K=256 instead of 8 bf16 matmuls.
  The 1/64 de-scale folds into the PSUM-consuming op (ACT scale / DVE
  scalar / affine_then_add scale).
- The AdaLN mods GEMV is sharded 8 ways: every core computes all 4 batch
  elements' mods over 1/8 of the 9216 columns, then an 8-way AllGather
  (147KB) broadcasts them; a per-core one-hot (bsel) selects the core's
  batch row with 4 vector ops. Kills the 18.9MB ada load + 54us of PE.
- Cross-attention K/V are computed for the LOCAL 512 source tokens only
  and pair-exchanged (AllGather over core pairs, 2.1MB) during the
  self-attention phase: both cores then read back both halves into the
  full K/V tiles (identical layout on both cores, so no per-core
  branching is needed).
- Softmax exp is split across engines: even key-tiles use ScalarE Exp,
  odd key-tiles use a DVE Schraudolph approximation (single tensor_scalar
  writing int16 bf16-bits: bits = rint(s*ATT_SCALE*log2e*128 + 16248)),
  halving the ACT-bound stretches of attention.
- Attention scores (K=64 contraction) are emitted half-pair-interleaved
  so the two 64-row-group matmuls run concurrently in the PE array.

Layout: as the baseline - activations feature-major (features on
partitions, tokens free), per-token scalars broadcast via small selector
matmuls, per-feature scalars as per-partition operands. Residual stream
f32; scores/PV bf16; projections fp8.
"""

import numpy as np
import ml_dtypes

BF16 = ml_dtypes.bfloat16
F8 = ml_dtypes.float8_e4m3

B, N, M, D, H, HD = 4, 1024, 1024, 1024, 16, 64
MH = 2730
MHP = 2816          # MH padded to 22*128
EPS = 1e-6
NCORES = 8
T = 512             # local query tokens per core
DT = D // 128       # 8
KP = DT // 2        # 4 contraction k-pairs for D
FHT = MHP // 128    # 22
FHP = FHT // 2      # 11
NMOD = 9
ASH = NMOD * D // NCORES  # 1152 ada columns per core
ATT_SCALE = HD ** -0.5
WS = 64.0           # fp8 weight pre-scale (pow2)
IWS = 1.0 / WS
LOG2E = 1.4426950408889634
SCHR_A = ATT_SCALE * LOG2E * 128.0
SCHR_B = 16248.0
DVE_KTS = (1, 3, 5, 7)  # key-tiles whose exp runs on DVE (Schraudolph)


# ==========================================================================
# device graph
# ==========================================================================

def build_graph(sim_compat=False):
    import concourse.bacc as bacc
    import concourse.mybir as mybir
    import concourse.tile as tile

    F32 = mybir.dt.float32
    BT = mybir.dt.bfloat16

    nc = bacc.Bacc("TRN2", target_bir_lowering=False, debug=False,
                   num_devices=NCORES)

    def din(name, shape, dtype):
        return nc.dram_tensor(name, shape, dtype, kind="ExternalInput").ap()

    F8D = mybir.dt.float8e4
    p = {}
    # activations
    p["xt"] = din("xt", [D, N], BT)          # x[b].T, local tokens first
    p["xres"] = din("xres", [D, T], F32)     # f32 residual columns (local)
    p["srcp"] = din("srcp", [4 * 128, 2 * T], F8D)  # local src tokens, paired
    p["cmat"] = din("cmat", [D, B], F32)     # c for all batch elements
    p["bsel"] = din("bsel", [128, B], F32)   # one-hot row of this core's b
    p["adash"] = din("adash", [D, ASH], BT)   # ada columns of this core
    # fp8 pair-interleaved weights [K/256*128, 2*F], pre-scaled by WS
    p["wqkv"] = din("wqkv", [512, 2 * 3 * D], F8D)
    p["wo"] = din("wo", [512, 2 * D], F8D)
    p["wcq"] = din("wcq", [512, 2 * D], F8D)
    p["wckv"] = din("wckv", [512, 2 * 2 * D], F8D)
    p["wco"] = din("wco", [512, 2 * D], F8D)
    p["w1"] = din("w1", [512, 2 * MHP], F8D)
    p["w2"] = din("w2", [512, 2 * MHP], F8D)
    p["w3"] = din("w3", [FHP * 128, 2 * D], F8D)
    # feature-major f32 vectors [128, k]  (column j = feature tile j)
    p["adab"] = din("adab", [128, NMOD * DT], F32)
    p["n1w"] = din("n1w", [128, DT], F32)
    p["ncw"] = din("ncw", [128, DT], F32)
    p["n2w"] = din("n2w", [128, DT], F32)
    p["qkvb"] = din("qkvb", [128, 3 * DT], F32)
    p["obf"] = din("obf", [128, DT], F32)    # sa_o_b + v_bias @ Wo (host fold)
    p["cqb"] = din("cqb", [128, DT], F32)
    p["ckb"] = din("ckb", [128, DT], F32)    # cross-k bias
    p["cobf"] = din("cobf", [128, DT], F32)  # ca_o_b + cross-v bias @ Wco
    p["b1f"] = din("b1f", [128, FHT], F32)
    p["b2f"] = din("b2f", [128, FHT], F32)
    p["b3f"] = din("b3f", [128, DT], F32)
    # constant selector matrices, bf16
    p["ones128"] = din("ones128", [128, 128], BT)
    p["bd16"] = din("bd16", [128, 128], BT)
    p["qsel"] = din("qsel", [16, D], BT)
    p["ksel"] = din("ksel", [16, D], BT)
    p["cqsel"] = din("cqsel", [16, D], BT)
    p["cksel"] = din("cksel", [16, D], BT)
    p["rsel2"] = din("rsel2", [2, 128], BT)
    p["eye72"] = din("eye72", [NMOD * DT, NMOD * DT], F32)

    p["out"] = nc.dram_tensor("out", [D, T], F32, kind="ExternalOutput").ap()

    with tile.TileContext(nc) as tc:
        _emit(nc, tc, p, mybir)
    nc.compile()
    return nc


def _emit(nc, tc, p, mybir):
    ALU = mybir.AluOpType
    ACTF = mybir.ActivationFunctionType
    F32 = mybir.dt.float32
    BT = mybir.dt.bfloat16
    F8D = mybir.dt.float8e4
    I16 = mybir.dt.int16
    DR = mybir.MatmulPerfMode.DoubleRow

    pg = tc.alloc_tile_pool(name="pg", bufs=1)
    ps = tc.alloc_tile_pool(name="ps", bufs=4, space="PSUM")
    dram = tc.alloc_tile_pool(name="dram", bufs=1, space="DRAM")

    # shared-tag allocators
    def bigw(name):   # wide bf16 tiles (xt / k / v)
        return pg.tile([128, 1040], BT, tag="bigw", name=name, bufs=26)

    def xf(name):     # f32 [128, T] residual-stream tiles
        return pg.tile([128, T], F32, tag="xf", name=name, bufs=9)

    def qt(name):     # bf16 [128, T] q tiles
        return pg.tile([128, T], BT, tag="qt", name=name, bufs=10)

    def xp(name):     # fp8 pair tiles [128, 2048] (xn1 over N)
        return pg.tile([128, 2 * N], F8D, tag="xp", name=name, bufs=4)

    def sp(name):     # fp8 pair tiles [128, 1024] (T-sized pairs, h, o)
        return pg.tile([128, 2 * T], F8D, tag="sp", name=name, bufs=14)

    def wg8(name):    # fp8 DR weight group tiles [128, 2048]
        return pg.tile([128, 2048], F8D, tag="wg8", name=name, bufs=8)

    def w38(name):    # fp8 DR w3 tiles [128, 1024]
        return pg.tile([128, 1024], F8D, tag="w38", name=name, bufs=11)

    def ptile(name):  # exp(p) double tiles (two key-tiles side by side)
        return pg.tile([128, 2 * T], BT, tag="pt", name=name, bufs=6)

    def sqt(name, wid=512):    # square scratch bf16
        return pg.tile([128, wid], BT, tag="sq", name=name, bufs=4)

    def xnb(name):    # bf16 normed-x scratch [128, 1024]
        return pg.tile([128, N], BT, tag="xnb", name=name, bufs=2)

    def scratch4k(name, rows=128, wid=1024):  # f32 scratch (rr/ssq/den)
        return pg.tile([rows, wid], F32, tag="s4k", name=name, bufs=1)

    def scrbf(name, rows=16, wid=1024):
        return pg.tile([rows, wid], BT, tag="sbf", name=name, bufs=1)

    def psum(name):
        return ps.tile([128, 512], F32, tag="ps_n", name=name, bufs=4)

    # ---------------- PE warmup + early ada loads ----------------
    warm = pg.tile([128, 2], BT, tag="warm", name="warm")
    nc.vector.memset(warm[:], 1.0)
    wps = ps.tile([128, 512], F32, tag="ps_n", name="warm_ps")
    for i in range(80):
        nc.tensor.matmul(wps[0:1, 0:1], warm[:, 0:1], warm[:, 1:2],
                         start=True, stop=True, skip_group_check=True)
    cv = pg.tile([128, DT * B], F32, tag="cv", name="cv")
    nc.sync.dma_start(cv[:].rearrange("p (k b) -> p k b", k=DT),
                      p["cmat"][:].rearrange("(k p) b -> p k b", p=128))
    adat = []
    for k in range(DT):
        t = pg.tile([128, ASH], BT, tag="adat", name=f"adat{k}", bufs=DT)
        nc.sync.dma_start(t[:], p["adash"][k * 128:(k + 1) * 128, :])
        adat.append(t)

    # ---------------- constants ----------------
    cst = {}
    c_eps = pg.tile([128, 1], F32, tag="c_eps", name="c_eps")
    nc.vector.memset(c_eps[:], EPS)
    for nm, k in (("ones128", 128), ("bd16", 128)):
        t = pg.tile([128, k], BT, tag=nm, name=f"c_{nm}")
        nc.sync.dma_start(t[:], p[nm][:])
        cst[nm] = t
    for nm in ("qsel", "ksel", "cqsel", "cksel"):
        t = pg.tile([16, D], BT, tag=nm, name=f"c_{nm}")
        nc.sync.dma_start(t[:], p[nm][:])
        cst[nm] = t
    t = pg.tile([2, 128], BT, tag="rsel2", name="c_rsel2")
    nc.sync.dma_start(t[:], p["rsel2"][:])
    cst["rsel2"] = t
    t = pg.tile([NMOD * DT, NMOD * DT], F32, tag="eye72", name="c_eye72")
    nc.sync.dma_start(t[:], p["eye72"][:])
    cst["eye72"] = t
    for nm, k in (("adab", NMOD * DT), ("n1w", DT), ("ncw", DT), ("n2w", DT),
                  ("qkvb", 3 * DT), ("obf", DT), ("cqb", DT), ("ckb", DT),
                  ("cobf", DT), ("b1f", FHT), ("b2f", FHT), ("b3f", DT),
                  ("bsel", B)):
        t = pg.tile([128, k], F32, tag=nm, name=f"c_{nm}")
        nc.sync.dma_start(t[:], p[nm][:])
        cst[nm] = t

    # =====================================================================
    # Stage 0a: sharded ada GEMV + 8-way AllGather of mods.
    # Every core computes mods[all 4 b, its 1152 columns].
    # =====================================================================
    scs = pg.tile([128, DT * B], BT, tag="sc", name="scs")
    nc.scalar.activation(scs[:], cv[:], ACTF.Sigmoid)
    nc.vector.tensor_tensor(scs[:], scs[:], cv[:], ALU.mult)

    strip = pg.tile([4, ASH], F32, tag="strip", name="strip")
    for ch in range(3):  # 3 chunks of 384 columns
        pm = psum(f"pm{ch}")
        for k in range(DT):
            nc.tensor.matmul(pm[0:4, 0:384], scs[:, k * B:(k + 1) * B],
                             adat[k][:, ch * 384:(ch + 1) * 384],
                             start=(k == 0), stop=(k == DT - 1))
        nc.vector.tensor_copy(strip[:, ch * 384:(ch + 1) * 384],
                                pm[0:4, 0:384])

    agin = dram.tile([B * ASH], F32, tag="agin", name="agin")
    ago = dram.tile([NCORES * B * ASH], F32, tag="ago", name="ago",
                    addr_space="Shared")
    nc.gpsimd.dma_start(agin[:].rearrange("(g j) -> g j", g=4), strip[:])
    nc.gpsimd.collective_compute(
        "AllGather", ALU.bypass, replica_groups=[list(range(NCORES))],
        ins=[agin[:]], outs=[ago[:]])

    sh_col = {"sa": 0, "ca": 3, "ff": 6}

    # =====================================================================
    # helpers
    # =====================================================================
    def load_wp(w_ap, cols0, cols, tagname, alloc=wg8):
        """Load DR weight tiles: per k-pair a [128, 2*cols] tile."""
        nkp = w_ap.shape[0] // 128
        tiles = []
        for kp in range(nkp):
            t = alloc(f"{tagname}_{kp}")
            nc.sync.dma_start(
                t[:, 0:2 * cols].rearrange("p (ko m) -> p ko m", ko=2),
                w_ap[kp * 128:(kp + 1) * 128, :]
                .rearrange("p (ko m) -> p ko m", ko=2)[:, :, cols0:cols0 + cols])
            tiles.append(t)
        return tiles

    def norm_mod(xtiles, Ttok, seff_t, sh_slice, name, sq_engine):
        """RMS + AdaLN modulate of feature-major tiles -> fp8 pair tiles."""
        NCH = Ttok // 512
        pss = [psum(f"ssn_{name}{c}") for c in range(NCH)]
        for k in range(DT):
            for c in range(NCH):
                sq = sqt(f"sq_{name}{k}_{c}")
                nc.scalar.activation(sq[:], xtiles[k][:, c * 512:(c + 1) * 512],
                                     ACTF.Square)
                nc.tensor.matmul(pss[c][:], cst["ones128"][:], sq[:],
                                 start=(k == 0), stop=(k == DT - 1))
        rr = scratch4k(f"rr_{name}")
        for c in range(NCH):
            nc.scalar.activation(rr[:, c * 512:(c + 1) * 512], pss[c][:],
                                 ACTF.Sqrt, bias=c_eps[:], scale=1.0 / D)
        nc.vector.reciprocal_approx_fast(rr[:, 0:Ttok], rr[:, 0:Ttok])
        alloc = xp if Ttok == N else sp
        xn = [alloc(f"xn_{name}{j}") for j in range(KP)]
        for k in range(DT):
            t1 = xnb(f"xnb_{name}{k}")
            nc.vector.tensor_tensor(t1[:, 0:Ttok], xtiles[k][:, 0:Ttok],
                                    rr[:, 0:Ttok], ALU.mult)
            half = xn[k // 2][:, (k % 2) * Ttok:(k % 2 + 1) * Ttok]
            nc.vector.tensor_scalar(half, t1[:, 0:Ttok],
                                    seff_t[:, k:k + 1], sh_slice[:, k:k + 1],
                                    ALU.mult, ALU.add)
        return xn

    def qk_norm(qtiles, Ttok, selname, name, sq_eng="gpsimd"):
        """Per-head RMS norm in place; head-norm weight folded into sel."""
        NCH = Ttok // 512
        ssq = scratch4k(f"ssq_{name}", rows=16)
        eng = nc.gpsimd if sq_eng == "gpsimd" else nc.vector
        for c in range(NCH):
            pq = psum(f"psq_{name}{c}")
            for t in range(DT):
                sq = sqt(f"qs_{name}{t}_{c}")
                eng.tensor_tensor(sq[:], qtiles[t][:, c * 512:(c + 1) * 512],
                                  qtiles[t][:, c * 512:(c + 1) * 512],
                                  ALU.mult)
                nc.tensor.matmul(pq[0:16, :],
                                 cst["bd16"][:, t * 16:(t + 1) * 16], sq[:],
                                 start=(t == 0), stop=(t == DT - 1))
            nc.scalar.activation(ssq[:, c * 512:(c + 1) * 512], pq[0:16, :],
                                 ACTF.Sqrt, bias=c_eps[0:16, :], scale=1.0 / HD)
        nc.vector.reciprocal_approx_fast(ssq[:, 0:Ttok], ssq[:, 0:Ttok])
        rqb = scrbf(f"rqb_{name}")
        nc.scalar.activation(rqb[:, 0:Ttok], ssq[:, 0:Ttok], ACTF.Copy)
        for t in range(DT):
            for c in range(NCH):
                pb = psum(f"qb_{name}{t}_{c}")
                nc.tensor.matmul(pb[:], cst[selname][:, t * 128:(t + 1) * 128],
                                 rqb[:, c * 512:(c + 1) * 512],
                                 start=True, stop=True)
                nc.vector.tensor_tensor(qtiles[t][:, c * 512:(c + 1) * 512],
                                        qtiles[t][:, c * 512:(c + 1) * 512],
                                        pb[:], ALU.mult)

    def attention(q_sb, k_sb, v_sb, Tk, name, o_pair):
        """softmax(q k^T / 8) v. Scores are emitted half-pair interleaved
        (concurrent 64-row-group matmuls); exp alternates ACT/DVE per
        (kt, half) so both engines run every step. PV trails two kt steps
        so its operands are always ready and the PE streams back-to-back.
        Per-pair denominator handling (recip + K=2 broadcast matmul) and
        the 1/den scaling writes the fp8 o_pair halves straight from PSUM."""
        KTk = Tk // 128
        KT2 = KTk // 2

        deferred = [None]

        def do_pair(t):
            po = [psum(f"po_{name}{2 * t}"), psum(f"po_{name}{2 * t + 1}")]
            pipe = []

            def pv(kt2):
                for ktsub in range(2):
                    kt = 2 * kt2 + ktsub
                    for half in range(2):
                        h16 = 2 * t + half
                        ptd = pipe[kt2][half]
                        nc.tensor.matmul(
                            po[half][0:65, :],
                            v_sb[kt][:, h16 * 65:(h16 + 1) * 65],
                            ptd[:, ktsub * 512:(ktsub + 1) * 512],
                            start=(kt == 0), stop=(kt == KTk - 1),
                            skip_group_check=True)

            for kt2 in range(KT2):
                sds = [ps.tile([128, 1024], F32, tag="ps_d",
                               name=f"sd_{name}{2 * t + h}_{kt2}", bufs=2)
                       for h in range(2)]
                for ktsub in range(2):
                    kt = 2 * kt2 + ktsub
                    for half in range(2):
                        lo = 64 * half
                        nc.tensor.matmul(
                            sds[half][:, ktsub * 512:(ktsub + 1) * 512],
                            k_sb[t][lo:lo + 64, kt * 128:(kt + 1) * 128],
                            q_sb[t][lo:lo + 64, 0:T], start=True, stop=True,
                            skip_group_check=True)
                cur = []
                for half in range(2):
                    h16 = 2 * t + half
                    ptd = ptile(f"pt_{name}{h16}_{kt2}")
                    if (kt2 + half) % 2 == 1:
                        nc.vector.tensor_scalar(ptd[:].bitcast(I16),
                                                sds[half][:],
                                                SCHR_A, SCHR_B,
                                                ALU.mult, ALU.add)
                    else:
                        nc.scalar.activation(ptd[:], sds[half][:], ACTF.Exp,
                                             scale=ATT_SCALE)
                    cur.append(ptd)
                pipe.append(cur)
                if kt2 >= 1:
                    pv(kt2 - 1)
                if kt2 == 2 and deferred[0] is not None:
                    deferred[0]()
                    deferred[0] = None
            pv(KT2 - 1)
            # denominator: 1/row64 -> [2,T] base-0 tile (via Act-queue DMA;
            # engine ops need 32-aligned partition bases). The broadcast +
            # o_pair write is deferred into the next pair's matmul stream so
            # the chain latency never stalls the PE.
            drow = pg.tile([2, T], F32, tag="drow", name=f"dr_{name}{t}", bufs=2)
            rdb2 = pg.tile([2, T], BT, tag="rdb2", name=f"rb_{name}{t}", bufs=2)
            osb = qt(f"o_{name}{t}")
            for half in range(2):
                ds = pg.tile([1, T], F32, tag="dstr", name=f"ds_{name}{t}_{half}",
                             bufs=3)
                nc.vector.tensor_copy(ds[:], po[half][64:65, :])
                nc.scalar.dma_start(drow[half:half + 1, :], ds[:])
                nc.scalar.activation(osb[64 * half:64 * half + 64, :],
                                     po[half][0:64, :], ACTF.Copy)

            def tail(t=t, drow=drow, rdb2=rdb2, osb=osb):
                nc.vector.reciprocal_approx_fast(drow[:], drow[:])
                nc.scalar.activation(rdb2[:], drow[:], ACTF.Copy)
                pb = psum(f"ob_{name}{t}")
                nc.tensor.matmul(pb[:], cst["rsel2"][:], rdb2[:],
                                 start=True, stop=True)
                nc.vector.tensor_tensor(
                    o_pair[t // 2][:, (t % 2) * T:(t % 2 + 1) * T],
                    osb[:], pb[:], ALU.mult)

            deferred[0] = tail

        for t in range(DT):
            do_pair(t)
        deferred[0]()
        return o_pair

    def proj_dr(wap, wcols0, xnp, Tt, name, n_f=DT, nkp=KP,
                consume=None):
        """Feature-major DR projection: out f-tiles via 4 K=256 matmuls.
        `consume(f, c, pp)` turns each PSUM chunk into SBUF."""
        NCH = Tt // 512
        for f0 in range(0, n_f, 8):
            nf = min(8, n_f - f0)
            wt = load_wp(wap, wcols0 + f0 * 128, nf * 128, f"{name}_w{f0}")
            for f in range(nf):
                pps = [psum(f"p_{name}{f0 + f}_{c}") for c in range(NCH)]
                for kp in range(nkp):
                    for c in range(NCH):
                        nc.tensor.matmul(
                            pps[c][:],
                            wt[kp][:, 0:2 * nf * 128].rearrange(
                                "p (ko m) -> p ko m", ko=2)[:, :, f * 128:(f + 1) * 128],
                            xnp[kp][:, 0:2 * Tt].rearrange(
                                "p (ko n) -> p ko n", ko=2)[:, :, c * 512:(c + 1) * 512],
                            start=(kp == 0), stop=(kp == nkp - 1),
                            perf_mode=DR, skip_group_check=True)
                for c in range(NCH):
                    consume(f0 + f, c, pps[c])

    def proj_tok_dr(wap, wcols0, xnp, Tt, name, outs):
        """Token-major V projection (DR): stationary = xn pair slices."""
        ntt = Tt // 128
        wt = load_wp(wap, wcols0, D, f"{name}_w")
        for tt in range(ntt):
            pps = [psum(f"pv_{name}{tt}_{c}") for c in range(2)]
            for kp in range(KP):
                for c in range(2):
                    nc.tensor.matmul(
                        pps[c][:],
                        xnp[kp][:, 0:2 * Tt].rearrange(
                            "p (ko n) -> p ko n", ko=2)[:, :, tt * 128:(tt + 1) * 128],
                        wt[kp][:, 0:2 * D].rearrange(
                            "p (ko m) -> p ko m", ko=2)[:, :, c * 512:(c + 1) * 512],
                        start=(kp == 0), stop=(kp == KP - 1),
                        perf_mode=DR, skip_group_check=True)
            for c in range(2):
                dst = outs[tt][:, c * 8 * 65:(c * 8 + 8) * 65].rearrange(
                    "p (g e) -> p g e", g=8)[:, :, 0:64]
                nc.scalar.activation(dst, pps[c][:].rearrange("p (g e) -> p g e", g=8),
                                     ACTF.Copy, scale=IWS)

    # =====================================================================
    # Stage 0b: local cross-attention K/V from the local 512 source
    # tokens, then pair AllGather; both halves are read back into the
    # full-width tiles (same layout on both cores of a pair).
    # =====================================================================
    srcp = []
    for kp in range(KP):
        t = pg.tile([128, 2 * T], F8D, tag="srcp", name=f"srcp{kp}", bufs=KP)
        nc.sync.dma_start(t[:], p["srcp"][kp * 128:(kp + 1) * 128, :])
        srcp.append(t)

    kcaL = [qt(f"kcaL{f}") for f in range(DT)]

    def ckv_consume(f, c, pp):
        nc.scalar.activation(kcaL[f][:], pp[:], ACTF.Identity,
                             bias=cst["ckb"][:, f:f + 1], scale=IWS)

    proj_dr(p["wckv"], 0, srcp, T, "kca", consume=ckv_consume)
    vcaL = []
    for tt in range(T // 128):
        o = bigw(f"vcaL{tt}")
        nc.vector.memset(o[:], 1.0)
        vcaL.append(o)
    proj_tok_dr(p["wckv"], D, srcp, T, "vca", vcaL)
    qk_norm(kcaL, T, "cksel", "kca", sq_eng="vector")

    KBYTES = 128 * 512
    VBYTES = 128 * 1040
    kvin = dram.tile([DT * KBYTES + 4 * VBYTES], BT, tag="kvin", name="kvin")
    kvout = dram.tile([2 * (DT * KBYTES + 4 * VBYTES)], BT, tag="kvout",
                      name="kvout")
    for f in range(DT):
        nc.scalar.dma_start(
            kvin[f * KBYTES:(f + 1) * KBYTES].rearrange("(p n) -> p n", p=128),
            kcaL[f][:, 0:512])
    for tt in range(4):
        nc.scalar.dma_start(
            kvin[DT * KBYTES + tt * VBYTES:DT * KBYTES + (tt + 1) * VBYTES]
            .rearrange("(p n) -> p n", p=128), vcaL[tt][:])
    nc.gpsimd.collective_compute(
        "AllGather", ALU.bypass,
        replica_groups=[[2 * i, 2 * i + 1] for i in range(B)],
        ins=[kvin[:]], outs=[kvout[:]])

    # Gather read-back in transposed layout (contiguous 512B runs, cheap
    # gpsimd descriptors), batch-select there, then transpose back with one
    # tiny f32 matmul against the identity.
    NM = NMOD * DT
    mall_T = pg.tile([NM, B * 128], F32, tag="mallT", name="mall_T")
    ago3 = ago[:].rearrange("(c g q p) -> c q g p", c=NCORES, g=B, p=128)
    for cc in range(NCORES):
        nc.gpsimd.dma_start(
            mall_T[cc * NMOD:(cc + 1) * NMOD, :]
            .rearrange("q (g pp) -> q g pp", g=B),
            ago3[cc])
    maT3 = mall_T[:].rearrange("q (g pp) -> q g pp", g=B)

    modsT = pg.tile([NM, 128], F32, tag="modsT", name="modsT")
    nc.vector.tensor_scalar(modsT[:], maT3[:, 0], cst["bsel"][0:NM, 0:1],
                            None, ALU.mult)
    for g in range(1, B):
        nc.vector.scalar_tensor_tensor(modsT[:], maT3[:, g],
                                       cst["bsel"][0:NM, g:g + 1], modsT[:],
                                       ALU.mult, ALU.add)
    pmT = psum("pmT")
    nc.tensor.matmul(pmT[:, 0:NM], modsT[:], cst["eye72"][:],
                     start=True, stop=True)
    mods = pg.tile([128, NM], F32, tag="mods", name="mods")
    nc.vector.tensor_tensor(mods[:], pmT[:, 0:NM], cst["adab"][:], ALU.add)

    def msl(i):  # mods columns of modulation param i
        return mods[:, i * DT:(i + 1) * DT]

    seff = {}
    for nm, i_scale, w in (("sa", 1, "n1w"), ("ca", 4, "ncw"), ("ff", 7, "n2w")):
        s1 = pg.tile([128, DT], F32, tag=f"seff_{nm}", name=f"seff_{nm}")
        nc.vector.tensor_scalar(s1[:], msl(i_scale), 1.0, None, ALU.add)
        nc.vector.tensor_tensor(s1[:], s1[:], cst[w][:], ALU.mult)
        seff[nm] = s1
    gb = {}
    g64 = {}
    for nm, i_gate, bias in (("sa", 2, "obf"), ("ca", 5, "cobf"), ("ff", 8, "b3f")):
        t = pg.tile([128, DT], F32, tag=f"gb_{nm}", name=f"gb_{nm}")
        nc.vector.tensor_tensor(t[:], msl(i_gate), cst[bias][:], ALU.mult)
        gb[nm] = t
        t2 = pg.tile([128, DT], F32, tag=f"g64_{nm}", name=f"g64_{nm}")
        nc.vector.tensor_scalar(t2[:], msl(i_gate), IWS, None, ALU.mult)
        g64[nm] = t2
    # =====================================================================
    # Stage 1: self-attention sublayer
    # =====================================================================
    xt_sb = []
    for k in range(DT):
        t = bigw(f"xt{k}")
        nc.sync.dma_start(t[:, 0:N], p["xt"][k * 128:(k + 1) * 128, :])
        xt_sb.append(t)
    xres_sb = []
    for k in range(DT):
        t = xf(f"xres{k}")
        nc.sync.dma_start(t[:], p["xres"][k * 128:(k + 1) * 128, :])
        xres_sb.append(t)

    xn1 = norm_mod(xt_sb, N, seff["sa"], msl(sh_col["sa"]), "n1", "scalar")
    q_sa = [qt(f"qsa{f}") for f in range(DT)]

    def q_consume(f, c, pp):
        nc.scalar.activation(q_sa[f][:], pp[:], ACTF.Identity,
                             bias=cst["qkvb"][:, f:f + 1], scale=IWS)

    k_sa = [bigw(f"ksa{f}") for f in range(DT)]

    def k_consume(f, c, pp):
        nc.scalar.activation(k_sa[f][:, c * 512:(c + 1) * 512], pp[:],
                             ACTF.Identity,
                             bias=cst["qkvb"][:, DT + f:DT + f + 1], scale=IWS)

    proj_dr(p["wqkv"], 0, xn1, T, "qsa", consume=q_consume)
    qk_norm(q_sa, T, "qsel", "qsa", sq_eng="vector")
    proj_dr(p["wqkv"], D, xn1, N, "ksa", consume=k_consume)
    qk_norm(k_sa, N, "ksel", "ksa", sq_eng="gpsimd")
    v_sa = []
    for tt in range(N // 128):
        o = bigw(f"vsa{tt}")
        nc.vector.memset(o[:], 1.0)
        v_sa.append(o)
    proj_tok_dr(p["wqkv"], 2 * D, xn1, N, "vsa", v_sa)

    o1p = [sp(f"o1p{j}") for j in range(KP)]
    attention(q_sa, k_sa, v_sa, N, "a1", o1p)

    x1 = xres_sb

    def o1_consume(f, c, pp):
        nc.vector.affine_then_add(x1[f][:], pp[:], x1[f][:],
                                  g64["sa"][:, f:f + 1], gb["sa"][:, f:f + 1])

    proj_dr(p["wo"], 0, o1p, T, "o1", consume=o1_consume)

    kca = [bigw(f"kca{f}") for f in range(DT)]
    vca = [bigw(f"vca{tt}") for tt in range(8)]
    HALF_OFF = DT * KBYTES + 4 * VBYTES
    for h in range(2):
        for f in range(DT):
            o = h * HALF_OFF + f * KBYTES
            nc.sync.dma_start(
                kca[f][:, h * 512:(h + 1) * 512],
                kvout[o:o + KBYTES].rearrange("(p n) -> p n", p=128))
        for tt in range(4):
            o = h * HALF_OFF + DT * KBYTES + tt * VBYTES
            nc.sync.dma_start(
                vca[h * 4 + tt][:, 0:1040],
                kvout[o:o + VBYTES].rearrange("(p n) -> p n", p=128))

    # =====================================================================
    # Stage 2: cross-attention sublayer
    # =====================================================================
    xnc = norm_mod(x1, T, seff["ca"], msl(sh_col["ca"]), "nc", "scalar")
    q_ca = [qt(f"qca{f}") for f in range(DT)]

    def qca_consume(f, c, pp):
        nc.scalar.activation(q_ca[f][:], pp[:], ACTF.Identity,
                             bias=cst["cqb"][:, f:f + 1], scale=IWS)

    proj_dr(p["wcq"], 0, xnc, T, "qca", consume=qca_consume)
    qk_norm(q_ca, T, "cqsel", "qca", sq_eng="gpsimd")
    o2p = [sp(f"o2p{j}") for j in range(KP)]
    attention(q_ca, kca, vca, M, "a2", o2p)

    x2 = x1

    def o2_consume(f, c, pp):
        nc.vector.affine_then_add(x2[f][:], pp[:], x2[f][:],
                                  g64["ca"][:, f:f + 1], gb["ca"][:, f:f + 1])

    proj_dr(p["wco"], 0, o2p, T, "o2", consume=o2_consume)

    # =====================================================================
    # Stage 3: SwiGLU FFN sublayer
    # =====================================================================
    xn2 = norm_mod(x2, T, seff["ff"], msl(sh_col["ff"]), "n2", "scalar")
    h_pair = [sp(f"hp{j}") for j in range(FHP)]
    for f0 in range(0, FHT, 8):
        nf = min(8, FHT - f0)
        w1t = load_wp(p["w1"], f0 * 128, nf * 128, f"w1_{f0}")
        w2t = load_wp(p["w2"], f0 * 128, nf * 128, f"w2_{f0}")
        for f in range(nf):
            fi = f0 + f
            pp1 = psum(f"ph1_{fi}")
            for kp in range(KP):
                nc.tensor.matmul(
                    pp1[:],
                    w1t[kp][:, 0:2 * nf * 128].rearrange(
                        "p (ko m) -> p ko m", ko=2)[:, :, f * 128:(f + 1) * 128],
                    xn2[kp][:].rearrange("p (ko n) -> p ko n", ko=2),
                    start=(kp == 0), stop=(kp == KP - 1), perf_mode=DR)
            h1 = sqt(f"h1_{fi}")
            nc.scalar.activation(h1[:], pp1[:], ACTF.Silu,
                                 bias=cst["b1f"][:, fi:fi + 1], scale=IWS)
            pp2 = psum(f"ph2_{fi}")
            for kp in range(KP):
                nc.tensor.matmul(
                    pp2[:],
                    w2t[kp][:, 0:2 * nf * 128].rearrange(
                        "p (ko m) -> p ko m", ko=2)[:, :, f * 128:(f + 1) * 128],
                    xn2[kp][:].rearrange("p (ko n) -> p ko n", ko=2),
                    start=(kp == 0), stop=(kp == KP - 1), perf_mode=DR)
            h2 = sqt(f"h2_{fi}")
            nc.vector.tensor_scalar(h2[:], pp2[:], IWS,
                                    cst["b2f"][:, fi:fi + 1],
                                    ALU.mult, ALU.add)
            nc.vector.tensor_tensor(
                h_pair[fi // 2][:, (fi % 2) * T:(fi % 2 + 1) * T],
                h1[:], h2[:], ALU.mult)

    # out = h @ w3: 2 groups of 4 feature tiles, 4 live psums each
    for fg in range(0, DT, 4):
        psf = [psum(f"pf{fg + f}") for f in range(4)]
        for kp in range(FHP):
            w3t = w38(f"w3_{fg}_{kp}")
            nc.sync.dma_start(
                w3t[:].rearrange("p (ko m) -> p ko m", ko=2),
                p["w3"][kp * 128:(kp + 1) * 128, :]
                .rearrange("p (ko m) -> p ko m", ko=2)[:, :, fg * 128:(fg + 4) * 128])
            for f in range(4):
                nc.tensor.matmul(
                    psf[f][:],
                    w3t[:].rearrange("p (ko m) -> p ko m", ko=2)[:, :, f * 128:(f + 1) * 128],
                    h_pair[kp][:].rearrange("p (ko n) -> p ko n", ko=2),
                    start=(kp == 0), stop=(kp == FHP - 1), perf_mode=DR)
        for f in range(4):
            xo = x2[fg + f]
            nc.vector.affine_then_add(
                xo[:], psf[f][:], xo[:],
                g64["ff"][:, fg + f:fg + f + 1],
                gb["ff"][:, fg + f:fg + f + 1])
            nc.sync.dma_start(p["out"][(fg + f) * 128:(fg + f + 1) * 128, :], xo[:])

    pg.release()
    ps.release()
    dram.release()


# ==========================================================================
# host side
# ==========================================================================

def _fm(vec):
    """[128*k] f32 vector -> feature-major [128, k] (col j = feature tile j)."""
    v = np.asarray(vec, np.float32)
    return np.ascontiguousarray(v.reshape(-1, 128).T)


def _pair8(W, scale=WS):
    """[K, F] f32 -> DR pair-interleaved fp8 [K/256*128, 2*F], x scale."""
    W = np.asarray(W, np.float32) * scale
    W = np.clip(W, -240.0, 240.0)
    K, F = W.shape
    assert K % 256 == 0
    Wp = W.reshape(K // 256, 2, 128, F).transpose(0, 2, 1, 3).reshape(
        K // 256 * 128, 2 * F)
    return np.ascontiguousarray(Wp).astype(F8)


def _bd16():
    bd = np.zeros((128, 128), np.float32)
    for t in range(8):
        for p_ in range(128):
            bd[p_, t * 16 + 2 * t + p_ // 64] = 1.0
    return bd.astype(BF16)


def _rsel2():
    r = np.zeros((2, 128), np.float32)
    r[0, 0:64] = 1.0
    r[1, 64:128] = 1.0
    return r.astype(BF16)


def _sel(weights64):
    """[16, 1024] selector: sel[i, t*128+p] = w[p%64] * (i == 2t + p//64)."""
    w = np.ones(64, np.float32) if weights64 is None else \
        np.asarray(weights64, np.float32)
    s = np.zeros((16, D), np.float32)
    for col in range(D):
        i = 2 * (col // 128) + (col % 128) // 64
        s[i, col] = w[col % 64]
    return s.astype(BF16)


def make_in_maps(inputs):
    f32 = lambda a: np.ascontiguousarray(np.asarray(a, np.float32))
    bf = lambda a: np.ascontiguousarray(np.asarray(a, np.float32)).astype(BF16)

    x = f32(inputs["x"]); src = f32(inputs["source_tokens"]); c = f32(inputs["c"])
    qkv_b = f32(inputs["sa_qkv_b"])
    o_w = f32(inputs["sa_o_w"]); o_b = f32(inputs["sa_o_b"])
    ckv_b = f32(inputs["ca_kv_b"])
    co_w = f32(inputs["ca_o_w"]); co_b = f32(inputs["ca_o_b"])
    w1 = f32(inputs["mlp_w1"]); b1 = f32(inputs["mlp_b1"])
    w2 = f32(inputs["mlp_w2"]); b2 = f32(inputs["mlp_b2"])
    w3 = f32(inputs["mlp_w3"]); b3 = f32(inputs["mlp_b3"])

    # pad SwiGLU hidden to 2816; zero pads keep silu(0)*0 == 0 exact
    w1p = np.zeros((D, MHP), np.float32); w1p[:, :MH] = w1
    w2p = np.zeros((D, MHP), np.float32); w2p[:, :MH] = w2
    w3p = np.zeros((MHP, D), np.float32); w3p[:MH, :] = w3
    b1p = np.zeros(MHP, np.float32); b1p[:MH] = b1
    b2p = np.zeros(MHP, np.float32); b2p[:MH] = b2

    # fold the V biases through the linear attention + output projection
    obf = qkv_b[2 * D:3 * D] @ o_w + o_b
    cobf = ckv_b[D:2 * D] @ co_w + co_b

    ada_w = f32(inputs["ada_w"])
    shared = dict(
        wqkv=_pair8(inputs["sa_qkv_w"]), wo=_pair8(o_w),
        wcq=_pair8(inputs["ca_q_w"]), wckv=_pair8(inputs["ca_kv_w"]),
        wco=_pair8(co_w),
        w1=_pair8(w1p), w2=_pair8(w2p), w3=_pair8(w3p),
        adab=_fm(f32(inputs["ada_b"])), n1w=_fm(f32(inputs["n1_w"])),
        ncw=_fm(f32(inputs["nc_w"])), n2w=_fm(f32(inputs["n2_w"])),
        qkvb=_fm(qkv_b), obf=_fm(obf), cqb=_fm(f32(inputs["ca_q_b"])),
        ckb=_fm(ckv_b[0:D]), cobf=_fm(cobf),
        b1f=_fm(b1p), b2f=_fm(b2p), b3f=_fm(b3),
        ones128=np.ones((128, 128), BF16),
        bd16=_bd16(),
        qsel=_sel(inputs["sa_qn_w"]), ksel=_sel(inputs["sa_kn_w"]),
        cqsel=_sel(inputs["ca_qn_w"]), cksel=_sel(inputs["ca_kn_w"]),
        rsel2=_rsel2(), eye72=np.eye(NMOD * DT, dtype=np.float32),
        cmat=np.ascontiguousarray(c.T),
    )

    in_maps = []
    for cidx in range(NCORES):
        b, half = divmod(cidx, 2)
        xT = x[b].T  # [D, N]
        if half:
            xTp = np.concatenate([xT[:, T:], xT[:, :T]], axis=1)
        else:
            xTp = xT
        m = dict(shared)
        m["xt"] = np.ascontiguousarray(xTp).astype(BF16)
        m["xres"] = np.ascontiguousarray(xTp[:, :T])
        # local source tokens, fp8 pair-interleaved [512, 1024]
        sl = src[b].T[:, half * T:(half + 1) * T]  # [D, T]
        sl8 = np.clip(sl, -240, 240).reshape(4, 2, 128, T).transpose(
            0, 2, 1, 3).reshape(512, 2 * T)
        m["srcp"] = np.ascontiguousarray(sl8).astype(F8)
        m["adash"] = np.ascontiguousarray(
            ada_w[:, cidx * ASH:(cidx + 1) * ASH]).astype(BF16)
        bs = np.zeros((128, B), np.float32)
        bs[:, b] = 1.0
        m["bsel"] = bs
        in_maps.append(m)
    return in_maps


def assemble(results):
    out = np.empty((B, N, D), np.float32)
    for cidx in range(NCORES):
        b, half = divmod(cidx, 2)
        out[b, half * T:(half + 1) * T, :] = results[cidx]["out"].T
    return out


_NC_CACHE = []


def kernel(**inputs):
    from concourse.bass_utils import run_bass_kernel_spmd
    if not _NC_CACHE:
        _NC_CACHE.append(build_graph())
    nc = _NC_CACHE[0]
    in_maps = make_in_maps(inputs)
    res = run_bass_kernel_spmd(nc, in_maps, core_ids=list(range(NCORES)))
    return assemble(res.results)


if __name__ == "__main__":
    nc = build_graph()
    print("graph built OK; instructions:",
          sum(len(bb.instructions) for bb in nc.main_func.blocks))


# revision 35
# speedup vs baseline: 1.1799x; 1.0010x over previous
"""Trainium2 Bass kernel for nn_ConditionalJiTBlock (DiT-style block with
AdaLN modulation, self-attention, cross-attention and SwiGLU FFN).

Sharding: 8 NeuronCores = 4 batch elements x 2 token-halves. Each core
computes its 512 query tokens end-to-end. v2 additions over the baseline:

- All projection/FFN GEMMs run in fp8(e4m3) with perf_mode=DoubleRow
  (2 MACs/cell/cycle): weights are host-prescaled by 64 (pow2) and stored
  pair-interleaved [K/256*128, 2*F]; activations are written on-chip as
  fp8 "pair tiles" [128, 2*T] (feature tiles 2j/2j+1 side by side), so
  every contraction is 4 DR matmuls of K=256 instead of 8 bf16 matmuls.
  The 1/64 de-scale folds into the PSUM-consuming op (ACT scale / DVE
  scalar / affine_then_add scale).
- The AdaLN mods GEMV is sharded 8 ways: every core computes all 4 batch
  elements' mods over 1/8 of the 9216 columns, then an 8-way AllGather
  (147KB) broadcasts them; a per-core one-hot (bsel) selects the core's
  batch row with 4 vector ops. Kills the 18.9MB ada load + 54us of PE.
- Cross-attention K/V are computed for the LOCAL 512 source tokens only
  and pair-exchanged (AllGather over core pairs, 2.1MB) during the
  self-attention phase: both cores then read back both halves into the
  full K/V tiles (identical layout on both cores, so no per-core
  branching is needed).
- Softmax exp is split across engines: even key-tiles use ScalarE Exp,
  odd key-tiles use a DVE Schraudolph approximation (single tensor_scalar
  writing int16 bf16-bits: bits = rint(s*ATT_SCALE*log2e*128 + 16248)),
  halving the ACT-bound stretches of attention.
- Attention scores (K=64 contraction) are emitted half-pair-interleaved
  so the two 64-row-group matmuls run concurrently in the PE array.

Layout: as the baseline - activations feature-major (features on
partitions, tokens free), per-token scalars broadcast via small selector
matmuls, per-feature scalars as per-partition operands. Residual stream
f32; scores/PV bf16; projections fp8.
"""

import numpy as np
import ml_dtypes

BF16 = ml_dtypes.bfloat16
F8 = ml_dtypes.float8_e4m3

B, N, M, D, H, HD = 4, 1024, 1024, 1024, 16, 64
MH = 2730
MHP = 2816          # MH padded to 22*128
EPS = 1e-6
NCORES = 8
T = 512             # local query tokens per core
DT = D // 128       # 8
KP = DT // 2        # 4 contraction k-pairs for D
FHT = MHP // 128    # 22
FHP = FHT // 2      # 11
NMOD = 9
ASH = NMOD * D // NCORES  # 1152 ada columns per core
ATT_SCALE = HD ** -0.5
WS = 64.0           # fp8 weight pre-scale (pow2)
IWS = 1.0 / WS
LOG2E = 1.4426950408889634
SCHR_A = ATT_SCALE * LOG2E * 128.0
SCHR_B = 16248.0
DVE_KTS = (1, 3, 5, 7)  # key-tiles whose exp runs on DVE (Schraudolph)


# ==========================================================================
# device graph
# ==========================================================================

def build_graph(sim_compat=False):
    import concourse.bacc as bacc
    import concourse.mybir as mybir
    import concourse.tile as tile

    F32 = mybir.dt.float32
    BT = mybir.dt.bfloat16

    nc = bacc.Bacc("TRN2", target_bir_lowering=False, debug=False,
                   num_devices=NCORES)

    def din(name, shape, dtype):
        return nc.dram_tensor(name, shape, dtype, kind="ExternalInput").ap()

    F8D = mybir.dt.float8e4
    p = {}
    # activations
    p["xt"] = din("xt", [D, N], BT)          # x[b].T, local tokens first
    p["xres"] = din("xres", [D, T], F32)     # f32 residual columns (local)
    p["srcp"] = din("srcp", [4 * 128, 2 * T], F8D)  # local src tokens, paired
    p["cmat"] = din("cmat", [D, B], F32)     # c for all batch elements
    p["bsel"] = din("bsel", [128, B], F32)   # one-hot row of this core's b
    p["adash"] = din("adash", [D, ASH], BT)   # ada columns of this core
    # fp8 pair-interleaved weights [K/256*128, 2*F], pre-scaled by WS
    p["wqkv"] = din("wqkv", [512, 2 * 3 * D], F8D)
    p["wo"] = din("wo", [512, 2 * D], F8D)
    p["wcq"] = din("wcq", [512, 2 * D], F8D)
    p["wckv"] = din("wckv", [512, 2 * 2 * D], F8D)
    p["wco"] = din("wco", [512, 2 * D], F8D)
    p["w1"] = din("w1", [512, 2 * MHP], F8D)
    p["w2"] = din("w2", [512, 2 * MHP], F8D)
    p["w3"] = din("w3", [FHP * 128, 2 * D], F8D)
    # feature-major f32 vectors [128, k]  (column j = feature tile j)
    p["adab"] = din("adab", [128, NMOD * DT], F32)
    p["n1w"] = din("n1w", [128, DT], F32)
    p["ncw"] = din("ncw", [128, DT], F32)
    p["n2w"] = din("n2w", [128, DT], F32)
    p["qkvb"] = din("qkvb", [128, 3 * DT], F32)
    p["obf"] = din("obf", [128, DT], F32)    # sa_o_b + v_bias @ Wo (host fold)
    p["cqb"] = din("cqb", [128, DT], F32)
    p["ckb"] = din("ckb", [128, DT], F32)    # cross-k bias
    p["cobf"] = din("cobf", [128, DT], F32)  # ca_o_b + cross-v bias @ Wco
    p["b1f"] = din("b1f", [128, FHT], F32)
    p["b2f"] = din("b2f", [128, FHT], F32)
    p["b3f"] = din("b3f", [128, DT], F32)
    # constant selector matrices, bf16
    p["ones128"] = din("ones128", [128, 128], BT)
    p["bd16"] = din("bd16", [128, 128], BT)
    p["qsel"] = din("qsel", [16, D], BT)
    p["ksel"] = din("ksel", [16, D], BT)
    p["cqsel"] = din("cqsel", [16, D], BT)
    p["cksel"] = din("cksel", [16, D], BT)
    p["rsel2"] = din("rsel2", [2, 128], BT)
    p["eye72"] = din("eye72", [NMOD * DT, NMOD * DT], F32)

    p["out"] = nc.dram_tensor("out", [D, T], F32, kind="ExternalOutput").ap()

    with tile.TileContext(nc) as tc:
        _emit(nc, tc, p, mybir)
    nc.compile()
    return nc


def _emit(nc, tc, p, mybir):
    ALU = mybir.AluOpType
    ACTF = mybir.ActivationFunctionType
    F32 = mybir.dt.float32
    BT = mybir.dt.bfloat16
    F8D = mybir.dt.float8e4
    I16 = mybir.dt.int16
    DR = mybir.MatmulPerfMode.DoubleRow

    pg = tc.alloc_tile_pool(name="pg", bufs=1)
    ps = tc.alloc_tile_pool(name="ps", bufs=8, space="PSUM")
    dram = tc.alloc_tile_pool(name="dram", bufs=1, space="DRAM")

    # shared-tag allocators
    def bigw(name):   # wide bf16 tiles (xt / k / v)
        return pg.tile([128, 1040], BT, tag="bigw", name=name, bufs=26)

    def xf(name):     # f32 [128, T] residual-stream tiles
        return pg.tile([128, T], F32, tag="xf", name=name, bufs=9)

    def qt(name):     # bf16 [128, T] q tiles
        return pg.tile([128, T], BT, tag="qt", name=name, bufs=10)

    def xp(name):     # fp8 pair tiles [128, 2048] (xn1 over N)
        return pg.tile([128, 2 * N], F8D, tag="xp", name=name, bufs=4)

    def sp(name):     # fp8 pair tiles [128, 1024] (T-sized pairs, h, o)
        return pg.tile([128, 2 * T], F8D, tag="sp", name=name, bufs=14)

    def wg8(name):    # fp8 DR weight group tiles [128, 2048]
        return pg.tile([128, 2048], F8D, tag="wg8", name=name, bufs=8)

    def w38(name):    # fp8 DR w3 tiles [128, 1024]
        return pg.tile([128, 1024], F8D, tag="w38", name=name, bufs=11)

    def ptile(name):  # exp(p) tiles
        return pg.tile([128, T], BT, tag="pt", name=name, bufs=12)

    def sqt(name, wid=512):    # square scratch bf16
        return pg.tile([128, wid], BT, tag="sq", name=name, bufs=4)

    def xnb(name):    # bf16 normed-x scratch [128, 1024]
        return pg.tile([128, N], BT, tag="xnb", name=name, bufs=2)

    def scratch4k(name, rows=128, wid=1024):  # f32 scratch (rr/ssq/den)
        return pg.tile([rows, wid], F32, tag="s4k", name=name, bufs=1)

    def scrbf(name, rows=16, wid=1024):
        return pg.tile([rows, wid], BT, tag="sbf", name=name, bufs=1)

    def psum(name):
        return ps.tile([128, 512], F32, tag="ps_n", name=name, bufs=8)

    # ---------------- PE warmup + early ada loads ----------------
    warm = pg.tile([128, 2], BT, tag="warm", name="warm")
    nc.vector.memset(warm[:], 1.0)
    wps = ps.tile([128, 512], F32, tag="ps_n", name="warm_ps")
    for i in range(80):
        nc.tensor.matmul(wps[0:1, 0:1], warm[:, 0:1], warm[:, 1:2],
                         start=True, stop=True, skip_group_check=True)
    cv = pg.tile([128, DT * B], F32, tag="cv", name="cv")
    nc.sync.dma_start(cv[:].rearrange("p (k b) -> p k b", k=DT),
                      p["cmat"][:].rearrange("(k p) b -> p k b", p=128))
    adat = []
    for k in range(DT):
        t = pg.tile([128, ASH], BT, tag="adat", name=f"adat{k}", bufs=DT)
        nc.sync.dma_start(t[:], p["adash"][k * 128:(k + 1) * 128, :])
        adat.append(t)

    # ---------------- constants ----------------
    cst = {}
    c_eps = pg.tile([128, 1], F32, tag="c_eps", name="c_eps")
    nc.vector.memset(c_eps[:], EPS)
    for nm, k in (("ones128", 128), ("bd16", 128)):
        t = pg.tile([128, k], BT, tag=nm, name=f"c_{nm}")
        nc.sync.dma_start(t[:], p[nm][:])
        cst[nm] = t
    for nm in ("qsel", "ksel", "cqsel", "cksel"):
        t = pg.tile([16, D], BT, tag=nm, name=f"c_{nm}")
        nc.sync.dma_start(t[:], p[nm][:])
        cst[nm] = t
    t = pg.tile([2, 128], BT, tag="rsel2", name="c_rsel2")
    nc.sync.dma_start(t[:], p["rsel2"][:])
    cst["rsel2"] = t
    t = pg.tile([NMOD * DT, NMOD * DT], F32, tag="eye72", name="c_eye72")
    nc.sync.dma_start(t[:], p["eye72"][:])
    cst["eye72"] = t
    for nm, k in (("adab", NMOD * DT), ("n1w", DT), ("ncw", DT), ("n2w", DT),
                  ("qkvb", 3 * DT), ("obf", DT), ("cqb", DT), ("ckb", DT),
                  ("cobf", DT), ("b1f", FHT), ("b2f", FHT), ("b3f", DT),
                  ("bsel", B)):
        t = pg.tile([128, k], F32, tag=nm, name=f"c_{nm}")
        nc.sync.dma_start(t[:], p[nm][:])
        cst[nm] = t

    # =====================================================================
    # Stage 0a: sharded ada GEMV + 8-way AllGather of mods.
    # Every core computes mods[all 4 b, its 1152 columns].
    # =====================================================================
    scs = pg.tile([128, DT * B], BT, tag="sc", name="scs")
    nc.scalar.activation(scs[:], cv[:], ACTF.Sigmoid)
    nc.vector.tensor_tensor(scs[:], scs[:], cv[:], ALU.mult)

    strip = pg.tile([4, ASH], F32, tag="strip", name="strip")
    for ch in range(3):  # 3 chunks of 384 columns
        pm = psum(f"pm{ch}")
        for k in range(DT):
            nc.tensor.matmul(pm[0:4, 0:384], scs[:, k * B:(k + 1) * B],
                             adat[k][:, ch * 384:(ch + 1) * 384],
                             start=(k == 0), stop=(k == DT - 1))
        nc.vector.tensor_copy(strip[:, ch * 384:(ch + 1) * 384],
                                pm[0:4, 0:384])

    agin = dram.tile([B * ASH], F32, tag="agin", name="agin")
    ago = dram.tile([NCORES * B * ASH], F32, tag="ago", name="ago",
                    addr_space="Shared")
    nc.gpsimd.dma_start(agin[:].rearrange("(g j) -> g j", g=4), strip[:])
    nc.gpsimd.collective_compute(
        "AllGather", ALU.bypass, replica_groups=[list(range(NCORES))],
        ins=[agin[:]], outs=[ago[:]])

    sh_col = {"sa": 0, "ca": 3, "ff": 6}

    # =====================================================================
    # helpers
    # =====================================================================
    def load_wp(w_ap, cols0, cols, tagname, alloc=wg8):
        """Load DR weight tiles: per k-pair a [128, 2*cols] tile."""
        nkp = w_ap.shape[0] // 128
        tiles = []
        for kp in range(nkp):
            t = alloc(f"{tagname}_{kp}")
            nc.sync.dma_start(
                t[:, 0:2 * cols].rearrange("p (ko m) -> p ko m", ko=2),
                w_ap[kp * 128:(kp + 1) * 128, :]
                .rearrange("p (ko m) -> p ko m", ko=2)[:, :, cols0:cols0 + cols])
            tiles.append(t)
        return tiles

    def norm_mod(xtiles, Ttok, seff_t, sh_slice, name, sq_engine):
        """RMS + AdaLN modulate of feature-major tiles -> fp8 pair tiles."""
        NCH = Ttok // 512
        pss = [psum(f"ssn_{name}{c}") for c in range(NCH)]
        for k in range(DT):
            for c in range(NCH):
                sq = sqt(f"sq_{name}{k}_{c}")
                nc.scalar.activation(sq[:], xtiles[k][:, c * 512:(c + 1) * 512],
                                     ACTF.Square)
                nc.tensor.matmul(pss[c][:], cst["ones128"][:], sq[:],
                                 start=(k == 0), stop=(k == DT - 1))
        rr = scratch4k(f"rr_{name}")
        for c in range(NCH):
            nc.scalar.activation(rr[:, c * 512:(c + 1) * 512], pss[c][:],
                                 ACTF.Sqrt, bias=c_eps[:], scale=1.0 / D)
        nc.vector.reciprocal_approx_fast(rr[:, 0:Ttok], rr[:, 0:Ttok])
        alloc = xp if Ttok == N else sp
        xn = [alloc(f"xn_{name}{j}") for j in range(KP)]
        for k in range(DT):
            t1 = xnb(f"xnb_{name}{k}")
            nc.vector.tensor_tensor(t1[:, 0:Ttok], xtiles[k][:, 0:Ttok],
                                    rr[:, 0:Ttok], ALU.mult)
            half = xn[k // 2][:, (k % 2) * Ttok:(k % 2 + 1) * Ttok]
            nc.vector.tensor_scalar(half, t1[:, 0:Ttok],
                                    seff_t[:, k:k + 1], sh_slice[:, k:k + 1],
                                    ALU.mult, ALU.add)
        return xn

    def qk_norm(qtiles, Ttok, selname, name, sq_eng="gpsimd"):
        """Per-head RMS norm in place; head-norm weight folded into sel."""
        NCH = Ttok // 512
        ssq = scratch4k(f"ssq_{name}", rows=16)
        eng = nc.gpsimd if sq_eng == "gpsimd" else nc.vector
        for c in range(NCH):
            pq = psum(f"psq_{name}{c}")
            for t in range(DT):
                sq = sqt(f"qs_{name}{t}_{c}")
                eng.tensor_tensor(sq[:], qtiles[t][:, c * 512:(c + 1) * 512],
                                  qtiles[t][:, c * 512:(c + 1) * 512],
                                  ALU.mult)
                nc.tensor.matmul(pq[0:16, :],
                                 cst["bd16"][:, t * 16:(t + 1) * 16], sq[:],
                                 start=(t == 0), stop=(t == DT - 1))
            nc.scalar.activation(ssq[:, c * 512:(c + 1) * 512], pq[0:16, :],
                                 ACTF.Sqrt, bias=c_eps[0:16, :], scale=1.0 / HD)
        nc.vector.reciprocal_approx_fast(ssq[:, 0:Ttok], ssq[:, 0:Ttok])
        rqb = scrbf(f"rqb_{name}")
        nc.scalar.activation(rqb[:, 0:Ttok], ssq[:, 0:Ttok], ACTF.Copy)
        for t in range(DT):
            for c in range(NCH):
                pb = psum(f"qb_{name}{t}_{c}")
                nc.tensor.matmul(pb[:], cst[selname][:, t * 128:(t + 1) * 128],
                                 rqb[:, c * 512:(c + 1) * 512],
                                 start=True, stop=True)
                nc.vector.tensor_tensor(qtiles[t][:, c * 512:(c + 1) * 512],
                                        qtiles[t][:, c * 512:(c + 1) * 512],
                                        pb[:], ALU.mult)

    def attention(q_sb, k_sb, v_sb, Tk, name, o_pair):
        """softmax(q k^T / 8) v. Scores are emitted half-pair interleaved
        (concurrent 64-row-group matmuls); exp alternates ACT/DVE per
        (kt, half) so both engines run every step. PV trails two kt steps
        so its operands are always ready and the PE streams back-to-back.
        Per-pair denominator handling (recip + K=2 broadcast matmul) and
        the 1/den scaling writes the fp8 o_pair halves straight from PSUM."""
        KTk = Tk // 128
        KT2 = KTk // 2

        deferred = [None]

        def do_pair(t):
            po = [psum(f"po_{name}{2 * t}"), psum(f"po_{name}{2 * t + 1}")]
            pipe = []

            def pv(kt):
                for half in range(2):
                    h16 = 2 * t + half
                    nc.tensor.matmul(
                        po[half][0:65, :],
                        v_sb[kt][:, h16 * 65:(h16 + 1) * 65],
                        pipe[kt][half][:],
                        start=(kt == 0), stop=(kt == KTk - 1),
                        skip_group_check=True)

            for kt in range(KTk):
                sps = []
                for half in range(2):
                    lo = 64 * half
                    s_ps = psum(f"s_{name}{2 * t + half}_{kt}")
                    nc.tensor.matmul(
                        s_ps[:], k_sb[t][lo:lo + 64, kt * 128:(kt + 1) * 128],
                        q_sb[t][lo:lo + 64, 0:T], start=True, stop=True)
                    sps.append(s_ps)
                cur = []
                for half in range(2):
                    h16 = 2 * t + half
                    pt = ptile(f"pt_{name}{h16}_{kt}")
                    if (kt + half) % 2 == 1:
                        nc.vector.tensor_scalar(pt[:].bitcast(I16),
                                                sps[half][:],
                                                SCHR_A, SCHR_B,
                                                ALU.mult, ALU.add)
                    else:
                        nc.scalar.activation(pt[:], sps[half][:], ACTF.Exp,
                                             scale=ATT_SCALE)
                    cur.append(pt)
                pipe.append(cur)
                if kt >= 2:
                    pv(kt - 2)
                if kt == 4 and deferred[0] is not None:
                    deferred[0]()
                    deferred[0] = None
            pv(KTk - 2)
            pv(KTk - 1)
            # denominator: 1/row64 -> [2,T] base-0 tile (via Act-queue DMA;
            # engine ops need 32-aligned partition bases). The broadcast +
            # o_pair write is deferred into the next pair's matmul stream so
            # the chain latency never stalls the PE.
            drow = pg.tile([2, T], F32, tag="drow", name=f"dr_{name}{t}", bufs=2)
            rdb2 = pg.tile([2, T], BT, tag="rdb2", name=f"rb_{name}{t}", bufs=2)
            osb = qt(f"o_{name}{t}")
            for half in range(2):
                ds = pg.tile([1, T], F32, tag="dstr", name=f"ds_{name}{t}_{half}",
                             bufs=3)
                nc.vector.tensor_copy(ds[:], po[half][64:65, :])
                nc.scalar.dma_start(drow[half:half + 1, :], ds[:])
                nc.scalar.activation(osb[64 * half:64 * half + 64, :],
                                     po[half][0:64, :], ACTF.Copy)

            def tail(t=t, drow=drow, rdb2=rdb2, osb=osb):
                nc.vector.reciprocal_approx_fast(drow[:], drow[:])
                nc.scalar.activation(rdb2[:], drow[:], ACTF.Copy)
                pb = psum(f"ob_{name}{t}")
                nc.tensor.matmul(pb[:], cst["rsel2"][:], rdb2[:],
                                 start=True, stop=True)
                nc.vector.tensor_tensor(
                    o_pair[t // 2][:, (t % 2) * T:(t % 2 + 1) * T],
                    osb[:], pb[:], ALU.mult)

            deferred[0] = tail

        for t in range(DT):
            do_pair(t)
        deferred[0]()
        return o_pair

    def proj_dr(wap, wcols0, xnp, Tt, name, n_f=DT, nkp=KP,
                consume=None):
        """Feature-major DR projection: out f-tiles via 4 K=256 matmuls.
        `consume(f, c, pp)` turns each PSUM chunk into SBUF."""
        NCH = Tt // 512
        for f0 in range(0, n_f, 8):
            nf = min(8, n_f - f0)
            wt = load_wp(wap, wcols0 + f0 * 128, nf * 128, f"{name}_w{f0}")
            for f in range(nf):
                pps = [psum(f"p_{name}{f0 + f}_{c}") for c in range(NCH)]
                for kp in range(nkp):
                    for c in range(NCH):
                        nc.tensor.matmul(
                            pps[c][:],
                            wt[kp][:, 0:2 * nf * 128].rearrange(
                                "p (ko m) -> p ko m", ko=2)[:, :, f * 128:(f + 1) * 128],
                            xnp[kp][:, 0:2 * Tt].rearrange(
                                "p (ko n) -> p ko n", ko=2)[:, :, c * 512:(c + 1) * 512],
                            start=(kp == 0), stop=(kp == nkp - 1),
                            perf_mode=DR, skip_group_check=True)
                for c in range(NCH):
                    consume(f0 + f, c, pps[c])

    def proj_tok_dr(wap, wcols0, xnp, Tt, name, outs):
        """Token-major V projection (DR): stationary = xn pair slices."""
        ntt = Tt // 128
        wt = load_wp(wap, wcols0, D, f"{name}_w")
        for tt in range(ntt):
            pps = [psum(f"pv_{name}{tt}_{c}") for c in range(2)]
            for kp in range(KP):
                for c in range(2):
                    nc.tensor.matmul(
                        pps[c][:],
                        xnp[kp][:, 0:2 * Tt].rearrange(
                            "p (ko n) -> p ko n", ko=2)[:, :, tt * 128:(tt + 1) * 128],
                        wt[kp][:, 0:2 * D].rearrange(
                            "p (ko m) -> p ko m", ko=2)[:, :, c * 512:(c + 1) * 512],
                        start=(kp == 0), stop=(kp == KP - 1),
                        perf_mode=DR, skip_group_check=True)
            for c in range(2):
                dst = outs[tt][:, c * 8 * 65:(c * 8 + 8) * 65].rearrange(
                    "p (g e) -> p g e", g=8)[:, :, 0:64]
                nc.scalar.activation(dst, pps[c][:].rearrange("p (g e) -> p g e", g=8),
                                     ACTF.Copy, scale=IWS)

    # =====================================================================
    # Stage 0b: local cross-attention K/V from the local 512 source
    # tokens, then pair AllGather; both halves are read back into the
    # full-width tiles (same layout on both cores of a pair).
    # =====================================================================
    srcp = []
    for kp in range(KP):
        t = pg.tile([128, 2 * T], F8D, tag="srcp", name=f"srcp{kp}", bufs=KP)
        nc.sync.dma_start(t[:], p["srcp"][kp * 128:(kp + 1) * 128, :])
        srcp.append(t)

    kcaL = [qt(f"kcaL{f}") for f in range(DT)]

    def ckv_consume(f, c, pp):
        nc.scalar.activation(kcaL[f][:], pp[:], ACTF.Identity,
                             bias=cst["ckb"][:, f:f + 1], scale=IWS)

    proj_dr(p["wckv"], 0, srcp, T, "kca", consume=ckv_consume)
    vcaL = []
    for tt in range(T // 128):
        o = bigw(f"vcaL{tt}")
        nc.vector.memset(o[:], 1.0)
        vcaL.append(o)
    proj_tok_dr(p["wckv"], D, srcp, T, "vca", vcaL)
    qk_norm(kcaL, T, "cksel", "kca", sq_eng="vector")

    KBYTES = 128 * 512
    VBYTES = 128 * 1040
    kvin = dram.tile([DT * KBYTES + 4 * VBYTES], BT, tag="kvin", name="kvin")
    kvout = dram.tile([2 * (DT * KBYTES + 4 * VBYTES)], BT, tag="kvout",
                      name="kvout")
    for f in range(DT):
        nc.scalar.dma_start(
            kvin[f * KBYTES:(f + 1) * KBYTES].rearrange("(p n) -> p n", p=128),
            kcaL[f][:, 0:512])
    for tt in range(4):
        nc.scalar.dma_start(
            kvin[DT * KBYTES + tt * VBYTES:DT * KBYTES + (tt + 1) * VBYTES]
            .rearrange("(p n) -> p n", p=128), vcaL[tt][:])
    nc.gpsimd.collective_compute(
        "AllGather", ALU.bypass,
        replica_groups=[[2 * i, 2 * i + 1] for i in range(B)],
        ins=[kvin[:]], outs=[kvout[:]])

    # Gather read-back in transposed layout (contiguous 512B runs, cheap
    # gpsimd descriptors), batch-select there, then transpose back with one
    # tiny f32 matmul against the identity.
    NM = NMOD * DT
    mall_T = pg.tile([NM, B * 128], F32, tag="mallT", name="mall_T")
    ago3 = ago[:].rearrange("(c g q p) -> c q g p", c=NCORES, g=B, p=128)
    for cc in range(NCORES):
        nc.gpsimd.dma_start(
            mall_T[cc * NMOD:(cc + 1) * NMOD, :]
            .rearrange("q (g pp) -> q g pp", g=B),
            ago3[cc])
    maT3 = mall_T[:].rearrange("q (g pp) -> q g pp", g=B)

    modsT = pg.tile([NM, 128], F32, tag="modsT", name="modsT")
    nc.vector.tensor_scalar(modsT[:], maT3[:, 0], cst["bsel"][0:NM, 0:1],
                            None, ALU.mult)
    for g in range(1, B):
        nc.vector.scalar_tensor_tensor(modsT[:], maT3[:, g],
                                       cst["bsel"][0:NM, g:g + 1], modsT[:],
                                       ALU.mult, ALU.add)
    pmT = psum("pmT")
    nc.tensor.matmul(pmT[:, 0:NM], modsT[:], cst["eye72"][:],
                     start=True, stop=True)
    mods = pg.tile([128, NM], F32, tag="mods", name="mods")
    nc.vector.tensor_tensor(mods[:], pmT[:, 0:NM], cst["adab"][:], ALU.add)

    def msl(i):  # mods columns of modulation param i
        return mods[:, i * DT:(i + 1) * DT]

    seff = {}
    for nm, i_scale, w in (("sa", 1, "n1w"), ("ca", 4, "ncw"), ("ff", 7, "n2w")):
        s1 = pg.tile([128, DT], F32, tag=f"seff_{nm}", name=f"seff_{nm}")
        nc.vector.tensor_scalar(s1[:], msl(i_scale), 1.0, None, ALU.add)
        nc.vector.tensor_tensor(s1[:], s1[:], cst[w][:], ALU.mult)
        seff[nm] = s1
    gb = {}
    g64 = {}
    for nm, i_gate, bias in (("sa", 2, "obf"), ("ca", 5, "cobf"), ("ff", 8, "b3f")):
        t = pg.tile([128, DT], F32, tag=f"gb_{nm}", name=f"gb_{nm}")
        nc.vector.tensor_tensor(t[:], msl(i_gate), cst[bias][:], ALU.mult)
        gb[nm] = t
        t2 = pg.tile([128, DT], F32, tag=f"g64_{nm}", name=f"g64_{nm}")
        nc.vector.tensor_scalar(t2[:], msl(i_gate), IWS, None, ALU.mult)
        g64[nm] = t2
    # =====================================================================
    # Stage 1: self-attention sublayer
    # =====================================================================
    xt_sb = []
    for k in range(DT):
        t = bigw(f"xt{k}")
        nc.sync.dma_start(t[:, 0:N], p["xt"][k * 128:(k + 1) * 128, :])
        xt_sb.append(t)
    xres_sb = []
    for k in range(DT):
        t = xf(f"xres{k}")
        nc.sync.dma_start(t[:], p["xres"][k * 128:(k + 1) * 128, :])
        xres_sb.append(t)

    xn1 = norm_mod(xt_sb, N, seff["sa"], msl(sh_col["sa"]), "n1", "scalar")
    q_sa = [qt(f"qsa{f}") for f in range(DT)]

    def q_consume(f, c, pp):
        nc.scalar.activation(q_sa[f][:], pp[:], ACTF.Identity,
                             bias=cst["qkvb"][:, f:f + 1], scale=IWS)

    k_sa = [bigw(f"ksa{f}") for f in range(DT)]

    def k_consume(f, c, pp):
        nc.scalar.activation(k_sa[f][:, c * 512:(c + 1) * 512], pp[:],
                             ACTF.Identity,
                             bias=cst["qkvb"][:, DT + f:DT + f + 1], scale=IWS)

    proj_dr(p["wqkv"], 0, xn1, T, "qsa", consume=q_consume)
    qk_norm(q_sa, T, "qsel", "qsa", sq_eng="vector")
    proj_dr(p["wqkv"], D, xn1, N, "ksa", consume=k_consume)
    qk_norm(k_sa, N, "ksel", "ksa", sq_eng="gpsimd")
    v_sa = []
    for tt in range(N // 128):
        o = bigw(f"vsa{tt}")
        nc.vector.memset(o[:], 1.0)
        v_sa.append(o)
    proj_tok_dr(p["wqkv"], 2 * D, xn1, N, "vsa", v_sa)

    o1p = [sp(f"o1p{j}") for j in range(KP)]
    attention(q_sa, k_sa, v_sa, N, "a1", o1p)

    x1 = xres_sb

    def o1_consume(f, c, pp):
        nc.vector.affine_then_add(x1[f][:], pp[:], x1[f][:],
                                  g64["sa"][:, f:f + 1], gb["sa"][:, f:f + 1])

    proj_dr(p["wo"], 0, o1p, T, "o1", consume=o1_consume)

    kca = [bigw(f"kca{f}") for f in range(DT)]
    vca = [bigw(f"vca{tt}") for tt in range(8)]
    HALF_OFF = DT * KBYTES + 4 * VBYTES
    for h in range(2):
        for f in range(DT):
            o = h * HALF_OFF + f * KBYTES
            nc.sync.dma_start(
                kca[f][:, h * 512:(h + 1) * 512],
                kvout[o:o + KBYTES].rearrange("(p n) -> p n", p=128))
        for tt in range(4):
            o = h * HALF_OFF + DT * KBYTES + tt * VBYTES
            nc.sync.dma_start(
                vca[h * 4 + tt][:, 0:1040],
                kvout[o:o + VBYTES].rearrange("(p n) -> p n", p=128))

    # =====================================================================
    # Stage 2: cross-attention sublayer
    # =====================================================================
    xnc = norm_mod(x1, T, seff["ca"], msl(sh_col["ca"]), "nc", "scalar")
    q_ca = [qt(f"qca{f}") for f in range(DT)]

    def qca_consume(f, c, pp):
        nc.scalar.activation(q_ca[f][:], pp[:], ACTF.Identity,
                             bias=cst["cqb"][:, f:f + 1], scale=IWS)

    proj_dr(p["wcq"], 0, xnc, T, "qca", consume=qca_consume)
    qk_norm(q_ca, T, "cqsel", "qca", sq_eng="gpsimd")
    o2p = [sp(f"o2p{j}") for j in range(KP)]
    attention(q_ca, kca, vca, M, "a2", o2p)

    x2 = x1

    def o2_consume(f, c, pp):
        nc.vector.affine_then_add(x2[f][:], pp[:], x2[f][:],
                                  g64["ca"][:, f:f + 1], gb["ca"][:, f:f + 1])

    proj_dr(p["wco"], 0, o2p, T, "o2", consume=o2_consume)

    # =====================================================================
    # Stage 3: SwiGLU FFN sublayer
    # =====================================================================
    xn2 = norm_mod(x2, T, seff["ff"], msl(sh_col["ff"]), "n2", "scalar")
    h_pair = [sp(f"hp{j}") for j in range(FHP)]
    for f0 in range(0, FHT, 8):
        nf = min(8, FHT - f0)
        w1t = load_wp(p["w1"], f0 * 128, nf * 128, f"w1_{f0}")
        w2t = load_wp(p["w2"], f0 * 128, nf * 128, f"w2_{f0}")
        for f in range(nf):
            fi = f0 + f
            pp1 = psum(f"ph1_{fi}")
            for kp in range(KP):
                nc.tensor.matmul(
                    pp1[:],
                    w1t[kp][:, 0:2 * nf * 128].rearrange(
                        "p (ko m) -> p ko m", ko=2)[:, :, f * 128:(f + 1) * 128],
                    xn2[kp][:].rearrange("p (ko n) -> p ko n", ko=2),
                    start=(kp == 0), stop=(kp == KP - 1), perf_mode=DR)
            h1 = sqt(f"h1_{fi}")
            nc.scalar.activation(h1[:], pp1[:], ACTF.Silu,
                                 bias=cst["b1f"][:, fi:fi + 1], scale=IWS)
            pp2 = psum(f"ph2_{fi}")
            for kp in range(KP):
                nc.tensor.matmul(
                    pp2[:],
                    w2t[kp][:, 0:2 * nf * 128].rearrange(
                        "p (ko m) -> p ko m", ko=2)[:, :, f * 128:(f + 1) * 128],
                    xn2[kp][:].rearrange("p (ko n) -> p ko n", ko=2),
                    start=(kp == 0), stop=(kp == KP - 1), perf_mode=DR)
            h2 = sqt(f"h2_{fi}")
            nc.vector.tensor_scalar(h2[:], pp2[:], IWS,
                                    cst["b2f"][:, fi:fi + 1],
                                    ALU.mult, ALU.add)
            nc.vector.tensor_tensor(
                h_pair[fi // 2][:, (fi % 2) * T:(fi % 2 + 1) * T],
                h1[:], h2[:], ALU.mult)

    # out = h @ w3: 2 groups of 4 feature tiles, 4 live psums each
    for fg in range(0, DT, 4):
        psf = [psum(f"pf{fg + f}") for f in range(4)]
        for kp in range(FHP):
            w3t = w38(f"w3_{fg}_{kp}")
            nc.sync.dma_start(
                w3t[:].rearrange("p (ko m) -> p ko m", ko=2),
                p["w3"][kp * 128:(kp + 1) * 128, :]
                .rearrange("p (ko m) -> p ko m", ko=2)[:, :, fg * 128:(fg + 4) * 128])
            for f in range(4):
                nc.tensor.matmul(
                    psf[f][:],
                    w3t[:].rearrange("p (ko m) -> p ko m", ko=2)[:, :, f * 128:(f + 1) * 128],
                    h_pair[kp][:].rearrange("p (ko n) -> p ko n", ko=2),
                    start=(kp == 0), stop=(kp == FHP - 1), perf_mode=DR)
        for f in range(4):
            xo = x2[fg + f]
            nc.vector.affine_then_add(
                xo[:], psf[f][:], xo[:],
                g64["ff"][:, fg + f:fg + f + 1],
                gb["ff"][:, fg + f:fg + f + 1])
            nc.sync.dma_start(p["out"][(fg + f) * 128:(fg + f + 1) * 128, :], xo[:])

    pg.release()
    ps.release()
    dram.release()


# ==========================================================================
# host side
# ==========================================================================

def _fm(vec):
    """[128*k] f32 vector -> feature-major [128, k] (col j = feature tile j)."""
    v = np.asarray(vec, np.float32)
    return np.ascontiguousarray(v.reshape(-1, 128).T)


def _pair8(W, scale=WS):
    """[K, F] f32 -> DR pair-interleaved fp8 [K/256*128, 2*F], x scale."""
    W = np.asarray(W, np.float32) * scale
    W = np.clip(W, -240.0, 240.0)
    K, F = W.shape
    assert K % 256 == 0
    Wp = W.reshape(K // 256, 2, 128, F).transpose(0, 2, 1, 3).reshape(
        K // 256 * 128, 2 * F)
    return np.ascontiguousarray(Wp).astype(F8)


def _bd16():
    bd = np.zeros((128, 128), np.float32)
    for t in range(8):
        for p_ in range(128):
            bd[p_, t * 16 + 2 * t + p_ // 64] = 1.0
    return bd.astype(BF16)


def _rsel2():
    r = np.zeros((2, 128), np.float32)
    r[0, 0:64] = 1.0
    r[1, 64:128] = 1.0
    return r.astype(BF16)


def _sel(weights64):
    """[16, 1024] selector: sel[i, t*128+p] = w[p%64] * (i == 2t + p//64)."""
    w = np.ones(64, np.float32) if weights64 is None else \
        np.asarray(weights64, np.float32)
    s = np.zeros((16, D), np.float32)
    for col in range(D):
        i = 2 * (col // 128) + (col % 128) // 64
        s[i, col] = w[col % 64]
    return s.astype(BF16)


def make_in_maps(inputs):
    f32 = lambda a: np.ascontiguousarray(np.asarray(a, np.float32))
    bf = lambda a: np.ascontiguousarray(np.asarray(a, np.float32)).astype(BF16)

    x = f32(inputs["x"]); src = f32(inputs["source_tokens"]); c = f32(inputs["c"])
    qkv_b = f32(inputs["sa_qkv_b"])
    o_w = f32(inputs["sa_o_w"]); o_b = f32(inputs["sa_o_b"])
    ckv_b = f32(inputs["ca_kv_b"])
    co_w = f32(inputs["ca_o_w"]); co_b = f32(inputs["ca_o_b"])
    w1 = f32(inputs["mlp_w1"]); b1 = f32(inputs["mlp_b1"])
    w2 = f32(inputs["mlp_w2"]); b2 = f32(inputs["mlp_b2"])
    w3 = f32(inputs["mlp_w3"]); b3 = f32(inputs["mlp_b3"])

    # pad SwiGLU hidden to 2816; zero pads keep silu(0)*0 == 0 exact
    w1p = np.zeros((D, MHP), np.float32); w1p[:, :MH] = w1
    w2p = np.zeros((D, MHP), np.float32); w2p[:, :MH] = w2
    w3p = np.zeros((MHP, D), np.float32); w3p[:MH, :] = w3
    b1p = np.zeros(MHP, np.float32); b1p[:MH] = b1
    b2p = np.zeros(MHP, np.float32); b2p[:MH] = b2

    # fold the V biases through the linear attention + output projection
    obf = qkv_b[2 * D:3 * D] @ o_w + o_b
    cobf = ckv_b[D:2 * D] @ co_w + co_b

    ada_w = f32(inputs["ada_w"])
    shared = dict(
        wqkv=_pair8(inputs["sa_qkv_w"]), wo=_pair8(o_w),
        wcq=_pair8(inputs["ca_q_w"]), wckv=_pair8(inputs["ca_kv_w"]),
        wco=_pair8(co_w),
        w1=_pair8(w1p), w2=_pair8(w2p), w3=_pair8(w3p),
        adab=_fm(f32(inputs["ada_b"])), n1w=_fm(f32(inputs["n1_w"])),
        ncw=_fm(f32(inputs["nc_w"])), n2w=_fm(f32(inputs["n2_w"])),
        qkvb=_fm(qkv_b), obf=_fm(obf), cqb=_fm(f32(inputs["ca_q_b"])),
        ckb=_fm(ckv_b[0:D]), cobf=_fm(cobf),
        b1f=_fm(b1p), b2f=_fm(b2p), b3f=_fm(b3),
        ones128=np.ones((128, 128), BF16),
        bd16=_bd16(),
        qsel=_sel(inputs["sa_qn_w"]), ksel=_sel(inputs["sa_kn_w"]),
        cqsel=_sel(inputs["ca_qn_w"]), cksel=_sel(inputs["ca_kn_w"]),
        rsel2=_rsel2(), eye72=np.eye(NMOD * DT, dtype=np.float32),
        cmat=np.ascontiguousarray(c.T),
    )

    in_maps = []
    for cidx in range(NCORES):
        b, half = divmod(cidx, 2)
        xT = x[b].T  # [D, N]
        if half:
            xTp = np.concatenate([xT[:, T:], xT[:, :T]], axis=1)
        else:
            xTp = xT
        m = dict(shared)
        m["xt"] = np.ascontiguousarray(xTp).astype(BF16)
        m["xres"] = np.ascontiguousarray(xTp[:, :T])
        # local source tokens, fp8 pair-interleaved [512, 1024]
        sl = src[b].T[:, half * T:(half + 1) * T]  # [D, T]
        sl8 = np.clip(sl, -240, 240).reshape(4, 2, 128, T).transpose(
            0, 2, 1, 3).reshape(512, 2 * T)
        m["srcp"] = np.ascontiguousarray(sl8).astype(F8)
        m["adash"] = np.ascontiguousarray(
            ada_w[:, cidx * ASH:(cidx + 1) * ASH]).astype(BF16)
        bs = np.zeros((128, B), np.float32)
        bs[:, b] = 1.0
        m["bsel"] = bs
        in_maps.append(m)
    return in_maps


def assemble(results):
    out = np.empty((B, N, D), np.float32)
    for cidx in range(NCORES):
        b, half = divmod(cidx, 2)
        out[b, half * T:(half + 1) * T, :] = results[cidx]["out"].T
    return out


_NC_CACHE = []


def kernel(**inputs):
    from concourse.bass_utils import run_bass_kernel_spmd
    if not _NC_CACHE:
        _NC_CACHE.append(build_graph())
    nc = _NC_CACHE[0]
    in_maps = make_in_maps(inputs)
    res = run_bass_kernel_spmd(nc, in_maps, core_ids=list(range(NCORES)))
    return assemble(res.results)


if __name__ == "__main__":
    nc = build_graph()
    print("graph built OK; instructions:",
          sum(len(bb.instructions) for bb in nc.main_func.blocks))
